# revision 1
# baseline (speedup 1.0000x reference)
"""Trainium2 Bass kernel for nn_DepthPrediction (multi-view stereo depth).

Strategy (8 NeuronCores, SPMD single program):
  - Shard: core k handles batch b = k//4 and depth planes [8*(k%4), 8*(k%4)+8).
  - Per (b,d): homography warp of 2 src views via on-device fp16 "patch maps"
    (132x132 positions x [dy2,dx2,c16] fp16 chunks = 256B) gathered with one
    indirect-DMA descriptor per pixel; bilinear interp + cumulative cost
    (L2 over 16 ch) in pixel-major fp16 on DVE; 5x5 adaptive aggregation
    (depth-similarity softmax x feature-similarity weight) in tap-major
    layout; per-core softmax partials over the 8 local planes.
  - Engine budget: DVE does the packed-fp16 tensor math; Activation does
    affine/exp/ln/square/casts (single act table: natural_log_exp);
    Pool does gathers + memsets; PE folds gather indices; SP does DMAs.
  - Host: trivial glue — 4x4 matrix algebra, shard/pack inputs, combine the
    4-way softmax partials per batch (log-sum-exp merge) into the output.

Self-contained: hardcodes all shapes from the problem spec.
"""

import numpy as np

import concourse.bacc as bacc
import concourse.bass as bass
import concourse.mybir as mybir
import concourse.tile as tile
from concourse.bass_utils import run_bass_kernel_spmd
from concourse.tile_rust import add_dep_helper

F32 = mybir.dt.float32
F16 = mybir.dt.float16
I32 = mybir.dt.int32
I16 = mybir.dt.int16

AF = mybir.ActivationFunctionType
OP = mybir.AluOpType
AX = mybir.AxisListType

# problem shapes
V, B, C, H, W, D = 3, 2, 16, 128, 128, 32
NCORES = 8
DPC = D // (NCORES // B)  # depth planes per core = 8
NV = V - 1  # src views = 2

PW = W + 4          # padded map width (x0 in [-2,129] -> cols 0..131)
POS = PW * PW       # patch positions
CH = 128            # chunk elems: [dy2,dx2,c16]+pad64 fp16 = 256B
PADX = W + 4        # x-padded tiles for 5-tap aggregation
SCALE = W / (W - 1.0)  # grid_sample align_corners=False fold
ESHIFT = -9.0       # exp(|dnb-d|) stabilization shift (|delta| <= 9)
NT = 25             # aggregation taps
TM = NT * W         # tap-major volume free size

_cached = {}


def _ap(base, off, dims):
    """Raw AP on the same tensor as `base` (an AP), offset in elements."""
    return bass.AP(base.tensor, base.offset + off, dims)


def _prefer_ln_exp_table(arch):
    """Reorder the cached act-table dict so the one table containing ALL the
    funcs this kernel uses (exp, ln, identity, copy, abs, square) is picked
    first -> a single LoadActFuncSet instead of per-op thrash."""
    try:
        from concourse.hw_specs import get_activation_tables
        t = get_activation_tables(arch)
        key = "natural_log_exp_and_others"
        if key in t and next(iter(t)) != key:
            items = [(key, t[key])] + [(k, v) for k, v in t.items()
                                       if k != key]
            t.clear()
            t.update(items)
    except Exception:
        pass


def build_program(debug=False):
    nc = bacc.Bacc("TRN2", target_bir_lowering=False, debug=False,
                   num_devices=NCORES)
    _prefer_ln_exp_table(nc.m.arch)

    refF = nc.dram_tensor("refF", [C, H, W], F32, kind="ExternalInput")
    srcF = nc.dram_tensor("srcF", [NV, C, H, W], F32, kind="ExternalInput")
    dep = nc.dram_tensor("dep", [DPC, H, W], F32, kind="ExternalInput")
    amap = nc.dram_tensor("amap", [NV, 3, H, W], F32, kind="ExternalInput")
    tvec = nc.dram_tensor("tvec", [128, 12], F32, kind="ExternalInput")
    ident = nc.dram_tensor("ident", [128, 128], F32, kind="ExternalInput")
    out3 = nc.dram_tensor("out3", [3, H, W], F32, kind="ExternalOutput")
    # internal patch maps, one per src view
    pmap = [nc.dram_tensor(f"pmap{v}", [POS, CH], F16, kind="Internal")
            for v in range(NV)]

    with nc.allow_low_precision("fp16 pipeline by design"), \
            tile.TileContext(nc) as tc:
        ctx_pools = []

        def pool(name, bufs=1, **kw):
            p = tc.tile_pool(name=name, bufs=bufs, **kw)
            ctx_pools.append(p)
            return p.__enter__()

        pp = pool("persist", 1)     # long-lived tiles
        psp = pool("psum", 2, space="PSUM")
        prep = tc.tile_pool(name="prep", bufs=1)
        pr = prep.__enter__()

        # ---------------- constant / persistent loads ----------------
        amapL = pp.tile([128, NV * 3 * W], F32, tag="amapL")  # [y,(v,row,x)]
        nc.sync.dma_start(out=amapL[:], in_=_ap(
            amap.ap(), 0, [[W, 128], [3 * H * W, NV], [H * W, 3], [1, W]]))
        tvecT = pp.tile([128, 12], F32, tag="tvecT")
        nc.sync.dma_start(out=tvecT[:], in_=tvec.ap())
        identT = pp.tile([128, 128], F32, tag="identT")
        nc.sync.dma_start(out=identT[:], in_=ident.ap())

        def tv(col):  # [128,1] per-partition scalar AP
            return tvecT[:, col:col + 1]

        # zero-source tile for edge DMAs (f16)
        ztile = pp.tile([128, PADX * C], F16, tag="ztile")
        nc.gpsimd.memset(ztile[:], 0.0)

        # ---------------- ref prep ----------------
        tr = pr.tile([128, C * W], F32, tag="tr")  # [y,(c,x)] f32
        nc.sync.dma_start(out=tr[:], in_=_ap(
            refF.ap(), 0, [[W, 128], [H * W, C], [1, W]]))
        # refC [y,(x,c)] fp16
        refC = pp.tile([128, W * C], F16, tag="refC")
        nc.vector.tensor_copy(
            out=_ap(refC[:], 0, [[W * C, 128], [C, W], [1, C]]),
            in_=_ap(tr[:], 0, [[C * W, 128], [1, W], [W, C]]))
        # refPadC [y,(xp132,c)] fp16, x' = x+2
        refPadC = pr.tile([128, PADX * C], F16, tag="refPadC")
        nc.vector.memset(
            _ap(refPadC[:], 0, [[PADX * C, 128], [C, 2], [1, C]]), 0.0)
        nc.vector.memset(
            _ap(refPadC[:], 130 * C, [[PADX * C, 128], [C, 2], [1, C]]), 0.0)
        nc.scalar.copy(out=refPadC[:, 2 * C:(2 + W) * C], in_=refC[:])
        # 4 partition-shifted copies (ty in {0,1,3,4}; center ty=2 = refPadC)
        refSC = {}
        for ty in (0, 1, 3, 4):
            t = pr.tile([128, PADX * C], F16, tag=f"refSC{ty}")
            k = ty - 2
            if k < 0:
                nc.sync.dma_start(out=t[-k:128, :], in_=refPadC[0:128 + k, :])
                nc.sync.dma_start(out=t[0:-k, :], in_=ztile[0:-k, :])
            else:
                nc.sync.dma_start(out=t[0:128 - k, :], in_=refPadC[k:128, :])
                nc.sync.dma_start(out=t[128 - k:128, :], in_=ztile[0:k, :])
            refSC[ty] = t
        refSC[2] = refPadC

        # ---------------- patch map build (per src view) ----------------
        build_dumps = []
        for v in range(NV):
            # staged [y, (xp132, dy2, dx2, c16, pad64)] fp16; row y -> pos
            # row y+2.  Zero only the data slots of border cols xp in
            # {0,1} u {129,130,131}; pad elems are never read downstream.
            staged = pr.tile([128, PW * CH], F16, tag=f"staged{v}")
            nc.vector.memset(
                _ap(staged[:], 0, [[PW * CH, 128], [CH, 2], [1, 64]]), 0.0)
            nc.vector.memset(
                _ap(staged[:], 129 * CH, [[PW * CH, 128], [CH, 3], [1, 64]]),
                0.0)
            # pad slots [64:128) of every chunk: zeroed once on Pool so the
            # dumped pmap is fully initialized (gather reads whole chunks)
            nc.gpsimd.memset(
                _ap(staged[:], 64, [[PW * CH, 128], [CH, PW], [1, 64]]), 0.0)
            # TF16 [y, (c,x)] fp16  (cast during DMA, SWDGE)
            tf = pr.tile([128, C * W], F16, tag="tf")
            nc.gpsimd.dma_start(
                out=tf[:], in_=_ap(srcF.ap(), v * C * H * W,
                                   [[W, 128], [H * W, C], [1, W]]))
            # down-shifted copy: tfdn[y] = tf[y+1]; row 127 = 0
            tfdn = pr.tile([128, C * W], F16, tag="tfdn")
            nc.sync.dma_start(out=tfdn[0:127, :], in_=tf[1:128, :])
            nc.sync.dma_start(out=tfdn[127:128, :],
                              in_=_ap(ztile[:], 0, [[PADX * C, 1], [1, C * W]]))

            copy_fns = [nc.vector.tensor_copy,
                        lambda out, in_: nc.scalar.copy(out=out, in_=in_),
                        nc.gpsimd.tensor_copy,
                        lambda out, in_: nc.scalar.copy(out=out, in_=in_)]
            i = 0
            for dy, src in ((0, tf), (1, tfdn)):
                for dx in (0, 1):
                    # staged[y, xp, dy, dx, c] = src[y, c, xp-2+dx]
                    xp_lo = 2 - dx
                    n_xp = 128
                    src_ap = _ap(src[:], 0,
                                 [[C * W, 128], [1, n_xp], [W, C]])
                    dst_ap = _ap(staged[:], xp_lo * CH + dy * 32 + dx * 16,
                                 [[PW * CH, 128], [CH, n_xp], [1, C]])
                    copy_fns[i](out=dst_ap, in_=src_ap)
                    i += 1
            # dump rows 2..129 of the pos grid (partition y -> row y+2)
            d1 = nc.sync.dma_start(
                out=_ap(pmap[v].ap(), 2 * PW * CH,
                        [[PW * CH, 128], [1, PW * CH]]),
                in_=staged[:])
            # zero rows 0, 130, 131 from ztile (128x132 slice covers a row)
            dz = []
            for r in (0, 130, 131):
                dz.append(nc.sync.dma_start(
                    out=_ap(pmap[v].ap(), r * PW * CH,
                            [[PW * CH, 1], [1, PW * CH]]),
                    in_=_ap(ztile[:], 0, [[PADX * C, 8], [1, PW * CH // 8]])))
            # row r=1 (y0=-1): dy=1 slots hold F row 0 = staged[0]'s dy=0
            # window (same cells, slot offset +32); rest zero
            d4z = nc.sync.dma_start(
                out=_ap(pmap[v].ap(), PW * CH, [[PW * CH, 1], [1, PW * CH]]),
                in_=_ap(ztile[:], 0, [[PADX * C, 8], [1, PW * CH // 8]]))
            d4 = nc.sync.dma_start(
                out=_ap(pmap[v].ap(), PW * CH + 32,
                        [[PW * CH, 1], [CH, PW], [1, 32]]),
                in_=_ap(staged[0:1, :], 0, [[PW * CH, 1], [CH, PW], [1, 32]]))
            add_dep_helper(d4.ins, d4z.ins, reason="row1 zero before windows")
            build_dumps.append(tuple([d1, d4, d4z] + dz))

        # ---------------- w_feat volume [y, (t25, x)] fp16 ----
        wf25 = pp.tile([128, TM], F16, tag="wf25")
        nc.vector.memset(wf25[:, 12 * W:13 * W], 0.0)
        for ty in (2, 1, 3, 0, 4):
            for tx in range(5):
                t = ty * 5 + tx
                if t == 12:
                    continue
                # diff = ref[y,x,c] - refSC[ty][y, x+tx, c]
                dfw = pr.tile([128, W * C], F16, tag="wfdf")
                nc.vector.tensor_tensor(
                    out=dfw[:],
                    in0=_ap(refSC[ty][:], tx * C,
                            [[PADX * C, 128], [C, W], [1, C]]),
                    in1=refC[:], op=OP.subtract)
                sqw = pr.tile([128, W * C], F16, tag="wfsq")
                nc.scalar.activation(out=sqw[:], in_=dfw[:], func=AF.Square)
                # c16 tree reduction (packed fp16 halves)
                c8 = pr.tile([128, W * 8], F16, tag="c8w")
                nc.vector.tensor_tensor(
                    out=_ap(c8[:], 0, [[W * 8, 128], [8, W], [1, 8]]),
                    in0=_ap(sqw[:], 0, [[W * C, 128], [C, W], [1, 8]]),
                    in1=_ap(sqw[:], 8, [[W * C, 128], [C, W], [1, 8]]),
                    op=OP.add)
                c4 = pr.tile([128, W * 4], F16, tag="c4w")
                nc.vector.tensor_tensor(
                    out=_ap(c4[:], 0, [[W * 4, 128], [4, W], [1, 4]]),
                    in0=_ap(c8[:], 0, [[W * 8, 128], [8, W], [1, 4]]),
                    in1=_ap(c8[:], 4, [[W * 8, 128], [8, W], [1, 4]]),
                    op=OP.add)
                c2 = pr.tile([128, W * 2], F16, tag="c2w")
                nc.vector.tensor_tensor(
                    out=_ap(c2[:], 0, [[W * 2, 128], [2, W], [1, 2]]),
                    in0=_ap(c4[:], 0, [[W * 4, 128], [4, W], [1, 2]]),
                    in1=_ap(c4[:], 2, [[W * 4, 128], [4, W], [1, 2]]),
                    op=OP.add)
                cs = pr.tile([128, W], F16, tag="csw")
                nc.vector.tensor_tensor(
                    out=cs[:],
                    in0=_ap(c2[:], 0, [[W * 2, 128], [2, W]]),
                    in1=_ap(c2[:], 1, [[W * 2, 128], [2, W]]),
                    op=OP.add)
                # wf = sqrt(cs) = exp(0.5*ln(cs))  (ln(0) -> -inf -> exp 0)
                lnw = pr.tile([128, W], F16, tag="lnw")
                nc.scalar.activation(out=lnw[:], in_=cs[:], func=AF.Ln)
                nc.scalar.activation(out=wf25[:, t * W:(t + 1) * W],
                                     in_=lnw[:], func=AF.Exp, scale=tv(10))


        prep.__exit__(None, None, None)
        wp = pool("work", 2)        # small per-(b,d) working tiles
        bp = pool("big4", 4)        # shared 4KB scratch (tag s4k)
        cp = pool("cumdif", 1)      # cum/diff accumulators
        gp = pool("gath", 2)        # gather destinations
        ap_ = pool("aggp", 2)       # agg volumes

        # wrapped-16 gather indices; partitions 16.. stay 0 forever
        wr16ab = []
        for nm in ("wr16a", "wr16b"):
            t = pp.tile([128, W * 8], I16, tag=nm)
            nc.vector.memset(t[:], 0.0)
            wr16ab.append(t)

        # depth planes f32 [y,(x,d)] and agg store
        depD = pp.tile([128, W * DPC], F32, tag="depD")
        aggT = pp.tile([128, W * DPC], F32, tag="aggT")

        # ---------------- per depth-plane pipeline ----------------
        for di in range(DPC):
            depf = wp.tile([128, W], F32, tag="depf")
            nc.sync.dma_start(out=depf[:], in_=dep.ap()[di])
            nc.vector.tensor_copy(
                out=_ap(depD[:], di, [[W * DPC, 128], [DPC, W], [1, 1]]),
                in_=depf[:])
            # depSC5 [y, (ty5, xp132)] fp16: center block + 4 shifted blocks
            depSC5 = wp.tile([128, 5 * PADX], F16, tag="depSC5")
            nc.scalar.copy(
                out=_ap(depSC5[:], 2 * PADX + 2, [[5 * PADX, 128], [1, W]]),
                in_=depf[:])
            nc.vector.memset(
                _ap(depSC5[:], 2 * PADX, [[5 * PADX, 128], [130, 2], [1, 2]]),
                0.0)
            for ty in (0, 1, 3, 4):
                k = ty - 2
                dlo = ty * PADX
                clo = 2 * PADX
                if k < 0:
                    nc.sync.dma_start(
                        out=depSC5[-k:128, dlo:dlo + PADX],
                        in_=depSC5[0:128 + k, clo:clo + PADX])
                    nc.sync.dma_start(out=depSC5[0:-k, dlo:dlo + PADX],
                                      in_=ztile[0:-k, 0:PADX])
                else:
                    nc.sync.dma_start(
                        out=depSC5[0:128 - k, dlo:dlo + PADX],
                        in_=depSC5[k:128, clo:clo + PADX])
                    nc.sync.dma_start(out=depSC5[128 - k:128, dlo:dlo + PADX],
                                      in_=ztile[0:k, 0:PADX])

            cum = cp.tile([128, W * C], F16, tag="cum")
            diff = cp.tile([128, W * C], F16, tag="diff")
            csq = {}
            for v in range(NV):
                # ---- coordinates (pixel-major [y,x] f32) ----
                def arow(r):
                    return _ap(amapL[:], (v * 3 + r) * W,
                               [[NV * 3 * W, 128], [1, W]])
                mx = wp.tile([128, W], F32, tag="mx")
                my = wp.tile([128, W], F32, tag="my")
                dn = wp.tile([128, W], F32, tag="dn")
                nc.vector.tensor_tensor(out=mx[:], in0=arow(0), in1=depf[:],
                                        op=OP.mult)
                nc.vector.tensor_tensor(out=my[:], in0=arow(1), in1=depf[:],
                                        op=OP.mult)
                nc.vector.tensor_tensor(out=dn[:], in0=arow(2), in1=depf[:],
                                        op=OP.mult)
                nx = wp.tile([128, W], F32, tag="nx")
                ny = wp.tile([128, W], F32, tag="ny")
                dnt = wp.tile([128, W], F32, tag="dnt")
                nc.scalar.activation(out=nx[:], in_=mx[:], func=AF.Identity,
                                     bias=tv(v * 3 + 0))
                nc.scalar.activation(out=ny[:], in_=my[:], func=AF.Identity,
                                     bias=tv(v * 3 + 1))
                nc.scalar.activation(out=dnt[:], in_=dn[:], func=AF.Identity,
                                     bias=tv(v * 3 + 2))
                rec = wp.tile([128, W], F32, tag="rec")
                nc.vector.reciprocal(out=rec[:], in_=dnt[:])
                gx = wp.tile([128, W], F32, tag="gx")
                gy = wp.tile([128, W], F32, tag="gy")
                nc.vector.tensor_tensor(out=gx[:], in0=nx[:], in1=rec[:],
                                        op=OP.mult)
                nc.vector.tensor_tensor(out=gy[:], in0=ny[:], in1=rec[:],
                                        op=OP.mult)

                # xs = trunc(g + 1.5) = floor(g - 0.5) + 2 for g - 0.5 >= -2
                # (fully-OOB positions clamp into the zero border either way)
                def floorfrac(g, nm):
                    xi = wp.tile([128, W], I32, tag=f"i{nm}")
                    nc.scalar.activation(out=xi[:], in_=g[:],
                                         func=AF.Identity, bias=tv(6))
                    xf = wp.tile([128, W], F32, tag=f"xf{nm}")
                    nc.scalar.copy(out=xf[:], in_=xi[:])
                    fr = wp.tile([128, W], F32, tag=f"fr{nm}")
                    nc.vector.scalar_tensor_tensor(
                        out=fr[:], in0=g[:], scalar=1.5, in1=xf[:],
                        op0=OP.add, op1=OP.subtract)
                    xc = wp.tile([128, W], F32, tag=f"xc{nm}")
                    nc.vector.tensor_scalar(out=xc[:], in0=xf[:],
                                            scalar1=131.0, scalar2=0.0,
                                            op0=OP.min, op1=OP.max)
                    return xc, fr

                xsC, fx = floorfrac(gx, "x")
                ysC, fy = floorfrac(gy, "y")
                posF = wp.tile([128, W], F32, tag="posF")
                nc.vector.scalar_tensor_tensor(
                    out=posF[:], in0=ysC[:], scalar=float(PW), in1=xsC[:],
                    op0=OP.mult, op1=OP.add)

                # ---- fold posF [y, x] -> wrapped16 idx [16, x*8+xh] ----
                wr16 = wr16ab[(di * NV + v) % 2]
                for grp in range(2):
                    ps4 = psp.tile([16, W * 4], F32, tag="ps4")
                    for j in range(4):
                        xh = grp * 4 + j
                        nc.tensor.matmul(out=ps4[0:16, j * W:(j + 1) * W],
                                         lhsT=identT[:, 16 * xh:16 * xh + 16],
                                         rhs=posF[:], start=True, stop=True)
                    # Act copy f32->i16 (trunc; values are exact ints)
                    nc.scalar.copy(
                        out=_ap(wr16[:], grp * 4,
                                [[W * 8, 16], [8, W], [1, 4]]),
                        in_=_ap(ps4[:], 0, [[W * 4, 16], [1, W], [W, 4]]))

                # ---- gather (dma_gather; idx partitions 16.. are zero) ----
                G = gp.tile([128, W * CH], F16, tag="G")
                for t in range(4):
                    gi = nc.gpsimd.dma_gather(
                        out_ap=_ap(G[:], t * 32 * CH,
                                   [[W * CH, 128], [CH, 32], [1, CH]]),
                        in_ap=pmap[v].ap(),
                        idxs_ap=_ap(wr16[:], t * 32 * 8,
                                    [[W * 8, 128], [1, 32 * 8]]),
                        num_idxs=32 * 128, num_idxs_reg=32 * 128,
                        elem_size=CH, queue_num=0)
                    for dma_i in build_dumps[v]:
                        add_dep_helper(gi.ins, dma_i.ins,
                                       reason="patch map build before gather")

                # fp16 fractional weights (cast/affine on Act engine)
                fx16 = wp.tile([128, W], F16, tag="fx16")
                fy16 = wp.tile([128, W], F16, tag="fy16")
                fxm16 = wp.tile([128, W], F16, tag="fxm16")
                fym16 = wp.tile([128, W], F16, tag="fym16")
                nc.scalar.copy(out=fx16[:], in_=fx[:])
                nc.scalar.copy(out=fy16[:], in_=fy[:])
                nc.scalar.activation(out=fxm16[:], in_=fx[:],
                                     func=AF.Identity, scale=tv(9),
                                     bias=tv(8))
                nc.scalar.activation(out=fym16[:], in_=fy[:],
                                     func=AF.Identity, scale=tv(9),
                                     bias=tv(8))
                wd = {}
                for (tnm, fa, fb) in (("00", fxm16, fym16),
                                      ("01", fx16, fym16),
                                      ("10", fxm16, fy16),
                                      ("11", fx16, fy16)):
                    wt = wp.tile([128, W * 2], F16, tag=f"wd{tnm}")
                    nc.vector.tensor_tensor(
                        out=_ap(wt[:], 0, [[W * 2, 128], [2, W], [1, 2]]),
                        in0=_ap(fa[:], 0, [[W, 128], [1, W], [0, 2]]),
                        in1=_ap(fb[:], 0, [[W, 128], [1, W], [0, 2]]),
                        op=OP.mult)
                    wd[tnm] = wt

                # ---- bilinear taps: acc = sum_t w_t * G_t  [y,(x,c)] ----
                def tap(dy, dx):
                    return _ap(G[:], (dy * 2 + dx) * 16,
                               [[W * CH, 128], [CH, W], [2, 8], [1, 2]])

                def wap(tnm):
                    return _ap(wd[tnm][:], 0,
                               [[W * 2, 128], [2, W], [0, 8], [1, 2]])

                cview = [[2048, 128], [16, W], [2, 8], [1, 2]]
                acc = cum if v == 0 else bp.tile([128, W * C], F16, tag="s4k")
                p0 = bp.tile([128, W * C], F16, tag="s4k")
                p1 = bp.tile([128, W * C], F16, tag="s4k")
                nc.vector.tensor_tensor(out=_ap(acc[:], 0, cview),
                                        in0=tap(0, 0), in1=wap("00"),
                                        op=OP.mult)
                nc.vector.tensor_tensor(out=_ap(p0[:], 0, cview),
                                        in0=tap(0, 1), in1=wap("01"),
                                        op=OP.mult)
                nc.vector.tensor_tensor(out=_ap(p1[:], 0, cview),
                                        in0=tap(1, 0), in1=wap("10"),
                                        op=OP.mult)
                nc.vector.tensor_tensor(out=acc[:], in0=acc[:], in1=p0[:],
                                        op=OP.add)
                nc.vector.tensor_tensor(out=_ap(p0[:], 0, cview),
                                        in0=tap(1, 1), in1=wap("11"),
                                        op=OP.mult)
                nc.vector.tensor_tensor(out=p1[:], in0=p1[:], in1=p0[:],
                                        op=OP.add)
                nc.vector.tensor_tensor(out=acc[:], in0=acc[:], in1=p1[:],
                                        op=OP.add)

                # ---- cost_v = sum_c (ref - cum)^2 (Act square + DVE tree) --
                if v == 0:
                    nc.vector.tensor_tensor(out=diff[:], in0=refC[:],
                                            in1=cum[:], op=OP.subtract)
                else:
                    nc.vector.tensor_tensor(out=diff[:], in0=diff[:],
                                            in1=acc[:], op=OP.subtract)
                sq = bp.tile([128, W * C], F16, tag="s4k")
                nc.scalar.activation(out=sq[:], in_=diff[:], func=AF.Square)
                c8 = wp.tile([128, W * 8], F16, tag="c8")
                nc.vector.tensor_tensor(
                    out=_ap(c8[:], 0, [[W * 8, 128], [8, W], [1, 8]]),
                    in0=_ap(sq[:], 0, [[W * C, 128], [C, W], [1, 8]]),
                    in1=_ap(sq[:], 8, [[W * C, 128], [C, W], [1, 8]]),
                    op=OP.add)
                c4 = wp.tile([128, W * 4], F16, tag="c4")
                nc.vector.tensor_tensor(
                    out=_ap(c4[:], 0, [[W * 4, 128], [4, W], [1, 4]]),
                    in0=_ap(c8[:], 0, [[W * 8, 128], [8, W], [1, 4]]),
                    in1=_ap(c8[:], 4, [[W * 8, 128], [8, W], [1, 4]]),
                    op=OP.add)
                c2 = wp.tile([128, W * 2], F16, tag="c2")
                nc.vector.tensor_tensor(
                    out=_ap(c2[:], 0, [[W * 2, 128], [2, W], [1, 2]]),
                    in0=_ap(c4[:], 0, [[W * 4, 128], [4, W], [1, 2]]),
                    in1=_ap(c4[:], 2, [[W * 4, 128], [4, W], [1, 2]]),
                    op=OP.add)
                cs = wp.tile([128, W], F16, tag=f"csq{v}")
                nc.vector.tensor_tensor(
                    out=cs[:],
                    in0=_ap(c2[:], 0, [[W * 2, 128], [2, W]]),
                    in1=_ap(c2[:], 1, [[W * 2, 128], [2, W]]),
                    op=OP.add)
                csq[v] = cs

            # cost = sqrt(min(c1sq, c2sq)) into costSC5 center block
            cmin = wp.tile([128, W], F16, tag="cmin")
            nc.vector.tensor_tensor(out=cmin[:], in0=csq[0][:], in1=csq[1][:],
                                    op=OP.min)
            lnc = wp.tile([128, W], F16, tag="lnc")
            nc.scalar.activation(out=lnc[:], in_=cmin[:], func=AF.Ln)
            costSC5 = wp.tile([128, 5 * PADX], F16, tag="costSC5")
            nc.scalar.activation(
                out=_ap(costSC5[:], 2 * PADX + 2, [[5 * PADX, 128], [1, W]]),
                in_=lnc[:], func=AF.Exp, scale=tv(10))
            nc.vector.memset(
                _ap(costSC5[:], 2 * PADX, [[5 * PADX, 128], [130, 2], [1, 2]]),
                0.0)
            for ty in (0, 1, 3, 4):
                k = ty - 2
                dlo = ty * PADX
                clo = 2 * PADX
                if k < 0:
                    nc.sync.dma_start(
                        out=costSC5[-k:128, dlo:dlo + PADX],
                        in_=costSC5[0:128 + k, clo:clo + PADX])
                    nc.sync.dma_start(out=costSC5[0:-k, dlo:dlo + PADX],
                                      in_=ztile[0:-k, 0:PADX])
                else:
                    nc.sync.dma_start(
                        out=costSC5[0:128 - k, dlo:dlo + PADX],
                        in_=costSC5[k:128, clo:clo + PADX])
                    nc.sync.dma_start(out=costSC5[128 - k:128, dlo:dlo + PADX],
                                      in_=ztile[0:k, 0:PADX])

            # ---- aggregation, tap-major [y, (t25, x)] ----
            dvol = ap_.tile([128, TM], F16, tag="dvol")
            nc.vector.tensor_tensor(
                out=_ap(dvol[:], 0, [[TM, 128], [5 * W, 5], [W, 5], [1, W]]),
                in0=_ap(depSC5[:], 0,
                        [[5 * PADX, 128], [PADX, 5], [1, 5], [1, W]]),
                in1=_ap(depSC5[:], 2 * PADX + 2,
                        [[5 * PADX, 128], [0, 5], [0, 5], [1, W]]),
                op=OP.subtract)
            nc.scalar.activation(out=dvol[:], in_=dvol[:], func=AF.Abs)
            evol = ap_.tile([128, TM], F16, tag="evol")
            nc.scalar.activation(out=evol[:], in_=dvol[:], func=AF.Exp,
                                 bias=tv(7))
            uvol = dvol  # reuse: dvol's last reader is the exp above
            nc.vector.tensor_tensor(out=uvol[:], in0=evol[:], in1=wf25[:],
                                    op=OP.mult)
            nc.vector.tensor_tensor(
                out=_ap(uvol[:], 0, [[TM, 128], [5 * W, 5], [W, 5], [1, W]]),
                in0=_ap(uvol[:], 0, [[TM, 128], [5 * W, 5], [W, 5], [1, W]]),
                in1=_ap(costSC5[:], 0,
                        [[5 * PADX, 128], [PADX, 5], [1, 5], [1, W]]),
                op=OP.mult)

            def tap_tree(vol, nm):
                s12 = wp.tile([128, 12 * W], F16, tag="s12")
                nc.vector.tensor_tensor(out=s12[:], in0=vol[:, 0:12 * W],
                                        in1=vol[:, 12 * W:24 * W], op=OP.add)
                s6 = wp.tile([128, 6 * W], F16, tag="s6")
                nc.vector.tensor_tensor(out=s6[:], in0=s12[:, 0:6 * W],
                                        in1=s12[:, 6 * W:12 * W], op=OP.add)
                s3 = wp.tile([128, 3 * W], F16, tag="s3")
                nc.vector.tensor_tensor(out=s3[:], in0=s6[:, 0:3 * W],
                                        in1=s6[:, 3 * W:6 * W], op=OP.add)
                r = wp.tile([128, W], F16, tag=f"r{nm}")
                nc.vector.tensor_tensor(out=r[:], in0=s3[:, 0:W],
                                        in1=s3[:, W:2 * W], op=OP.add)
                nc.vector.tensor_tensor(out=r[:], in0=r[:],
                                        in1=s3[:, 2 * W:3 * W], op=OP.add)
                nc.vector.tensor_tensor(out=r[:], in0=r[:],
                                        in1=vol[:, 24 * W:25 * W], op=OP.add)
                return r

            num = tap_tree(uvol, "n")
            den = tap_tree(evol, "d")
            rden = wp.tile([128, W], F32, tag="rden")
            nc.vector.reciprocal(out=rden[:], in_=den[:])
            agg_ap = _ap(aggT[:], di, [[W * DPC, 128], [DPC, W], [1, 1]])
            nc.vector.tensor_tensor(out=agg_ap, in0=num[:], in1=rden[:],
                                    op=OP.mult)

        # ---------------- per-core softmax partials ----------------
        def aggap(di):
            return _ap(aggT[:], di, [[W * DPC, 128], [DPC, W], [1, 1]])

        def depap(di):
            return _ap(depD[:], di, [[W * DPC, 128], [DPC, W], [1, 1]])

        m = pp.tile([128, W], F32, tag="m")
        nc.vector.tensor_tensor(out=m[:], in0=aggap(0), in1=aggap(1),
                                op=OP.max)
        for di in range(2, DPC):
            nc.vector.tensor_tensor(out=m[:], in0=m[:], in1=aggap(di),
                                    op=OP.max)
        s0 = pp.tile([128, W], F32, tag="s0")
        s1 = pp.tile([128, W], F32, tag="s1")
        for di in range(DPC):
            t = wp.tile([128, W], F32, tag="et")
            nc.vector.tensor_tensor(out=t[:], in0=aggap(di), in1=m[:],
                                    op=OP.subtract)
            e = wp.tile([128, W], F32, tag="ee")
            nc.scalar.activation(out=e[:], in_=t[:], func=AF.Exp)
            t1 = wp.tile([128, W], F32, tag="t1")
            nc.vector.tensor_tensor(out=t1[:], in0=e[:], in1=depap(di),
                                    op=OP.mult)
            if di == 0:
                nc.vector.tensor_copy(out=s0[:], in_=e[:])
                nc.vector.tensor_copy(out=s1[:], in_=t1[:])
            else:
                nc.vector.tensor_tensor(out=s0[:], in0=s0[:], in1=e[:],
                                        op=OP.add)
                nc.vector.tensor_tensor(out=s1[:], in0=s1[:], in1=t1[:],
                                        op=OP.add)
        nc.sync.dma_start(out=out3.ap()[0], in_=m[:])
        nc.sync.dma_start(out=out3.ap()[1], in_=s0[:])
        nc.sync.dma_start(out=out3.ap()[2], in_=s1[:])

        for p in reversed(ctx_pools):
            p.__exit__(None, None, None)

    nc.compile()
    return nc


def host_prep(features, intrinsics, cam_to_world, depth_hypo):
    """Build the 8 per-core input maps. All O(small) except slicing."""
    features = np.asarray(features, dtype=np.float32)
    intrinsics = np.asarray(intrinsics, dtype=np.float32)
    cam_to_world = np.asarray(cam_to_world, dtype=np.float32)
    depth_hypo = np.asarray(depth_hypo, dtype=np.float32)

    ys, xs = np.meshgrid(np.arange(H, dtype=np.float32),
                         np.arange(W, dtype=np.float32), indexing="ij")
    in_maps = []
    for k in range(NCORES):
        b = k // (NCORES // B)
        dlo = DPC * (k % (NCORES // B))
        amap = np.zeros((NV, 3, H, W), np.float32)
        tvv = np.zeros((12,), np.float32)
        for vi in range(1, V):
            src_w2c = np.linalg.inv(cam_to_world[vi, b])
            ref_w2c = np.linalg.inv(cam_to_world[0, b])
            src_KK = src_w2c.copy()
            src_KK[:3, :3] = intrinsics[vi, b]
            ref_KK = ref_w2c.copy()
            ref_KK[:3, :3] = intrinsics[0, b]
            proj = (src_KK @ src_w2c) @ np.linalg.inv(ref_KK @ ref_w2c)
            rot, trans = proj[:3, :3], proj[:3, 3]
            A = (rot[:, 0:1, None] * xs[None] + rot[:, 1:2, None] * ys[None]
                 + rot[:, 2:3, None])  # [3, H, W]
            v = vi - 1
            amap[v, 0] = A[0] * SCALE
            amap[v, 1] = A[1] * SCALE
            amap[v, 2] = A[2]
            tvv[v * 3 + 0] = trans[0] * SCALE
            tvv[v * 3 + 1] = trans[1] * SCALE
            tvv[v * 3 + 2] = trans[2]
        tvv[6] = 1.5      # floor-by-trunc bias (+2 border shift - 0.5 center)
        tvv[7] = ESHIFT
        tvv[8] = 1.0
        tvv[9] = -1.0
        tvv[10] = 0.5
        in_maps.append({
            "refF": np.ascontiguousarray(features[0, b].transpose(0, 2, 1)),
            "srcF": np.ascontiguousarray(features[1:, b]),
            "dep": np.ascontiguousarray(
                depth_hypo[b, dlo:dlo + DPC].transpose(0, 2, 1)),
            "amap": np.ascontiguousarray(amap.transpose(0, 1, 3, 2)),
            "tvec": np.tile(tvv[None, :], (128, 1)).astype(np.float32),
            "ident": np.eye(128, dtype=np.float32),
        })
    return in_maps


def host_combine(results):
    """Merge per-core softmax partials (m, s0, s1) into [B, H, W]."""
    out = np.zeros((B, H, W), np.float32)
    per_b = NCORES // B
    for b in range(B):
        parts = [np.asarray(results[b * per_b + j]["out3"]) for j in range(per_b)]
        parts = [p.transpose(0, 2, 1) for p in parts]  # [3, x, y] -> [3, y, x]
        ms = np.stack([p[0] for p in parts])         # [4, H, W]
        M = ms.max(axis=0)
        S0 = np.zeros((H, W), np.float64)
        S1 = np.zeros((H, W), np.float64)
        for p in parts:
            w = np.exp(p[0] - M)
            S0 += w * p[1]
            S1 += w * p[2]
        out[b] = (S1 / S0).astype(np.float32)
    return out


def _run_sim(nc, in_maps):
    from concourse.bass_interp import CoreSim
    results = []
    for core in range(NCORES):
        sim = CoreSim(nc, trace=False, publish_trace=False)
        for k, v in in_maps[core].items():
            sim.tensor(k)[:] = v
        sim.simulate()
        results.append({"out3": np.array(sim.tensor("out3"))})
    return results


def kernel(**inputs):
    if "nc" not in _cached:
        _cached["nc"] = build_program()
    nc = _cached["nc"]
    in_maps = host_prep(**inputs)
    if _cached.get("hw_broken"):
        return host_combine(_run_sim(nc, in_maps))
    try:
        res = run_bass_kernel_spmd(nc, in_maps, core_ids=list(range(NCORES)))
        return host_combine(res.results)
    except Exception:
        _cached["hw_broken"] = True
        return host_combine(_run_sim(nc, in_maps))


if __name__ == "__main__":
    import reference
    inp = reference.setup_inputs()
    inp = {k: np.asarray(v) for k, v in inp.items()}
    out = kernel(**inp)
    print("kernel out", out.shape, out.dtype)



# revision 2
# speedup vs baseline: 1.6610x; 1.6610x over previous
"""Trainium2 Bass kernel for nn_DepthPrediction (multi-view stereo depth).

Strategy (8 NeuronCores, SPMD single program):
  - Shard: core k handles batch b = k//4 and depth planes [8*(k%4), 8*(k%4)+8).
  - Per (b,d): homography warp of 2 src views via on-device fp16 "patch maps"
    (132x132 positions x [dy2,dx2,c16] fp16 chunks = 256B) gathered with one
    indirect-DMA descriptor per pixel; bilinear interp + cumulative cost
    (L2 over 16 ch) in pixel-major fp16 on DVE; 5x5 adaptive aggregation
    (depth-similarity softmax x feature-similarity weight) in tap-major
    layout; per-core softmax partials over the 8 local planes.
  - Engine budget: DVE does the packed-fp16 tensor math; Activation does
    affine/exp/ln/square/casts (single act table: natural_log_exp);
    Pool does gathers + memsets; PE folds gather indices; SP does DMAs.
  - Host: trivial glue — 4x4 matrix algebra, shard/pack inputs, combine the
    4-way softmax partials per batch (log-sum-exp merge) into the output.

Self-contained: hardcodes all shapes from the problem spec.
"""

import numpy as np

import concourse.bacc as bacc
import concourse.bass as bass
import concourse.mybir as mybir
import concourse.tile as tile
from concourse.bass_utils import run_bass_kernel_spmd
from concourse.tile_rust import add_dep_helper

F32 = mybir.dt.float32
F16 = mybir.dt.float16
I32 = mybir.dt.int32
I16 = mybir.dt.int16

AF = mybir.ActivationFunctionType
OP = mybir.AluOpType
AX = mybir.AxisListType

# problem shapes
V, B, C, H, W, D = 3, 2, 16, 128, 128, 32
NCORES = 8
DPC = D // (NCORES // B)  # depth planes per core = 8
NV = V - 1  # src views = 2

PW = W + 4          # padded map width (x0 in [-2,129] -> cols 0..131)
POS = PW * PW       # patch positions
CH = 128            # chunk elems: [dy2,dx2,c16]+pad64 fp16 = 256B
PADX = W + 4        # x-padded tiles for 5-tap aggregation
SCALE = W / (W - 1.0)  # grid_sample align_corners=False fold
ESHIFT = -9.0       # exp(|dnb-d|) stabilization shift (|delta| <= 9)
NT = 25             # aggregation taps
TM = NT * W         # tap-major volume free size

_cached = {}


def _ap(base, off, dims):
    """Raw AP on the same tensor as `base` (an AP), offset in elements."""
    return bass.AP(base.tensor, base.offset + off, dims)


def _prefer_ln_exp_table(arch):
    """Reorder the cached act-table dict so the one table containing ALL the
    funcs this kernel uses (exp, ln, identity, copy, abs, square) is picked
    first -> a single LoadActFuncSet instead of per-op thrash."""
    try:
        from concourse.hw_specs import get_activation_tables
        t = get_activation_tables(arch)
        key = "natural_log_exp_and_others"
        if key in t and next(iter(t)) != key:
            items = [(key, t[key])] + [(k, v) for k, v in t.items()
                                       if k != key]
            t.clear()
            t.update(items)
    except Exception:
        pass


def build_program(debug=False):
    nc = bacc.Bacc("TRN2", target_bir_lowering=False, debug=False,
                   num_devices=NCORES)
    _prefer_ln_exp_table(nc.m.arch)

    refF = nc.dram_tensor("refF", [C, H, W], F32, kind="ExternalInput")
    srcF = nc.dram_tensor("srcF", [NV, C, H, W], F32, kind="ExternalInput")
    dep = nc.dram_tensor("dep", [DPC, H, W], F32, kind="ExternalInput")
    amap = nc.dram_tensor("amap", [NV, 3, H, W], F32, kind="ExternalInput")
    tvec = nc.dram_tensor("tvec", [128, 12], F32, kind="ExternalInput")
    ident = nc.dram_tensor("ident", [128, 128], F32, kind="ExternalInput")
    out3 = nc.dram_tensor("out3", [3, H, W], F32, kind="ExternalOutput")
    # internal patch maps, one per src view
    pmap = [nc.dram_tensor(f"pmap{v}", [POS, CH], F16, kind="Internal")
            for v in range(NV)]

    with nc.allow_low_precision("fp16 pipeline by design"), \
            tile.TileContext(nc) as tc:
        ctx_pools = []

        def pool(name, bufs=1, **kw):
            p = tc.tile_pool(name=name, bufs=bufs, **kw)
            ctx_pools.append(p)
            return p.__enter__()

        pp = pool("persist", 1)     # long-lived tiles
        psp = pool("psum", 2, space="PSUM")
        prep = tc.tile_pool(name="prep", bufs=1)
        pr = prep.__enter__()

        # ---------------- constant / persistent loads ----------------
        amapL = pp.tile([128, NV * 3 * W], F32, tag="amapL")  # [y,(v,row,x)]
        nc.sync.dma_start(out=amapL[:], in_=_ap(
            amap.ap(), 0, [[W, 128], [3 * H * W, NV], [H * W, 3], [1, W]]))
        tvecT = pp.tile([128, 12], F32, tag="tvecT")
        nc.sync.dma_start(out=tvecT[:], in_=tvec.ap())
        identT = pp.tile([128, 128], F32, tag="identT")
        nc.sync.dma_start(out=identT[:], in_=ident.ap())

        def tv(col):  # [128,1] per-partition scalar AP
            return tvecT[:, col:col + 1]

        # zero-source tile for edge DMAs (f16)
        ztile = pp.tile([128, PADX * C], F16, tag="ztile")
        nc.gpsimd.memset(ztile[:], 0.0)

        # ---------------- ref prep ----------------
        tr = pr.tile([128, C * W], F32, tag="tr")  # [y,(c,x)] f32
        nc.sync.dma_start(out=tr[:], in_=_ap(
            refF.ap(), 0, [[W, 128], [H * W, C], [1, W]]))
        # refC [y,(x,c)] fp16
        refC = pp.tile([128, W * C], F16, tag="refC")
        nc.vector.tensor_copy(
            out=_ap(refC[:], 0, [[W * C, 128], [C, W], [1, C]]),
            in_=_ap(tr[:], 0, [[C * W, 128], [1, W], [W, C]]))
        # refPadC [y,(xp132,c)] fp16, x' = x+2
        refPadC = pr.tile([128, PADX * C], F16, tag="refPadC")
        nc.vector.memset(
            _ap(refPadC[:], 0, [[PADX * C, 128], [C, 2], [1, C]]), 0.0)
        nc.vector.memset(
            _ap(refPadC[:], 130 * C, [[PADX * C, 128], [C, 2], [1, C]]), 0.0)
        nc.scalar.copy(out=refPadC[:, 2 * C:(2 + W) * C], in_=refC[:])
        # 4 partition-shifted copies (ty in {0,1,3,4}; center ty=2 = refPadC)
        refSC = {}
        for ty in (0, 1, 3, 4):
            t = pr.tile([128, PADX * C], F16, tag=f"refSC{ty}")
            k = ty - 2
            if k < 0:
                nc.sync.dma_start(out=t[-k:128, :], in_=refPadC[0:128 + k, :])
                nc.sync.dma_start(out=t[0:-k, :], in_=ztile[0:-k, :])
            else:
                nc.sync.dma_start(out=t[0:128 - k, :], in_=refPadC[k:128, :])
                nc.sync.dma_start(out=t[128 - k:128, :], in_=ztile[0:k, :])
            refSC[ty] = t
        refSC[2] = refPadC

        # ---------------- patch map build (per src view) ----------------
        build_dumps = []
        for v in range(NV):
            # staged [y, (xp132, dy2, dx2, c16, pad64)] fp16; row y -> pos
            # row y+2.  Zero only the data slots of border cols xp in
            # {0,1} u {129,130,131}; pad elems are never read downstream.
            staged = pr.tile([128, PW * CH], F16, tag=f"staged{v}")
            nc.vector.memset(
                _ap(staged[:], 0, [[PW * CH, 128], [CH, 2], [1, 64]]), 0.0)
            nc.vector.memset(
                _ap(staged[:], 129 * CH, [[PW * CH, 128], [CH, 3], [1, 64]]),
                0.0)
            # pad slots [64:128) of every chunk: zeroed once on Pool so the
            # dumped pmap is fully initialized (gather reads whole chunks)
            nc.gpsimd.memset(
                _ap(staged[:], 64, [[PW * CH, 128], [CH, PW], [1, 64]]), 0.0)
            # TF16 [y, (c,x)] fp16  (cast during DMA, SWDGE)
            tf = pr.tile([128, C * W], F16, tag="tf")
            nc.gpsimd.dma_start(
                out=tf[:], in_=_ap(srcF.ap(), v * C * H * W,
                                   [[W, 128], [H * W, C], [1, W]]))
            # down-shifted copy: tfdn[y] = tf[y+1]; row 127 = 0
            tfdn = pr.tile([128, C * W], F16, tag="tfdn")
            nc.sync.dma_start(out=tfdn[0:127, :], in_=tf[1:128, :])
            nc.sync.dma_start(out=tfdn[127:128, :],
                              in_=_ap(ztile[:], 0, [[PADX * C, 1], [1, C * W]]))

            copy_fns = [nc.vector.tensor_copy,
                        lambda out, in_: nc.scalar.copy(out=out, in_=in_),
                        nc.gpsimd.tensor_copy,
                        lambda out, in_: nc.scalar.copy(out=out, in_=in_)]
            i = 0
            for dy, src in ((0, tf), (1, tfdn)):
                for dx in (0, 1):
                    # staged[y, xp, dy, dx, c] = src[y, c, xp-2+dx]
                    xp_lo = 2 - dx
                    n_xp = 128
                    src_ap = _ap(src[:], 0,
                                 [[C * W, 128], [1, n_xp], [W, C]])
                    dst_ap = _ap(staged[:], xp_lo * CH + dy * 32 + dx * 16,
                                 [[PW * CH, 128], [CH, n_xp], [1, C]])
                    copy_fns[i](out=dst_ap, in_=src_ap)
                    i += 1
            # dump rows 2..129 of the pos grid (partition y -> row y+2)
            d1 = nc.sync.dma_start(
                out=_ap(pmap[v].ap(), 2 * PW * CH,
                        [[PW * CH, 128], [1, PW * CH]]),
                in_=staged[:])
            # zero rows 0, 130, 131 from ztile (128x132 slice covers a row)
            dz = []
            for r in (0, 130, 131):
                dz.append(nc.sync.dma_start(
                    out=_ap(pmap[v].ap(), r * PW * CH,
                            [[PW * CH, 1], [1, PW * CH]]),
                    in_=_ap(ztile[:], 0, [[PADX * C, 8], [1, PW * CH // 8]])))
            # row r=1 (y0=-1): dy=1 slots hold F row 0 = staged[0]'s dy=0
            # window (same cells, slot offset +32); rest zero
            d4z = nc.sync.dma_start(
                out=_ap(pmap[v].ap(), PW * CH, [[PW * CH, 1], [1, PW * CH]]),
                in_=_ap(ztile[:], 0, [[PADX * C, 8], [1, PW * CH // 8]]))
            d4 = nc.sync.dma_start(
                out=_ap(pmap[v].ap(), PW * CH + 32,
                        [[PW * CH, 1], [CH, PW], [1, 32]]),
                in_=_ap(staged[0:1, :], 0, [[PW * CH, 1], [CH, PW], [1, 32]]))
            add_dep_helper(d4.ins, d4z.ins, reason="row1 zero before windows")
            build_dumps.append(tuple([d1, d4, d4z] + dz))

        # ---------------- w_feat volume [y, (t25, x)] fp16 ----
        wf25 = pp.tile([128, TM], F16, tag="wf25")
        nc.vector.memset(wf25[:, 12 * W:13 * W], 0.0)
        for ty in (2, 1, 3, 0, 4):
            for tx in range(5):
                t = ty * 5 + tx
                if t == 12:
                    continue
                # diff = ref[y,x,c] - refSC[ty][y, x+tx, c]
                dfw = pr.tile([128, W * C], F16, tag="wfdf")
                nc.vector.tensor_tensor(
                    out=dfw[:],
                    in0=_ap(refSC[ty][:], tx * C,
                            [[PADX * C, 128], [C, W], [1, C]]),
                    in1=refC[:], op=OP.subtract)
                sqw = pr.tile([128, W * C], F16, tag="wfsq")
                nc.scalar.activation(out=sqw[:], in_=dfw[:], func=AF.Square)
                # c16 tree reduction (packed fp16 halves)
                c8 = pr.tile([128, W * 8], F16, tag="c8w")
                nc.vector.tensor_tensor(
                    out=_ap(c8[:], 0, [[W * 8, 128], [8, W], [1, 8]]),
                    in0=_ap(sqw[:], 0, [[W * C, 128], [C, W], [1, 8]]),
                    in1=_ap(sqw[:], 8, [[W * C, 128], [C, W], [1, 8]]),
                    op=OP.add)
                c4 = pr.tile([128, W * 4], F16, tag="c4w")
                nc.vector.tensor_tensor(
                    out=_ap(c4[:], 0, [[W * 4, 128], [4, W], [1, 4]]),
                    in0=_ap(c8[:], 0, [[W * 8, 128], [8, W], [1, 4]]),
                    in1=_ap(c8[:], 4, [[W * 8, 128], [8, W], [1, 4]]),
                    op=OP.add)
                c2 = pr.tile([128, W * 2], F16, tag="c2w")
                nc.vector.tensor_tensor(
                    out=_ap(c2[:], 0, [[W * 2, 128], [2, W], [1, 2]]),
                    in0=_ap(c4[:], 0, [[W * 4, 128], [4, W], [1, 2]]),
                    in1=_ap(c4[:], 2, [[W * 4, 128], [4, W], [1, 2]]),
                    op=OP.add)
                cs = pr.tile([128, W], F16, tag="csw")
                nc.vector.tensor_tensor(
                    out=cs[:],
                    in0=_ap(c2[:], 0, [[W * 2, 128], [2, W]]),
                    in1=_ap(c2[:], 1, [[W * 2, 128], [2, W]]),
                    op=OP.add)
                # wf = sqrt(cs) = exp(0.5*ln(cs))  (ln(0) -> -inf -> exp 0)
                lnw = pr.tile([128, W], F16, tag="lnw")
                nc.scalar.activation(out=lnw[:], in_=cs[:], func=AF.Ln)
                nc.scalar.activation(out=wf25[:, t * W:(t + 1) * W],
                                     in_=lnw[:], func=AF.Exp, scale=tv(10))


        prep.__exit__(None, None, None)
        wp = pool("work", 2)        # small per-(b,d) working tiles
        bp = pool("big4", 4)        # shared 4KB scratch (tag s4k)
        cp = pool("cumdif", 1)      # cum/diff accumulators
        gp = pool("gath", 2)        # gather destinations
        ap_ = pool("aggp", 2)       # agg volumes

        # wrapped-16 gather indices; partitions 16.. stay 0 forever
        wr16ab = []
        for nm in ("wr16a", "wr16b"):
            t = pp.tile([128, W * 8], I16, tag=nm)
            nc.vector.memset(t[:], 0.0)
            wr16ab.append(t)

        # depth planes f32 [y,(x,d)] and agg store
        depD = pp.tile([128, W * DPC], F32, tag="depD")
        aggT = pp.tile([128, W * DPC], F32, tag="aggT")

        # ---------------- per depth-plane pipeline ----------------
        for di in range(DPC):
            depf = wp.tile([128, W], F32, tag="depf")
            nc.sync.dma_start(out=depf[:], in_=dep.ap()[di])
            nc.vector.tensor_copy(
                out=_ap(depD[:], di, [[W * DPC, 128], [DPC, W], [1, 1]]),
                in_=depf[:])
            # depSC5 [y, (ty5, xp132)] fp16: center block + 4 shifted blocks
            depSC5 = wp.tile([128, 5 * PADX], F16, tag="depSC5")
            nc.scalar.copy(
                out=_ap(depSC5[:], 2 * PADX + 2, [[5 * PADX, 128], [1, W]]),
                in_=depf[:])
            nc.vector.memset(
                _ap(depSC5[:], 2 * PADX, [[5 * PADX, 128], [130, 2], [1, 2]]),
                0.0)
            for ty in (0, 1, 3, 4):
                k = ty - 2
                dlo = ty * PADX
                clo = 2 * PADX
                if k < 0:
                    nc.sync.dma_start(
                        out=depSC5[-k:128, dlo:dlo + PADX],
                        in_=depSC5[0:128 + k, clo:clo + PADX])
                    nc.sync.dma_start(out=depSC5[0:-k, dlo:dlo + PADX],
                                      in_=ztile[0:-k, 0:PADX])
                else:
                    nc.sync.dma_start(
                        out=depSC5[0:128 - k, dlo:dlo + PADX],
                        in_=depSC5[k:128, clo:clo + PADX])
                    nc.sync.dma_start(out=depSC5[128 - k:128, dlo:dlo + PADX],
                                      in_=ztile[0:k, 0:PADX])

            cum = cp.tile([128, W * C], F16, tag="cum")
            diff = cp.tile([128, W * C], F16, tag="diff")
            csq = {}
            for v in range(NV):
                # ---- coordinates (pixel-major [y,x] f32) ----
                def arow(r):
                    return _ap(amapL[:], (v * 3 + r) * W,
                               [[NV * 3 * W, 128], [1, W]])
                mx = wp.tile([128, W], F32, tag="mx")
                my = wp.tile([128, W], F32, tag="my")
                dn = wp.tile([128, W], F32, tag="dn")
                nc.vector.tensor_tensor(out=mx[:], in0=arow(0), in1=depf[:],
                                        op=OP.mult)
                nc.vector.tensor_tensor(out=my[:], in0=arow(1), in1=depf[:],
                                        op=OP.mult)
                nc.vector.tensor_tensor(out=dn[:], in0=arow(2), in1=depf[:],
                                        op=OP.mult)
                nx = wp.tile([128, W], F32, tag="nx")
                ny = wp.tile([128, W], F32, tag="ny")
                dnt = wp.tile([128, W], F32, tag="dnt")
                nc.scalar.activation(out=nx[:], in_=mx[:], func=AF.Identity,
                                     bias=tv(v * 3 + 0))
                nc.scalar.activation(out=ny[:], in_=my[:], func=AF.Identity,
                                     bias=tv(v * 3 + 1))
                nc.scalar.activation(out=dnt[:], in_=dn[:], func=AF.Identity,
                                     bias=tv(v * 3 + 2))
                rec = wp.tile([128, W], F32, tag="rec")
                nc.vector.reciprocal(out=rec[:], in_=dnt[:])
                gx = wp.tile([128, W], F32, tag="gx")
                gy = wp.tile([128, W], F32, tag="gy")
                nc.vector.tensor_tensor(out=gx[:], in0=nx[:], in1=rec[:],
                                        op=OP.mult)
                nc.vector.tensor_tensor(out=gy[:], in0=ny[:], in1=rec[:],
                                        op=OP.mult)

                # xs = trunc(g + 1.5) = floor(g - 0.5) + 2 for g - 0.5 >= -2
                # (fully-OOB positions clamp into the zero border either way)
                def floorfrac(g, nm):
                    xi = wp.tile([128, W], I32, tag=f"i{nm}")
                    nc.scalar.activation(out=xi[:], in_=g[:],
                                         func=AF.Identity, bias=tv(6))
                    xf = wp.tile([128, W], F32, tag=f"xf{nm}")
                    nc.scalar.copy(out=xf[:], in_=xi[:])
                    fr = wp.tile([128, W], F32, tag=f"fr{nm}")
                    nc.vector.scalar_tensor_tensor(
                        out=fr[:], in0=g[:], scalar=1.5, in1=xf[:],
                        op0=OP.add, op1=OP.subtract)
                    xc = wp.tile([128, W], F32, tag=f"xc{nm}")
                    nc.vector.tensor_scalar(out=xc[:], in0=xf[:],
                                            scalar1=131.0, scalar2=0.0,
                                            op0=OP.min, op1=OP.max)
                    return xc, fr

                xsC, fx = floorfrac(gx, "x")
                ysC, fy = floorfrac(gy, "y")
                posF = wp.tile([128, W], F32, tag="posF")
                nc.vector.scalar_tensor_tensor(
                    out=posF[:], in0=ysC[:], scalar=float(PW), in1=xsC[:],
                    op0=OP.mult, op1=OP.add)

                # ---- fold posF [y, x] -> wrapped16 idx [16, x*8+xh] ----
                wr16 = wr16ab[(di * NV + v) % 2]
                for grp in range(2):
                    ps4 = psp.tile([16, W * 4], F32, tag="ps4")
                    for j in range(4):
                        xh = grp * 4 + j
                        nc.tensor.matmul(out=ps4[0:16, j * W:(j + 1) * W],
                                         lhsT=identT[:, 16 * xh:16 * xh + 16],
                                         rhs=posF[:], start=True, stop=True)
                    # Act copy f32->i16 (trunc; values are exact ints)
                    nc.scalar.copy(
                        out=_ap(wr16[:], grp * 4,
                                [[W * 8, 16], [8, W], [1, 4]]),
                        in_=_ap(ps4[:], 0, [[W * 4, 16], [1, W], [W, 4]]))

                # ---- gather (dma_gather; idx partitions 16.. are zero) ----
                G = gp.tile([128, W * CH], F16, tag="G")
                for t in range(4):
                    gi = nc.gpsimd.dma_gather(
                        out_ap=_ap(G[:], t * 32 * CH,
                                   [[W * CH, 128], [CH, 32], [1, CH]]),
                        in_ap=pmap[v].ap(),
                        idxs_ap=_ap(wr16[:], t * 32 * 8,
                                    [[W * 8, 128], [1, 32 * 8]]),
                        num_idxs=32 * 128, num_idxs_reg=32 * 128,
                        elem_size=CH, queue_num=0)
                    for dma_i in build_dumps[v]:
                        add_dep_helper(gi.ins, dma_i.ins,
                                       reason="patch map build before gather")

                # fp16 fractional weights (cast/affine on Act engine)
                fx16 = wp.tile([128, W], F16, tag="fx16")
                fy16 = wp.tile([128, W], F16, tag="fy16")
                fxm16 = wp.tile([128, W], F16, tag="fxm16")
                fym16 = wp.tile([128, W], F16, tag="fym16")
                nc.scalar.copy(out=fx16[:], in_=fx[:])
                nc.scalar.copy(out=fy16[:], in_=fy[:])
                nc.scalar.activation(out=fxm16[:], in_=fx[:],
                                     func=AF.Identity, scale=tv(9),
                                     bias=tv(8))
                nc.scalar.activation(out=fym16[:], in_=fy[:],
                                     func=AF.Identity, scale=tv(9),
                                     bias=tv(8))
                wd = {}
                for (tnm, fa, fb) in (("00", fxm16, fym16),
                                      ("01", fx16, fym16),
                                      ("10", fxm16, fy16),
                                      ("11", fx16, fy16)):
                    wt = wp.tile([128, W * 2], F16, tag=f"wd{tnm}")
                    nc.vector.tensor_tensor(
                        out=_ap(wt[:], 0, [[W * 2, 128], [2, W], [1, 2]]),
                        in0=_ap(fa[:], 0, [[W, 128], [1, W], [0, 2]]),
                        in1=_ap(fb[:], 0, [[W, 128], [1, W], [0, 2]]),
                        op=OP.mult)
                    wd[tnm] = wt

                # ---- bilinear taps: acc = sum_t w_t * G_t  [y,(x,c)] ----
                def tap(dy, dx):
                    return _ap(G[:], (dy * 2 + dx) * 16,
                               [[W * CH, 128], [CH, W], [2, 8], [1, 2]])

                def wap(tnm):
                    return _ap(wd[tnm][:], 0,
                               [[W * 2, 128], [2, W], [0, 8], [1, 2]])

                cview = [[2048, 128], [16, W], [2, 8], [1, 2]]
                acc = cum if v == 0 else bp.tile([128, W * C], F16, tag="s4k")
                p0 = bp.tile([128, W * C], F16, tag="s4k")
                p1 = bp.tile([128, W * C], F16, tag="s4k")
                nc.vector.tensor_tensor(out=_ap(acc[:], 0, cview),
                                        in0=tap(0, 0), in1=wap("00"),
                                        op=OP.mult)
                nc.vector.tensor_tensor(out=_ap(p0[:], 0, cview),
                                        in0=tap(0, 1), in1=wap("01"),
                                        op=OP.mult)
                nc.vector.tensor_tensor(out=_ap(p1[:], 0, cview),
                                        in0=tap(1, 0), in1=wap("10"),
                                        op=OP.mult)
                nc.vector.tensor_tensor(out=acc[:], in0=acc[:], in1=p0[:],
                                        op=OP.add)
                nc.vector.tensor_tensor(out=_ap(p0[:], 0, cview),
                                        in0=tap(1, 1), in1=wap("11"),
                                        op=OP.mult)
                nc.vector.tensor_tensor(out=p1[:], in0=p1[:], in1=p0[:],
                                        op=OP.add)
                nc.vector.tensor_tensor(out=acc[:], in0=acc[:], in1=p1[:],
                                        op=OP.add)

                # ---- cost_v = sum_c (ref - cum)^2 (Act square + DVE tree) --
                if v == 0:
                    nc.vector.tensor_tensor(out=diff[:], in0=refC[:],
                                            in1=cum[:], op=OP.subtract)
                else:
                    nc.vector.tensor_tensor(out=diff[:], in0=diff[:],
                                            in1=acc[:], op=OP.subtract)
                sq = bp.tile([128, W * C], F16, tag="s4k")
                nc.scalar.activation(out=sq[:], in_=diff[:], func=AF.Square)
                c8 = wp.tile([128, W * 8], F16, tag="c8")
                nc.vector.tensor_tensor(
                    out=_ap(c8[:], 0, [[W * 8, 128], [8, W], [1, 8]]),
                    in0=_ap(sq[:], 0, [[W * C, 128], [C, W], [1, 8]]),
                    in1=_ap(sq[:], 8, [[W * C, 128], [C, W], [1, 8]]),
                    op=OP.add)
                c4 = wp.tile([128, W * 4], F16, tag="c4")
                nc.vector.tensor_tensor(
                    out=_ap(c4[:], 0, [[W * 4, 128], [4, W], [1, 4]]),
                    in0=_ap(c8[:], 0, [[W * 8, 128], [8, W], [1, 4]]),
                    in1=_ap(c8[:], 4, [[W * 8, 128], [8, W], [1, 4]]),
                    op=OP.add)
                c2 = wp.tile([128, W * 2], F16, tag="c2")
                nc.vector.tensor_tensor(
                    out=_ap(c2[:], 0, [[W * 2, 128], [2, W], [1, 2]]),
                    in0=_ap(c4[:], 0, [[W * 4, 128], [4, W], [1, 2]]),
                    in1=_ap(c4[:], 2, [[W * 4, 128], [4, W], [1, 2]]),
                    op=OP.add)
                cs = wp.tile([128, W], F16, tag=f"csq{v}")
                nc.vector.tensor_tensor(
                    out=cs[:],
                    in0=_ap(c2[:], 0, [[W * 2, 128], [2, W]]),
                    in1=_ap(c2[:], 1, [[W * 2, 128], [2, W]]),
                    op=OP.add)
                csq[v] = cs

            # cost = sqrt(min(c1sq, c2sq)) into costSC5 center block
            cmin = wp.tile([128, W], F16, tag="cmin")
            nc.vector.tensor_tensor(out=cmin[:], in0=csq[0][:], in1=csq[1][:],
                                    op=OP.min)
            lnc = wp.tile([128, W], F16, tag="lnc")
            nc.scalar.activation(out=lnc[:], in_=cmin[:], func=AF.Ln)
            costSC5 = wp.tile([128, 5 * PADX], F16, tag="costSC5")
            nc.scalar.activation(
                out=_ap(costSC5[:], 2 * PADX + 2, [[5 * PADX, 128], [1, W]]),
                in_=lnc[:], func=AF.Exp, scale=tv(10))
            nc.vector.memset(
                _ap(costSC5[:], 2 * PADX, [[5 * PADX, 128], [130, 2], [1, 2]]),
                0.0)
            for ty in (0, 1, 3, 4):
                k = ty - 2
                dlo = ty * PADX
                clo = 2 * PADX
                if k < 0:
                    nc.sync.dma_start(
                        out=costSC5[-k:128, dlo:dlo + PADX],
                        in_=costSC5[0:128 + k, clo:clo + PADX])
                    nc.sync.dma_start(out=costSC5[0:-k, dlo:dlo + PADX],
                                      in_=ztile[0:-k, 0:PADX])
                else:
                    nc.sync.dma_start(
                        out=costSC5[0:128 - k, dlo:dlo + PADX],
                        in_=costSC5[k:128, clo:clo + PADX])
                    nc.sync.dma_start(out=costSC5[128 - k:128, dlo:dlo + PADX],
                                      in_=ztile[0:k, 0:PADX])

            # ---- aggregation, tap-major [y, (t25, x)] ----
            dvol = ap_.tile([128, TM], F16, tag="dvol")
            nc.vector.tensor_tensor(
                out=_ap(dvol[:], 0, [[TM, 128], [5 * W, 5], [W, 5], [1, W]]),
                in0=_ap(depSC5[:], 0,
                        [[5 * PADX, 128], [PADX, 5], [1, 5], [1, W]]),
                in1=_ap(depSC5[:], 2 * PADX + 2,
                        [[5 * PADX, 128], [0, 5], [0, 5], [1, W]]),
                op=OP.subtract)
            nc.scalar.activation(out=dvol[:], in_=dvol[:], func=AF.Abs)
            evol = ap_.tile([128, TM], F16, tag="evol")
            nc.scalar.activation(out=evol[:], in_=dvol[:], func=AF.Exp,
                                 bias=tv(7))
            uvol = dvol  # reuse: dvol's last reader is the exp above
            nc.vector.tensor_tensor(out=uvol[:], in0=evol[:], in1=wf25[:],
                                    op=OP.mult)
            nc.vector.tensor_tensor(
                out=_ap(uvol[:], 0, [[TM, 128], [5 * W, 5], [W, 5], [1, W]]),
                in0=_ap(uvol[:], 0, [[TM, 128], [5 * W, 5], [W, 5], [1, W]]),
                in1=_ap(costSC5[:], 0,
                        [[5 * PADX, 128], [PADX, 5], [1, 5], [1, W]]),
                op=OP.mult)

            def tap_tree(vol, nm):
                s12 = wp.tile([128, 12 * W], F16, tag="s12")
                nc.vector.tensor_tensor(out=s12[:], in0=vol[:, 0:12 * W],
                                        in1=vol[:, 12 * W:24 * W], op=OP.add)
                s6 = wp.tile([128, 6 * W], F16, tag="s6")
                nc.vector.tensor_tensor(out=s6[:], in0=s12[:, 0:6 * W],
                                        in1=s12[:, 6 * W:12 * W], op=OP.add)
                s3 = wp.tile([128, 3 * W], F16, tag="s3")
                nc.vector.tensor_tensor(out=s3[:], in0=s6[:, 0:3 * W],
                                        in1=s6[:, 3 * W:6 * W], op=OP.add)
                r = wp.tile([128, W], F16, tag=f"r{nm}")
                nc.vector.tensor_tensor(out=r[:], in0=s3[:, 0:W],
                                        in1=s3[:, W:2 * W], op=OP.add)
                nc.vector.tensor_tensor(out=r[:], in0=r[:],
                                        in1=s3[:, 2 * W:3 * W], op=OP.add)
                nc.vector.tensor_tensor(out=r[:], in0=r[:],
                                        in1=vol[:, 24 * W:25 * W], op=OP.add)
                return r

            num = tap_tree(uvol, "n")
            den = tap_tree(evol, "d")
            rden = wp.tile([128, W], F32, tag="rden")
            nc.vector.reciprocal(out=rden[:], in_=den[:])
            agg_ap = _ap(aggT[:], di, [[W * DPC, 128], [DPC, W], [1, 1]])
            nc.vector.tensor_tensor(out=agg_ap, in0=num[:], in1=rden[:],
                                    op=OP.mult)

        # ---------------- per-core softmax partials ----------------
        def aggap(di):
            return _ap(aggT[:], di, [[W * DPC, 128], [DPC, W], [1, 1]])

        def depap(di):
            return _ap(depD[:], di, [[W * DPC, 128], [DPC, W], [1, 1]])

        m = pp.tile([128, W], F32, tag="m")
        nc.vector.tensor_tensor(out=m[:], in0=aggap(0), in1=aggap(1),
                                op=OP.max)
        for di in range(2, DPC):
            nc.vector.tensor_tensor(out=m[:], in0=m[:], in1=aggap(di),
                                    op=OP.max)
        s0 = pp.tile([128, W], F32, tag="s0")
        s1 = pp.tile([128, W], F32, tag="s1")
        for di in range(DPC):
            t = wp.tile([128, W], F32, tag="et")
            nc.vector.tensor_tensor(out=t[:], in0=aggap(di), in1=m[:],
                                    op=OP.subtract)
            e = wp.tile([128, W], F32, tag="ee")
            nc.scalar.activation(out=e[:], in_=t[:], func=AF.Exp)
            t1 = wp.tile([128, W], F32, tag="t1")
            nc.vector.tensor_tensor(out=t1[:], in0=e[:], in1=depap(di),
                                    op=OP.mult)
            if di == 0:
                nc.vector.tensor_copy(out=s0[:], in_=e[:])
                nc.vector.tensor_copy(out=s1[:], in_=t1[:])
            else:
                nc.vector.tensor_tensor(out=s0[:], in0=s0[:], in1=e[:],
                                        op=OP.add)
                nc.vector.tensor_tensor(out=s1[:], in0=s1[:], in1=t1[:],
                                        op=OP.add)
        nc.sync.dma_start(out=out3.ap()[0], in_=m[:])
        nc.sync.dma_start(out=out3.ap()[1], in_=s0[:])
        nc.sync.dma_start(out=out3.ap()[2], in_=s1[:])

        for p in reversed(ctx_pools):
            p.__exit__(None, None, None)

    nc.compile()
    return nc


def host_prep(features, intrinsics, cam_to_world, depth_hypo):
    """Build the 8 per-core input maps. All O(small) except slicing."""
    features = np.asarray(features, dtype=np.float32)
    intrinsics = np.asarray(intrinsics, dtype=np.float32)
    cam_to_world = np.asarray(cam_to_world, dtype=np.float32)
    depth_hypo = np.asarray(depth_hypo, dtype=np.float32)

    ys, xs = np.meshgrid(np.arange(H, dtype=np.float32),
                         np.arange(W, dtype=np.float32), indexing="ij")
    in_maps = []
    for k in range(NCORES):
        b = k // (NCORES // B)
        dlo = DPC * (k % (NCORES // B))
        amap = np.zeros((NV, 3, H, W), np.float32)
        tvv = np.zeros((12,), np.float32)
        for vi in range(1, V):
            src_w2c = np.linalg.inv(cam_to_world[vi, b])
            ref_w2c = np.linalg.inv(cam_to_world[0, b])
            src_KK = src_w2c.copy()
            src_KK[:3, :3] = intrinsics[vi, b]
            ref_KK = ref_w2c.copy()
            ref_KK[:3, :3] = intrinsics[0, b]
            proj = (src_KK @ src_w2c) @ np.linalg.inv(ref_KK @ ref_w2c)
            rot, trans = proj[:3, :3], proj[:3, 3]
            A = (rot[:, 0:1, None] * xs[None] + rot[:, 1:2, None] * ys[None]
                 + rot[:, 2:3, None])  # [3, H, W]
            v = vi - 1
            amap[v, 0] = A[0] * SCALE
            amap[v, 1] = A[1] * SCALE
            amap[v, 2] = A[2]
            tvv[v * 3 + 0] = trans[0] * SCALE
            tvv[v * 3 + 1] = trans[1] * SCALE
            tvv[v * 3 + 2] = trans[2]
        tvv[6] = 1.5      # floor-by-trunc bias (+2 border shift - 0.5 center)
        tvv[7] = ESHIFT
        tvv[8] = 1.0
        tvv[9] = -1.0
        tvv[10] = 0.5
        in_maps.append({
            "refF": np.ascontiguousarray(features[0, b].transpose(0, 2, 1)),
            "srcF": np.ascontiguousarray(features[1:, b]),
            "dep": np.ascontiguousarray(
                depth_hypo[b, dlo:dlo + DPC].transpose(0, 2, 1)),
            "amap": np.ascontiguousarray(amap.transpose(0, 1, 3, 2)),
            "tvec": np.tile(tvv[None, :], (128, 1)).astype(np.float32),
            "ident": np.eye(128, dtype=np.float32),
        })
    return in_maps


def host_combine(results):
    """Merge per-core softmax partials (m, s0, s1) into [B, H, W]."""
    out = np.zeros((B, H, W), np.float32)
    per_b = NCORES // B
    for b in range(B):
        parts = [np.asarray(results[b * per_b + j]["out3"]) for j in range(per_b)]
        parts = [p.transpose(0, 2, 1) for p in parts]  # [3, x, y] -> [3, y, x]
        ms = np.stack([p[0] for p in parts])         # [4, H, W]
        M = ms.max(axis=0)
        S0 = np.zeros((H, W), np.float64)
        S1 = np.zeros((H, W), np.float64)
        for p in parts:
            w = np.exp(p[0] - M)
            S0 += w * p[1]
            S1 += w * p[2]
        out[b] = (S1 / S0).astype(np.float32)
    return out


def _run_sim(nc, in_maps):
    from concourse.bass_interp import CoreSim
    results = []
    for core in range(NCORES):
        sim = CoreSim(nc, trace=False, publish_trace=False)
        for k, v in in_maps[core].items():
            sim.tensor(k)[:] = v
        sim.simulate()
        results.append({"out3": np.array(sim.tensor("out3"))})
    return results


def _get_exec():
    """Build the Bass program once and wrap it in a CACHED jitted
    shard_map executable (the stock run_bass_kernel_spmd path rebuilds
    the jax closure per call -> full retrace + compile every time)."""
    if "exec" in _cached:
        return _cached["exec"]
    import jax
    import concourse.mybir as _mybir
    from concourse import bass2jax
    from jax.experimental.shard_map import shard_map
    from jax.sharding import Mesh, NamedSharding, PartitionSpec

    nc = build_program()
    bass2jax.install_neuronx_cc_hook()

    partition_name = (nc.partition_id_tensor.name
                      if nc.partition_id_tensor else None)
    in_names, out_names, out_avals = [], [], []
    for alloc in nc.m.functions[0].allocations:
        if not isinstance(alloc, _mybir.MemoryLocationSet):
            continue
        name = alloc.memorylocations[0].name
        if alloc.kind == "ExternalInput":
            if name != partition_name:
                in_names.append(name)
        elif alloc.kind == "ExternalOutput":
            shape = tuple(alloc.tensor_shape)
            dtype = _mybir.dt.np(alloc.dtype)
            out_names.append(name)
            out_avals.append(jax.core.ShapedArray(shape, dtype))
    n_params = len(in_names)
    all_names = list(in_names) + list(out_names)
    if partition_name is not None:
        all_names.append(partition_name)
    donate = tuple(range(n_params, n_params + len(out_names)))

    def _body(*args):
        operands = list(args)
        if partition_name is not None:
            operands.append(bass2jax.partition_id_tensor())
        outs = bass2jax._bass_exec_p.bind(
            *operands,
            out_avals=tuple(out_avals),
            in_names=tuple(all_names),
            out_names=tuple(out_names),
            lowering_input_output_aliases=(),
            sim_require_finite=True,
            sim_require_nnan=True,
            nc=nc,
        )
        return tuple(outs)

    devices = jax.devices()[:NCORES]
    mesh = Mesh(np.asarray(devices), ("core",))
    n_in = n_params + len(out_names)
    jitted = jax.jit(
        shard_map(_body, mesh=mesh,
                  in_specs=(PartitionSpec("core"),) * n_in,
                  out_specs=(PartitionSpec("core"),) * len(out_names),
                  check_rep=False),
        donate_argnums=donate, keep_unused=True)
    sharding = NamedSharding(mesh, PartitionSpec("core"))
    st = {"nc": nc, "jitted": jitted, "in_names": in_names,
          "out_names": out_names, "out_avals": out_avals,
          "dbg_name": nc.dbg_addr.name if nc.dbg_addr is not None else None,
          "sharding": sharding}
    _cached["exec"] = st
    return st


def _concat_inputs(st, in_maps):
    dbg = st["dbg_name"]
    if dbg is not None:
        in_maps = [{**m, dbg: np.zeros((1, 2), np.uint32)} for m in in_maps]
    return [np.concatenate([np.asarray(m[name]) for m in in_maps], axis=0)
            for name in st["in_names"]]


def _zero_outs(st):
    return [np.zeros((NCORES * a.shape[0],) + tuple(a.shape[1:]), a.dtype)
            for a in st["out_avals"]]


def _run_cached(st, in_maps):
    out_arrs = st["jitted"](*_concat_inputs(st, in_maps), *_zero_outs(st))
    res = []
    for c in range(NCORES):
        res.append({name: np.asarray(out_arrs[i]).reshape(
            (NCORES,) + tuple(st["out_avals"][i].shape))[c]
            for i, name in enumerate(st["out_names"])})
    return res


def kernel(**inputs):
    in_maps = host_prep(**inputs)
    if _cached.get("hw_broken"):
        if "nc" not in _cached:
            _cached["nc"] = build_program()
        return host_combine(_run_sim(_cached["nc"], in_maps))
    try:
        st = _get_exec()
        return host_combine(_run_cached(st, in_maps))
    except Exception:
        try:
            if "nc" not in _cached:
                _cached["nc"] = build_program()
            res = run_bass_kernel_spmd(_cached["nc"], in_maps,
                                       core_ids=list(range(NCORES)))
            return host_combine(res.results)
        except Exception:
            _cached["hw_broken"] = True
            return host_combine(_run_sim(_cached["nc"], in_maps))


if __name__ == "__main__":
    import reference
    inp = reference.setup_inputs()
    inp = {k: np.asarray(v) for k, v in inp.items()}
    out = kernel(**inp)
    print("kernel out", out.shape, out.dtype)



# revision 27
# speedup vs baseline: 448.0613x; 269.7617x over previous
"""Trainium2 Bass kernel for nn_DepthPrediction (multi-view stereo depth).

Strategy (8 NeuronCores, SPMD single program):
  - Shard: core k handles batch b = k//4 and depth planes [8*(k%4), 8*(k%4)+8).
  - Per (b,d): homography warp of 2 src views via on-device fp16 "patch maps"
    (132x132 positions x [dy2,dx2,c16] fp16 chunks = 128B) gathered with ONE
    standard indirect DMA (DynamicAP, 16K indices) per (plane, view);
    bilinear interp + cumulative cost (L2 over 16 ch) in pixel-major fp16 on
    DVE; 5x5 adaptive aggregation (depth-similarity softmax x
    feature-similarity weight) in tap-major layout; per-core softmax
    partials over the 8 local planes.
  - Engine budget: DVE does the packed-fp16 tensor math; Activation does
    affine/exp/ln/square/casts (single act table: natural_log_exp);
    Pool does the indirect gathers + memsets; SP does DMAs.
  - NOTE: custom-ucode GPSIMD ops (dma_gather/ap_gather) hard-crash this
    runtime (NRT_EXEC_UNIT_UNRECOVERABLE) — only stock-NEFF instructions
    are used here.
  - Host: trivial glue — 4x4 matrix algebra, shard/pack inputs, combine the
    4-way softmax partials per batch (log-sum-exp merge) into the output.
  - Execution: the jitted shard_map executable is built ONCE and cached;
    repeat kernel() calls reuse it (the stock per-call path retraces).

Self-contained: hardcodes all shapes from the problem spec.
"""

import numpy as np

import concourse.bacc as bacc
import concourse.bass as bass
import concourse.mybir as mybir
import concourse.tile as tile
from concourse.bass_utils import run_bass_kernel_spmd
from concourse.tile_rust import add_dep_helper

F32 = mybir.dt.float32
F16 = mybir.dt.float16
I32 = mybir.dt.int32
I16 = mybir.dt.int16

AF = mybir.ActivationFunctionType
OP = mybir.AluOpType
AX = mybir.AxisListType

# problem shapes
V, B, C, H, W, D = 3, 2, 16, 128, 128, 32
NCORES = 8
DPC = D // (NCORES // B)  # depth planes per core = 8
NV = V - 1  # src views = 2

PW = W + 4          # padded map width (x0 in [-2,129] -> cols 0..131)
POS = PW * PW       # patch positions
CH = 64             # chunk elems: [dy2,dx2,c16] fp16 = 128B
PADX = W + 4        # x-padded tiles for 5-tap aggregation
SCALE = W / (W - 1.0)  # grid_sample align_corners=False fold
ESHIFT = -9.0       # exp(|dnb-d|) stabilization shift (|delta| <= 9)
NT = 25             # aggregation taps
TM = NT * W         # tap-major volume free size

_cached = {}


def _ap(base, off, dims):
    """Raw AP on the same tensor as `base` (an AP), offset in elements."""
    return bass.AP(base.tensor, base.offset + off, dims)


def build_program(debug=False):
    # NOTE: do NOT reorder the get_activation_tables() dict — the emitted
    # act_func_set_id indexes the ORIGINAL act_info.json order; walrus
    # resolves ids against that file, so a reorder makes HW load the wrong
    # table (Ln evaluated off the exp table = garbage). CoreSim ignores
    # tables, which hid this.
    nc = bacc.Bacc("TRN2", target_bir_lowering=False, debug=False,
                   num_devices=NCORES)

    refF = nc.dram_tensor("refF", [C, H, W], F32, kind="ExternalInput")
    srcF = nc.dram_tensor("srcF", [NV, C, H, W], F32, kind="ExternalInput")
    dep = nc.dram_tensor("dep", [DPC, H, W], F32, kind="ExternalInput")
    amap = nc.dram_tensor("amap", [NV, 3, H, W], F32, kind="ExternalInput")
    tvec = nc.dram_tensor("tvec", [128, 12], F32, kind="ExternalInput")
    out3 = nc.dram_tensor("out3", [3, H, W], F32, kind="ExternalOutput")
    # internal patch maps, one per src view
    pmap = [nc.dram_tensor(f"pmap{v}", [POS, CH], F16, kind="Internal")
            for v in range(NV)]
    dbg = {}
    if debug:
        for nm, shape, dt in (
                ("dbgPos", [128, W], I32), ("dbgG", [128, W * CH], F16),
                ("dbgCum", [128, W * C], F16), ("dbgDiff", [128, W * C], F16),
                ("dbgCsq0", [128, W], F16), ("dbgCsq1", [128, W], F16),
                ("dbgCost", [128, 5 * PADX], F16),
                ("dbgCmin", [128, W], F16),
                ("dbgWf", [128, NT * W], F16),
                ("dbgNum", [128, W], F16), ("dbgDen", [128, W], F16),
                ("dbgAgg", [128, W * DPC], F32),
                ("dbgPmap", [POS, CH], F16)):
            dbg[nm] = nc.dram_tensor(nm, shape, dt, kind="ExternalOutput")

    with nc.allow_low_precision("fp16 pipeline by design"), \
            tile.TileContext(nc) as tc:
        ctx_pools = []

        def pool(name, bufs=1, **kw):
            p = tc.tile_pool(name=name, bufs=bufs, **kw)
            ctx_pools.append(p)
            return p.__enter__()

        pp = pool("persist", 1)     # long-lived tiles
        prep = tc.tile_pool(name="prep", bufs=1)
        pr = prep.__enter__()

        # ---------------- constant / persistent loads ----------------
        amapL = pp.tile([128, NV * 3 * W], F32, tag="amapL")  # [y,(v,row,x)]
        nc.sync.dma_start(out=amapL[:], in_=_ap(
            amap.ap(), 0, [[W, 128], [3 * H * W, NV], [H * W, 3], [1, W]]))
        tvecT = pp.tile([128, 12], F32, tag="tvecT")
        nc.sync.dma_start(out=tvecT[:], in_=tvec.ap())

        def tv(col):  # [128,1] per-partition scalar AP
            return tvecT[:, col:col + 1]

        # zero-source tile for edge DMAs (f16)
        ztile = pp.tile([128, PADX * C], F16, tag="ztile")
        nc.gpsimd.memset(ztile[:], 0.0)

        # ---------------- ref prep ----------------
        tr = pr.tile([128, C * W], F32, tag="tr")  # [y,(c,x)] f32
        nc.sync.dma_start(out=tr[:], in_=_ap(
            refF.ap(), 0, [[W, 128], [H * W, C], [1, W]]))
        # refC [y,(x,c)] fp16
        refC = pp.tile([128, W * C], F16, tag="refC")
        nc.vector.tensor_copy(
            out=_ap(refC[:], 0, [[W * C, 128], [C, W], [1, C]]),
            in_=_ap(tr[:], 0, [[C * W, 128], [1, W], [W, C]]))
        # refPadC [y,(xp132,c)] fp16, x' = x+2
        refPadC = pr.tile([128, PADX * C], F16, tag="refPadC")
        nc.vector.memset(
            _ap(refPadC[:], 0, [[PADX * C, 128], [C, 2], [1, C]]), 0.0)
        nc.vector.memset(
            _ap(refPadC[:], 130 * C, [[PADX * C, 128], [C, 2], [1, C]]), 0.0)
        nc.scalar.copy(out=refPadC[:, 2 * C:(2 + W) * C], in_=refC[:])
        # 4 partition-shifted copies (ty in {0,1,3,4}; center ty=2 = refPadC)
        refSC = {}
        for ty in (0, 1, 3, 4):
            t = pr.tile([128, PADX * C], F16, tag=f"refSC{ty}")
            k = ty - 2
            if k < 0:
                nc.sync.dma_start(out=t[-k:128, :], in_=refPadC[0:128 + k, :])
                nc.sync.dma_start(out=t[0:-k, :], in_=ztile[0:-k, :])
            else:
                nc.sync.dma_start(out=t[0:128 - k, :], in_=refPadC[k:128, :])
                nc.sync.dma_start(out=t[128 - k:128, :], in_=ztile[0:k, :])
            refSC[ty] = t
        refSC[2] = refPadC

        # ---------------- patch map build (per src view) ----------------
        build_dumps = []
        for v in range(NV):
            # staged [y, (xp132, dy2, dx2, c16, pad64)] fp16; row y -> pos
            # row y+2.  Zero only the data slots of border cols xp in
            # {0,1} u {129,130,131}; pad elems are never read downstream.
            staged = pr.tile([128, PW * CH], F16, tag=f"staged{v}")
            nc.vector.memset(
                _ap(staged[:], 0, [[PW * CH, 128], [CH, 2], [1, 64]]), 0.0)
            nc.vector.memset(
                _ap(staged[:], 129 * CH, [[PW * CH, 128], [CH, 3], [1, 64]]),
                0.0)
            # TF16 [y, (c,x)] fp16  (cast during DMA, SWDGE)
            tf = pr.tile([128, C * W], F16, tag="tf")
            nc.gpsimd.dma_start(
                out=tf[:], in_=_ap(srcF.ap(), v * C * H * W,
                                   [[W, 128], [H * W, C], [1, W]]))
            # down-shifted copy: tfdn[y] = tf[y+1]; row 127 = 0
            tfdn = pr.tile([128, C * W], F16, tag="tfdn")
            nc.sync.dma_start(out=tfdn[0:127, :], in_=tf[1:128, :])
            nc.sync.dma_start(out=tfdn[127:128, :],
                              in_=_ap(ztile[:], 0, [[PADX * C, 1], [1, C * W]]))

            copy_fns = [nc.vector.tensor_copy,
                        lambda out, in_: nc.scalar.copy(out=out, in_=in_),
                        nc.gpsimd.tensor_copy,
                        lambda out, in_: nc.scalar.copy(out=out, in_=in_)]
            i = 0
            for dy, src in ((0, tf), (1, tfdn)):
                for dx in (0, 1):
                    # staged[y, xp, dy, dx, c] = src[y, c, xp-2+dx]
                    xp_lo = 2 - dx
                    n_xp = 128
                    src_ap = _ap(src[:], 0,
                                 [[C * W, 128], [1, n_xp], [W, C]])
                    dst_ap = _ap(staged[:], xp_lo * CH + dy * 32 + dx * 16,
                                 [[PW * CH, 128], [CH, n_xp], [1, C]])
                    copy_fns[i](out=dst_ap, in_=src_ap)
                    i += 1
            # dump rows 2..129 of the pos grid (partition y -> row y+2)
            d1 = nc.sync.dma_start(
                out=_ap(pmap[v].ap(), 2 * PW * CH,
                        [[PW * CH, 128], [1, PW * CH]]),
                in_=staged[:])
            # zero rows 0, 130, 131 from ztile (128x132 slice covers a row)
            dz = []
            for r in (0, 130, 131):
                dz.append(nc.sync.dma_start(
                    out=_ap(pmap[v].ap(), r * PW * CH,
                            [[PW * CH, 1], [1, PW * CH]]),
                    in_=_ap(ztile[:], 0, [[PADX * C, 8], [1, PW * CH // 8]])))
            # row r=1 (y0=-1): dy=1 slots hold F row 0 = staged[0]'s dy=0
            # window (same cells, slot offset +32); rest zero
            d4z = nc.sync.dma_start(
                out=_ap(pmap[v].ap(), PW * CH, [[PW * CH, 1], [1, PW * CH]]),
                in_=_ap(ztile[:], 0, [[PADX * C, 8], [1, PW * CH // 8]]))
            d4 = nc.sync.dma_start(
                out=_ap(pmap[v].ap(), PW * CH + 32,
                        [[PW * CH, 1], [CH, PW], [1, 32]]),
                in_=_ap(staged[0:1, :], 0, [[PW * CH, 1], [CH, PW], [1, 32]]))
            add_dep_helper(d4.ins, d4z.ins, reason="row1 zero before windows")
            build_dumps.append(tuple([d1, d4, d4z] + dz))
            if debug and v == 0:
                dp = nc.sync.dma_start(
                    out=_ap(dbg["dbgPmap"].ap(), 0,
                            [[PW * CH, PW], [1, PW * CH]]),
                    in_=_ap(pmap[0].ap(), 0, [[PW * CH, PW], [1, PW * CH]]))
                for dma_i in build_dumps[0]:
                    add_dep_helper(dp.ins, dma_i.ins,
                                   reason="pmap build before debug dump")

        # ---------------- w_feat volume [y, (t25, x)] fp16 ----
        wf25 = pp.tile([128, TM], F16, tag="wf25")
        nc.vector.memset(wf25[:, 12 * W:13 * W], 0.0)
        for ty in (2, 1, 3, 0, 4):
            for tx in range(5):
                t = ty * 5 + tx
                if t == 12:
                    continue
                # diff = ref[y,x,c] - refSC[ty][y, x+tx, c]
                dfw = pr.tile([128, W * C], F16, tag="wfdf")
                nc.vector.tensor_tensor(
                    out=dfw[:],
                    in0=_ap(refSC[ty][:], tx * C,
                            [[PADX * C, 128], [C, W], [1, C]]),
                    in1=refC[:], op=OP.subtract)
                sqw = pr.tile([128, W * C], F16, tag="wfsq")
                nc.scalar.activation(out=sqw[:], in_=dfw[:], func=AF.Square)
                # c16 tree reduction (packed fp16 halves)
                c8 = pr.tile([128, W * 8], F16, tag="c8w")
                nc.vector.tensor_tensor(
                    out=_ap(c8[:], 0, [[W * 8, 128], [8, W], [1, 8]]),
                    in0=_ap(sqw[:], 0, [[W * C, 128], [C, W], [1, 8]]),
                    in1=_ap(sqw[:], 8, [[W * C, 128], [C, W], [1, 8]]),
                    op=OP.add)
                c4 = pr.tile([128, W * 4], F16, tag="c4w")
                nc.vector.tensor_tensor(
                    out=_ap(c4[:], 0, [[W * 4, 128], [4, W], [1, 4]]),
                    in0=_ap(c8[:], 0, [[W * 8, 128], [8, W], [1, 4]]),
                    in1=_ap(c8[:], 4, [[W * 8, 128], [8, W], [1, 4]]),
                    op=OP.add)
                c2 = pr.tile([128, W * 2], F16, tag="c2w")
                nc.vector.tensor_tensor(
                    out=_ap(c2[:], 0, [[W * 2, 128], [2, W], [1, 2]]),
                    in0=_ap(c4[:], 0, [[W * 4, 128], [4, W], [1, 2]]),
                    in1=_ap(c4[:], 2, [[W * 4, 128], [4, W], [1, 2]]),
                    op=OP.add)
                cs = pr.tile([128, W], F16, tag="csw")
                nc.vector.tensor_tensor(
                    out=cs[:],
                    in0=_ap(c2[:], 0, [[W * 2, 128], [2, W]]),
                    in1=_ap(c2[:], 1, [[W * 2, 128], [2, W]]),
                    op=OP.add)
                # wf = sqrt(cs)  (direct Sqrt act; Ln is table-hostile here)
                nc.scalar.activation(out=wf25[:, t * W:(t + 1) * W],
                                     in_=cs[:], func=AF.Sqrt)


        prep.__exit__(None, None, None)
        wp = pool("work", 2)        # small per-(b,d) working tiles
        bp = pool("big4", 4)        # shared 4KB scratch (tag s4k)
        cp = pool("cumdif", 1)      # cum/diff accumulators
        gp = pool("gath", 2)        # gather destinations
        ap_ = pool("aggp", 2)       # agg volumes

        # depth planes f32 [y,(x,d)] and agg store
        depD = pp.tile([128, W * DPC], F32, tag="depD")
        aggT = pp.tile([128, W * DPC], F32, tag="aggT")

        # ---------------- per depth-plane pipeline ----------------
        for di in range(DPC):
            depf = wp.tile([128, W], F32, tag="depf")
            nc.sync.dma_start(out=depf[:], in_=dep.ap()[di])
            nc.vector.tensor_copy(
                out=_ap(depD[:], di, [[W * DPC, 128], [DPC, W], [1, 1]]),
                in_=depf[:])
            # depSC5 [y, (ty5, xp132)] fp16: center block + 4 shifted blocks
            depSC5 = wp.tile([128, 5 * PADX], F16, tag="depSC5")
            nc.scalar.copy(
                out=_ap(depSC5[:], 2 * PADX + 2, [[5 * PADX, 128], [1, W]]),
                in_=depf[:])
            nc.vector.memset(
                _ap(depSC5[:], 2 * PADX, [[5 * PADX, 128], [130, 2], [1, 2]]),
                0.0)
            for ty in (0, 1, 3, 4):
                k = ty - 2
                dlo = ty * PADX
                clo = 2 * PADX
                if k < 0:
                    nc.sync.dma_start(
                        out=depSC5[-k:128, dlo:dlo + PADX],
                        in_=depSC5[0:128 + k, clo:clo + PADX])
                    nc.sync.dma_start(out=depSC5[0:-k, dlo:dlo + PADX],
                                      in_=ztile[0:-k, 0:PADX])
                else:
                    nc.sync.dma_start(
                        out=depSC5[0:128 - k, dlo:dlo + PADX],
                        in_=depSC5[k:128, clo:clo + PADX])
                    nc.sync.dma_start(out=depSC5[128 - k:128, dlo:dlo + PADX],
                                      in_=ztile[0:k, 0:PADX])

            cum = cp.tile([128, W * C], F16, tag="cum")
            diff = cp.tile([128, W * C], F16, tag="diff")
            csq = {}
            for v in range(NV):
                # ---- coordinates (pixel-major [y,x] f32) ----
                def arow(r):
                    return _ap(amapL[:], (v * 3 + r) * W,
                               [[NV * 3 * W, 128], [1, W]])
                mx = wp.tile([128, W], F32, tag="mx")
                my = wp.tile([128, W], F32, tag="my")
                dn = wp.tile([128, W], F32, tag="dn")
                nc.vector.tensor_tensor(out=mx[:], in0=arow(0), in1=depf[:],
                                        op=OP.mult)
                nc.vector.tensor_tensor(out=my[:], in0=arow(1), in1=depf[:],
                                        op=OP.mult)
                nc.vector.tensor_tensor(out=dn[:], in0=arow(2), in1=depf[:],
                                        op=OP.mult)
                nx = wp.tile([128, W], F32, tag="nx")
                ny = wp.tile([128, W], F32, tag="ny")
                dnt = wp.tile([128, W], F32, tag="dnt")
                nc.scalar.activation(out=nx[:], in_=mx[:], func=AF.Identity,
                                     bias=tv(v * 3 + 0))
                nc.scalar.activation(out=ny[:], in_=my[:], func=AF.Identity,
                                     bias=tv(v * 3 + 1))
                nc.scalar.activation(out=dnt[:], in_=dn[:], func=AF.Identity,
                                     bias=tv(v * 3 + 2))
                rec = wp.tile([128, W], F32, tag="rec")
                nc.vector.reciprocal(out=rec[:], in_=dnt[:])
                gx = wp.tile([128, W], F32, tag="gx")
                gy = wp.tile([128, W], F32, tag="gy")
                nc.vector.tensor_tensor(out=gx[:], in0=nx[:], in1=rec[:],
                                        op=OP.mult)
                nc.vector.tensor_tensor(out=gy[:], in0=ny[:], in1=rec[:],
                                        op=OP.mult)

                # xs = floor(g + 1.5) = floor(g - 0.5) + 2. The f32->i32
                # cast ROUNDS to nearest-even on HW (truncates in sim), so
                # land within +-1 via the cast, then fix up with the sign
                # of the remainder — exact under either rounding mode.
                def floorfrac(g, nm):
                    xi = wp.tile([128, W], I32, tag=f"i{nm}")
                    nc.scalar.activation(out=xi[:], in_=g[:],
                                         func=AF.Identity, bias=tv(6))
                    xf = wp.tile([128, W], F32, tag=f"xf{nm}")
                    nc.scalar.copy(out=xf[:], in_=xi[:])
                    fr = wp.tile([128, W], F32, tag=f"fr{nm}")
                    nc.vector.scalar_tensor_tensor(
                        out=fr[:], in0=g[:], scalar=1.5, in1=xf[:],
                        op0=OP.add, op1=OP.subtract)
                    neg = wp.tile([128, W], F32, tag=f"ng{nm}")
                    nc.vector.tensor_scalar(out=neg[:], in0=fr[:],
                                            scalar1=0.0, scalar2=None,
                                            op0=OP.is_lt)
                    nc.vector.tensor_tensor(out=fr[:], in0=fr[:],
                                            in1=neg[:], op=OP.add)
                    xa = wp.tile([128, W], F32, tag=f"xa{nm}")
                    nc.vector.tensor_tensor(out=xa[:], in0=xf[:],
                                            in1=neg[:], op=OP.subtract)
                    xc = wp.tile([128, W], F32, tag=f"xc{nm}")
                    nc.vector.tensor_scalar(out=xc[:], in0=xa[:],
                                            scalar1=131.0, scalar2=0.0,
                                            op0=OP.min, op1=OP.max)
                    return xc, fr

                xsC, fx = floorfrac(gx, "x")
                ysC, fy = floorfrac(gy, "y")
                posF = wp.tile([128, W], F32, tag="posF")
                nc.vector.scalar_tensor_tensor(
                    out=posF[:], in0=ysC[:], scalar=float(PW), in1=xsC[:],
                    op0=OP.mult, op1=OP.add)
                posI = wp.tile([128, W], I32, tag="posI")
                nc.scalar.copy(out=posI[:], in_=posF[:])

                # ---- gather: HW DynamicAP honors ONE offset per partition
                # (scalar_dynamic_offset DGE level; vector offsets are
                # compiled out), so issue one 128-descriptor indirect DMA
                # per x column: idx [128,1] -> dest [128, CH].
                G = gp.tile([128, W * CH], F16, tag="G")
                prev = None
                for j in range(W):
                    gi = nc.gpsimd.indirect_dma_start(
                        out=_ap(G[:], j * CH, [[W * CH, 128], [1, CH]]),
                        out_offset=None,
                        in_=pmap[v].ap(),
                        in_offset=bass.IndirectOffsetOnAxis(
                            ap=posI[:, j:j + 1], axis=0))
                    if prev is None:
                        for dma_i in build_dumps[v]:
                            add_dep_helper(gi.ins, dma_i.ins,
                                           reason="pmap build before gather")
                    else:
                        add_dep_helper(gi.ins, prev.ins,
                                       reason="gather chain (same queue)")
                    prev = gi
                if debug and di == 0 and v == 0:
                    nc.sync.dma_start(out=dbg["dbgPos"].ap(), in_=posI[:])
                    nc.sync.dma_start(out=dbg["dbgG"].ap(), in_=G[:])

                # fp16 fractional weights (cast/affine on Act engine)
                fx16 = wp.tile([128, W], F16, tag="fx16")
                fy16 = wp.tile([128, W], F16, tag="fy16")
                fxm16 = wp.tile([128, W], F16, tag="fxm16")
                fym16 = wp.tile([128, W], F16, tag="fym16")
                nc.scalar.copy(out=fx16[:], in_=fx[:])
                nc.scalar.copy(out=fy16[:], in_=fy[:])
                nc.scalar.activation(out=fxm16[:], in_=fx[:],
                                     func=AF.Identity, scale=tv(9),
                                     bias=tv(8))
                nc.scalar.activation(out=fym16[:], in_=fy[:],
                                     func=AF.Identity, scale=tv(9),
                                     bias=tv(8))
                wd = {}
                for (tnm, fa, fb) in (("00", fxm16, fym16),
                                      ("01", fx16, fym16),
                                      ("10", fxm16, fy16),
                                      ("11", fx16, fy16)):
                    wt = wp.tile([128, W * 2], F16, tag=f"wd{tnm}")
                    nc.vector.tensor_tensor(
                        out=_ap(wt[:], 0, [[W * 2, 128], [2, W], [1, 2]]),
                        in0=_ap(fa[:], 0, [[W, 128], [1, W], [0, 2]]),
                        in1=_ap(fb[:], 0, [[W, 128], [1, W], [0, 2]]),
                        op=OP.mult)
                    wd[tnm] = wt

                # ---- bilinear taps: acc = sum_t w_t * G_t  [y,(x,c)] ----
                def tap(dy, dx):
                    return _ap(G[:], (dy * 2 + dx) * 16,
                               [[W * CH, 128], [CH, W], [2, 8], [1, 2]])

                def wap(tnm):
                    return _ap(wd[tnm][:], 0,
                               [[W * 2, 128], [2, W], [0, 8], [1, 2]])

                cview = [[2048, 128], [16, W], [2, 8], [1, 2]]
                acc = cum if v == 0 else bp.tile([128, W * C], F16, tag="s4k")
                p0 = bp.tile([128, W * C], F16, tag="s4k")
                p1 = bp.tile([128, W * C], F16, tag="s4k")
                nc.vector.tensor_tensor(out=_ap(acc[:], 0, cview),
                                        in0=tap(0, 0), in1=wap("00"),
                                        op=OP.mult)
                nc.vector.tensor_tensor(out=_ap(p0[:], 0, cview),
                                        in0=tap(0, 1), in1=wap("01"),
                                        op=OP.mult)
                nc.vector.tensor_tensor(out=_ap(p1[:], 0, cview),
                                        in0=tap(1, 0), in1=wap("10"),
                                        op=OP.mult)
                nc.vector.tensor_tensor(out=acc[:], in0=acc[:], in1=p0[:],
                                        op=OP.add)
                nc.vector.tensor_tensor(out=_ap(p0[:], 0, cview),
                                        in0=tap(1, 1), in1=wap("11"),
                                        op=OP.mult)
                nc.vector.tensor_tensor(out=p1[:], in0=p1[:], in1=p0[:],
                                        op=OP.add)
                nc.vector.tensor_tensor(out=acc[:], in0=acc[:], in1=p1[:],
                                        op=OP.add)

                # ---- cost_v = sum_c (ref - cum)^2 (Act square + DVE tree) --
                if v == 0:
                    nc.vector.tensor_tensor(out=diff[:], in0=refC[:],
                                            in1=cum[:], op=OP.subtract)
                else:
                    nc.vector.tensor_tensor(out=diff[:], in0=diff[:],
                                            in1=acc[:], op=OP.subtract)
                sq = bp.tile([128, W * C], F16, tag="s4k")
                nc.scalar.activation(out=sq[:], in_=diff[:], func=AF.Square)
                c8 = wp.tile([128, W * 8], F16, tag="c8")
                nc.vector.tensor_tensor(
                    out=_ap(c8[:], 0, [[W * 8, 128], [8, W], [1, 8]]),
                    in0=_ap(sq[:], 0, [[W * C, 128], [C, W], [1, 8]]),
                    in1=_ap(sq[:], 8, [[W * C, 128], [C, W], [1, 8]]),
                    op=OP.add)
                c4 = wp.tile([128, W * 4], F16, tag="c4")
                nc.vector.tensor_tensor(
                    out=_ap(c4[:], 0, [[W * 4, 128], [4, W], [1, 4]]),
                    in0=_ap(c8[:], 0, [[W * 8, 128], [8, W], [1, 4]]),
                    in1=_ap(c8[:], 4, [[W * 8, 128], [8, W], [1, 4]]),
                    op=OP.add)
                c2 = wp.tile([128, W * 2], F16, tag="c2")
                nc.vector.tensor_tensor(
                    out=_ap(c2[:], 0, [[W * 2, 128], [2, W], [1, 2]]),
                    in0=_ap(c4[:], 0, [[W * 4, 128], [4, W], [1, 2]]),
                    in1=_ap(c4[:], 2, [[W * 4, 128], [4, W], [1, 2]]),
                    op=OP.add)
                cs = wp.tile([128, W], F16, tag=f"csq{v}")
                nc.vector.tensor_tensor(
                    out=cs[:],
                    in0=_ap(c2[:], 0, [[W * 2, 128], [2, W]]),
                    in1=_ap(c2[:], 1, [[W * 2, 128], [2, W]]),
                    op=OP.add)
                csq[v] = cs
                if debug and di == 0:
                    if v == 0:
                        nc.sync.dma_start(out=dbg["dbgCum"].ap(), in_=cum[:])
                        nc.sync.dma_start(out=dbg["dbgCsq0"].ap(), in_=cs[:])
                    else:
                        nc.sync.dma_start(out=dbg["dbgDiff"].ap(), in_=diff[:])
                        nc.sync.dma_start(out=dbg["dbgCsq1"].ap(), in_=cs[:])

            # cost = sqrt(min(c1sq, c2sq)) into costSC5 center block
            cmin = wp.tile([128, W], F16, tag="cmin")
            nc.vector.tensor_tensor(out=cmin[:], in0=csq[0][:], in1=csq[1][:],
                                    op=OP.min)
            if debug and di == 0:
                nc.sync.dma_start(out=dbg["dbgCmin"].ap(), in_=cmin[:])
            costSC5 = wp.tile([128, 5 * PADX], F16, tag="costSC5")
            nc.scalar.activation(
                out=_ap(costSC5[:], 2 * PADX + 2, [[5 * PADX, 128], [1, W]]),
                in_=cmin[:], func=AF.Sqrt)
            nc.vector.memset(
                _ap(costSC5[:], 2 * PADX, [[5 * PADX, 128], [130, 2], [1, 2]]),
                0.0)
            for ty in (0, 1, 3, 4):
                k = ty - 2
                dlo = ty * PADX
                clo = 2 * PADX
                if k < 0:
                    nc.sync.dma_start(
                        out=costSC5[-k:128, dlo:dlo + PADX],
                        in_=costSC5[0:128 + k, clo:clo + PADX])
                    nc.sync.dma_start(out=costSC5[0:-k, dlo:dlo + PADX],
                                      in_=ztile[0:-k, 0:PADX])
                else:
                    nc.sync.dma_start(
                        out=costSC5[0:128 - k, dlo:dlo + PADX],
                        in_=costSC5[k:128, clo:clo + PADX])
                    nc.sync.dma_start(out=costSC5[128 - k:128, dlo:dlo + PADX],
                                      in_=ztile[0:k, 0:PADX])

            # ---- aggregation, tap-major [y, (t25, x)] ----
            dvol = ap_.tile([128, TM], F16, tag="dvol")
            nc.vector.tensor_tensor(
                out=_ap(dvol[:], 0, [[TM, 128], [5 * W, 5], [W, 5], [1, W]]),
                in0=_ap(depSC5[:], 0,
                        [[5 * PADX, 128], [PADX, 5], [1, 5], [1, W]]),
                in1=_ap(depSC5[:], 2 * PADX + 2,
                        [[5 * PADX, 128], [0, 5], [0, 5], [1, W]]),
                op=OP.subtract)
            nc.scalar.activation(out=dvol[:], in_=dvol[:], func=AF.Abs)
            evol = ap_.tile([128, TM], F16, tag="evol")
            nc.scalar.activation(out=evol[:], in_=dvol[:], func=AF.Exp,
                                 bias=tv(7))
            uvol = dvol  # reuse: dvol's last reader is the exp above
            nc.vector.tensor_tensor(out=uvol[:], in0=evol[:], in1=wf25[:],
                                    op=OP.mult)
            nc.vector.tensor_tensor(
                out=_ap(uvol[:], 0, [[TM, 128], [5 * W, 5], [W, 5], [1, W]]),
                in0=_ap(uvol[:], 0, [[TM, 128], [5 * W, 5], [W, 5], [1, W]]),
                in1=_ap(costSC5[:], 0,
                        [[5 * PADX, 128], [PADX, 5], [1, 5], [1, W]]),
                op=OP.mult)

            def tap_tree(vol, nm):
                s12 = wp.tile([128, 12 * W], F16, tag="s12")
                nc.vector.tensor_tensor(out=s12[:], in0=vol[:, 0:12 * W],
                                        in1=vol[:, 12 * W:24 * W], op=OP.add)
                s6 = wp.tile([128, 6 * W], F16, tag="s6")
                nc.vector.tensor_tensor(out=s6[:], in0=s12[:, 0:6 * W],
                                        in1=s12[:, 6 * W:12 * W], op=OP.add)
                s3 = wp.tile([128, 3 * W], F16, tag="s3")
                nc.vector.tensor_tensor(out=s3[:], in0=s6[:, 0:3 * W],
                                        in1=s6[:, 3 * W:6 * W], op=OP.add)
                r = wp.tile([128, W], F16, tag=f"r{nm}")
                nc.vector.tensor_tensor(out=r[:], in0=s3[:, 0:W],
                                        in1=s3[:, W:2 * W], op=OP.add)
                nc.vector.tensor_tensor(out=r[:], in0=r[:],
                                        in1=s3[:, 2 * W:3 * W], op=OP.add)
                nc.vector.tensor_tensor(out=r[:], in0=r[:],
                                        in1=vol[:, 24 * W:25 * W], op=OP.add)
                return r

            num = tap_tree(uvol, "n")
            den = tap_tree(evol, "d")
            if debug and di == 0:
                nc.sync.dma_start(out=dbg["dbgCost"].ap(), in_=costSC5[:])
                nc.sync.dma_start(out=dbg["dbgWf"].ap(), in_=wf25[:])
                nc.sync.dma_start(out=dbg["dbgNum"].ap(), in_=num[:])
                nc.sync.dma_start(out=dbg["dbgDen"].ap(), in_=den[:])
            rden = wp.tile([128, W], F32, tag="rden")
            nc.vector.reciprocal(out=rden[:], in_=den[:])
            agg_ap = _ap(aggT[:], di, [[W * DPC, 128], [DPC, W], [1, 1]])
            nc.vector.tensor_tensor(out=agg_ap, in0=num[:], in1=rden[:],
                                    op=OP.mult)

        # ---------------- per-core softmax partials ----------------
        def aggap(di):
            return _ap(aggT[:], di, [[W * DPC, 128], [DPC, W], [1, 1]])

        def depap(di):
            return _ap(depD[:], di, [[W * DPC, 128], [DPC, W], [1, 1]])

        if debug:
            nc.sync.dma_start(out=dbg["dbgAgg"].ap(), in_=aggT[:])
        m = pp.tile([128, W], F32, tag="m")
        nc.vector.tensor_tensor(out=m[:], in0=aggap(0), in1=aggap(1),
                                op=OP.max)
        for di in range(2, DPC):
            nc.vector.tensor_tensor(out=m[:], in0=m[:], in1=aggap(di),
                                    op=OP.max)
        s0 = pp.tile([128, W], F32, tag="s0")
        s1 = pp.tile([128, W], F32, tag="s1")
        for di in range(DPC):
            t = wp.tile([128, W], F32, tag="et")
            nc.vector.tensor_tensor(out=t[:], in0=aggap(di), in1=m[:],
                                    op=OP.subtract)
            e = wp.tile([128, W], F32, tag="ee")
            nc.scalar.activation(out=e[:], in_=t[:], func=AF.Exp)
            t1 = wp.tile([128, W], F32, tag="t1")
            nc.vector.tensor_tensor(out=t1[:], in0=e[:], in1=depap(di),
                                    op=OP.mult)
            if di == 0:
                nc.vector.tensor_copy(out=s0[:], in_=e[:])
                nc.vector.tensor_copy(out=s1[:], in_=t1[:])
            else:
                nc.vector.tensor_tensor(out=s0[:], in0=s0[:], in1=e[:],
                                        op=OP.add)
                nc.vector.tensor_tensor(out=s1[:], in0=s1[:], in1=t1[:],
                                        op=OP.add)
        nc.sync.dma_start(out=out3.ap()[0], in_=m[:])
        nc.sync.dma_start(out=out3.ap()[1], in_=s0[:])
        nc.sync.dma_start(out=out3.ap()[2], in_=s1[:])

        for p in reversed(ctx_pools):
            p.__exit__(None, None, None)

    nc.compile()
    return nc


def host_prep(features, intrinsics, cam_to_world, depth_hypo):
    """Build the 8 per-core input maps. All O(small) except slicing."""
    features = np.asarray(features, dtype=np.float32)
    intrinsics = np.asarray(intrinsics, dtype=np.float32)
    cam_to_world = np.asarray(cam_to_world, dtype=np.float32)
    depth_hypo = np.asarray(depth_hypo, dtype=np.float32)

    ys, xs = np.meshgrid(np.arange(H, dtype=np.float32),
                         np.arange(W, dtype=np.float32), indexing="ij")
    in_maps = []
    for k in range(NCORES):
        b = k // (NCORES // B)
        dlo = DPC * (k % (NCORES // B))
        amap = np.zeros((NV, 3, H, W), np.float32)
        tvv = np.zeros((12,), np.float32)
        for vi in range(1, V):
            src_w2c = np.linalg.inv(cam_to_world[vi, b])
            ref_w2c = np.linalg.inv(cam_to_world[0, b])
            src_KK = src_w2c.copy()
            src_KK[:3, :3] = intrinsics[vi, b]
            ref_KK = ref_w2c.copy()
            ref_KK[:3, :3] = intrinsics[0, b]
            proj = (src_KK @ src_w2c) @ np.linalg.inv(ref_KK @ ref_w2c)
            rot, trans = proj[:3, :3], proj[:3, 3]
            A = (rot[:, 0:1, None] * xs[None] + rot[:, 1:2, None] * ys[None]
                 + rot[:, 2:3, None])  # [3, H, W]
            v = vi - 1
            amap[v, 0] = A[0] * SCALE
            amap[v, 1] = A[1] * SCALE
            amap[v, 2] = A[2]
            tvv[v * 3 + 0] = trans[0] * SCALE
            tvv[v * 3 + 1] = trans[1] * SCALE
            tvv[v * 3 + 2] = trans[2]
        tvv[6] = 1.5      # floor-by-trunc bias (+2 border shift - 0.5 center)
        tvv[7] = ESHIFT
        tvv[8] = 1.0
        tvv[9] = -1.0
        tvv[10] = 0.5
        in_maps.append({
            "refF": np.ascontiguousarray(features[0, b].transpose(0, 2, 1)),
            "srcF": np.ascontiguousarray(features[1:, b]),
            "dep": np.ascontiguousarray(
                depth_hypo[b, dlo:dlo + DPC].transpose(0, 2, 1)),
            "amap": np.ascontiguousarray(amap.transpose(0, 1, 3, 2)),
            "tvec": np.tile(tvv[None, :], (128, 1)).astype(np.float32),
        })
    return in_maps


def host_combine(results):
    """Merge per-core softmax partials (m, s0, s1) into [B, H, W]."""
    out = np.zeros((B, H, W), np.float32)
    per_b = NCORES // B
    for b in range(B):
        parts = [np.asarray(results[b * per_b + j]["out3"]) for j in range(per_b)]
        parts = [p.transpose(0, 2, 1) for p in parts]  # [3, x, y] -> [3, y, x]
        ms = np.stack([p[0] for p in parts])         # [4, H, W]
        M = ms.max(axis=0)
        S0 = np.zeros((H, W), np.float64)
        S1 = np.zeros((H, W), np.float64)
        for p in parts:
            w = np.exp(p[0] - M)
            S0 += w * p[1]
            S1 += w * p[2]
        out[b] = (S1 / S0).astype(np.float32)
    return out


def _run_sim(nc, in_maps):
    from concourse.bass_interp import CoreSim
    results = []
    for core in range(NCORES):
        sim = CoreSim(nc, trace=False, publish_trace=False)
        for k, v in in_maps[core].items():
            sim.tensor(k)[:] = v
        sim.simulate()
        results.append({"out3": np.array(sim.tensor("out3"))})
    return results


def _get_exec():
    """Build the Bass program once and wrap it in a CACHED jitted
    shard_map executable (the stock run_bass_kernel_spmd path rebuilds
    the jax closure per call -> full retrace + compile every time)."""
    if "exec" in _cached:
        return _cached["exec"]
    import jax
    import concourse.mybir as _mybir
    from concourse import bass2jax
    from jax.experimental.shard_map import shard_map
    from jax.sharding import Mesh, NamedSharding, PartitionSpec

    nc = build_program()
    bass2jax.install_neuronx_cc_hook()

    partition_name = (nc.partition_id_tensor.name
                      if nc.partition_id_tensor else None)
    in_names, out_names, out_avals = [], [], []
    for alloc in nc.m.functions[0].allocations:
        if not isinstance(alloc, _mybir.MemoryLocationSet):
            continue
        name = alloc.memorylocations[0].name
        if alloc.kind == "ExternalInput":
            if name != partition_name:
                in_names.append(name)
        elif alloc.kind == "ExternalOutput":
            shape = tuple(alloc.tensor_shape)
            dtype = _mybir.dt.np(alloc.dtype)
            out_names.append(name)
            out_avals.append(jax.core.ShapedArray(shape, dtype))
    n_params = len(in_names)
    all_names = list(in_names) + list(out_names)
    if partition_name is not None:
        all_names.append(partition_name)
    donate = tuple(range(n_params, n_params + len(out_names)))

    def _body(*args):
        operands = list(args)
        if partition_name is not None:
            operands.append(bass2jax.partition_id_tensor())
        outs = bass2jax._bass_exec_p.bind(
            *operands,
            out_avals=tuple(out_avals),
            in_names=tuple(all_names),
            out_names=tuple(out_names),
            lowering_input_output_aliases=(),
            sim_require_finite=True,
            sim_require_nnan=True,
            nc=nc,
        )
        return tuple(outs)

    devices = jax.devices()[:NCORES]
    mesh = Mesh(np.asarray(devices), ("core",))
    n_in = n_params + len(out_names)
    jitted = jax.jit(
        shard_map(_body, mesh=mesh,
                  in_specs=(PartitionSpec("core"),) * n_in,
                  out_specs=(PartitionSpec("core"),) * len(out_names),
                  check_rep=False),
        donate_argnums=donate, keep_unused=True)
    sharding = NamedSharding(mesh, PartitionSpec("core"))
    st = {"nc": nc, "jitted": jitted, "in_names": in_names,
          "out_names": out_names, "out_avals": out_avals,
          "dbg_name": nc.dbg_addr.name if nc.dbg_addr is not None else None,
          "sharding": sharding}
    _cached["exec"] = st
    return st


def _concat_inputs(st, in_maps):
    dbg = st["dbg_name"]
    if dbg is not None:
        in_maps = [{**m, dbg: np.zeros((1, 2), np.uint32)} for m in in_maps]
    return [np.concatenate([np.asarray(m[name]) for m in in_maps], axis=0)
            for name in st["in_names"]]


def _zero_outs(st):
    return [np.zeros((NCORES * a.shape[0],) + tuple(a.shape[1:]), a.dtype)
            for a in st["out_avals"]]


def _run_cached(st, in_maps):
    out_arrs = st["jitted"](*_concat_inputs(st, in_maps), *_zero_outs(st))
    res = []
    for c in range(NCORES):
        res.append({name: np.asarray(out_arrs[i]).reshape(
            (NCORES,) + tuple(st["out_avals"][i].shape))[c]
            for i, name in enumerate(st["out_names"])})
    return res


def kernel(**inputs):
    in_maps = host_prep(**inputs)
    if _cached.get("hw_broken"):
        if "nc" not in _cached:
            _cached["nc"] = build_program()
        return host_combine(_run_sim(_cached["nc"], in_maps))
    try:
        st = _get_exec()
        return host_combine(_run_cached(st, in_maps))
    except Exception:
        try:
            if "nc" not in _cached:
                _cached["nc"] = build_program()
            res = run_bass_kernel_spmd(_cached["nc"], in_maps,
                                       core_ids=list(range(NCORES)))
            return host_combine(res.results)
        except Exception:
            _cached["hw_broken"] = True
            return host_combine(_run_sim(_cached["nc"], in_maps))


if __name__ == "__main__":
    import reference
    inp = reference.setup_inputs()
    inp = {k: np.asarray(v) for k, v in inp.items()}
    out = kernel(**inp)
    print("kernel out", out.shape, out.dtype)



# revision 38
# speedup vs baseline: 9794.4254x; 21.8596x over previous
"""Trainium2 Bass kernel for nn_DepthPrediction (multi-view stereo depth).

Strategy (8 NeuronCores, SPMD single program):
  - Shard: core k handles batch b = k//4 and depth planes [8*(k%4), 8*(k%4)+8).
  - Per (b,d): homography warp of 2 src views via on-device fp16 "patch maps"
    (132x132 positions x [dy2,dx2,c16] fp16 chunks = 128B) gathered with ONE
    standard indirect DMA (DynamicAP, 16K indices) per (plane, view);
    bilinear interp + cumulative cost (L2 over 16 ch) in pixel-major fp16 on
    DVE; 5x5 adaptive aggregation (depth-similarity softmax x
    feature-similarity weight) in tap-major layout; per-core softmax
    partials over the 8 local planes.
  - Engine budget: DVE does the packed-fp16 tensor math; Activation does
    affine/exp/ln/square/casts (single act table: natural_log_exp);
    Pool does the indirect gathers + memsets; SP does DMAs.
  - NOTE: custom-ucode GPSIMD ops (dma_gather/ap_gather) hard-crash this
    runtime (NRT_EXEC_UNIT_UNRECOVERABLE) — only stock-NEFF instructions
    are used here.
  - Host: trivial glue — 4x4 matrix algebra, shard/pack inputs, combine the
    4-way softmax partials per batch (log-sum-exp merge) into the output.
  - Execution: the jitted shard_map executable is built ONCE and cached;
    repeat kernel() calls reuse it (the stock per-call path retraces).

Self-contained: hardcodes all shapes from the problem spec.
"""

import numpy as np

import concourse.bacc as bacc
import concourse.bass as bass
import concourse.mybir as mybir
import concourse.tile as tile
from concourse.bass_utils import run_bass_kernel_spmd
from concourse.tile_rust import add_dep_helper

F32 = mybir.dt.float32
F16 = mybir.dt.float16
I32 = mybir.dt.int32
I16 = mybir.dt.int16

AF = mybir.ActivationFunctionType
OP = mybir.AluOpType
AX = mybir.AxisListType

# problem shapes
V, B, C, H, W, D = 3, 2, 16, 128, 128, 32
NCORES = 8
DPC = D // (NCORES // B)  # depth planes per core = 8
NV = V - 1  # src views = 2

PW = W + 4          # padded map width (x0 in [-2,129] -> cols 0..131)
POS = PW * PW       # patch positions
CH = 64             # chunk elems: [dy2,dx2,c16] fp16 = 128B
PADX = W + 4        # x-padded tiles for 5-tap aggregation
SCALE = W / (W - 1.0)  # grid_sample align_corners=False fold
ESHIFT = -9.0       # exp(|dnb-d|) stabilization shift (|delta| <= 9)
NT = 25             # aggregation taps
TM = NT * W         # tap-major volume free size
NDQ = 1             # SWDGE dynamic queues (2 measured no faster than 1)

_cached = {}


def _ap(base, off, dims):
    """Raw AP on the same tensor as `base` (an AP), offset in elements."""
    return bass.AP(base.tensor, base.offset + off, dims)


def build_program(debug=False, ablate_gather=False):
    # NOTE: do NOT reorder the get_activation_tables() dict — the emitted
    # act_func_set_id indexes the ORIGINAL act_info.json order; walrus
    # resolves ids against that file, so a reorder makes HW load the wrong
    # table (Ln evaluated off the exp table = garbage). CoreSim ignores
    # tables, which hid this.
    nc = bacc.Bacc("TRN2", target_bir_lowering=False, debug=False,
                   num_devices=NCORES, num_swdge_queues=NDQ)

    refF = nc.dram_tensor("refF", [C, H, W], F32, kind="ExternalInput")
    srcF = nc.dram_tensor("srcF", [NV, C, H, W], F32, kind="ExternalInput")
    dep = nc.dram_tensor("dep", [DPC, H, W], F32, kind="ExternalInput")
    amap = nc.dram_tensor("amap", [NV, 3, H, W], F32, kind="ExternalInput")
    tvec = nc.dram_tensor("tvec", [128, 12], F32, kind="ExternalInput")
    out3 = nc.dram_tensor("out3", [3, H, W], F32, kind="ExternalOutput")
    # internal patch maps, one per src view
    pmap = [nc.dram_tensor(f"pmap{v}", [POS, CH], F16, kind="Internal")
            for v in range(NV)]
    dbg = {}
    if debug:
        for nm, shape, dt in (
                ("dbgPos", [128, W], I32), ("dbgG", [128, W * CH], F16),
                ("dbgCum", [128, W * C], F16), ("dbgDiff", [128, W * C], F16),
                ("dbgCsq0", [128, W], F16), ("dbgCsq1", [128, W], F16),
                ("dbgCost", [128, 5 * PADX], F16),
                ("dbgCmin", [128, W], F16),
                ("dbgWf", [128, NT * W], F16),
                ("dbgNum", [128, W], F16), ("dbgDen", [128, W], F16),
                ("dbgAgg", [128, W * DPC], F32),
                ("dbgPmap", [POS, CH], F16)):
            dbg[nm] = nc.dram_tensor(nm, shape, dt, kind="ExternalOutput")

    with nc.allow_low_precision("fp16 pipeline by design"), \
            tile.TileContext(nc) as tc:
        ctx_pools = []

        def pool(name, bufs=1, **kw):
            p = tc.tile_pool(name=name, bufs=bufs, **kw)
            ctx_pools.append(p)
            return p.__enter__()

        pp = pool("persist", 1)     # long-lived tiles
        prep = tc.tile_pool(name="prep", bufs=1)
        pr = prep.__enter__()

        # ---------------- constant / persistent loads ----------------
        amapL = pp.tile([128, NV * 3 * W], F32, tag="amapL")  # [y,(v,row,x)]
        nc.sync.dma_start(out=amapL[:], in_=_ap(
            amap.ap(), 0, [[W, 128], [3 * H * W, NV], [H * W, 3], [1, W]]))
        tvecT = pp.tile([128, 12], F32, tag="tvecT")
        nc.sync.dma_start(out=tvecT[:], in_=tvec.ap())

        def tv(col):  # [128,1] per-partition scalar AP
            return tvecT[:, col:col + 1]

        # zero-source tile for edge DMAs (f16)
        ztile = pp.tile([128, PADX * C], F16, tag="ztile")
        nc.gpsimd.memset(ztile[:], 0.0)

        # ---------------- ref prep ----------------
        tr = pr.tile([128, C * W], F32, tag="tr")  # [y,(c,x)] f32
        nc.sync.dma_start(out=tr[:], in_=_ap(
            refF.ap(), 0, [[W, 128], [H * W, C], [1, W]]))
        # refC [y,(x,c)] fp16
        refC = pp.tile([128, W * C], F16, tag="refC")
        nc.vector.tensor_copy(
            out=_ap(refC[:], 0, [[W * C, 128], [C, W], [1, C]]),
            in_=_ap(tr[:], 0, [[C * W, 128], [1, W], [W, C]]))
        # refPadC [y,(xp132,c)] fp16, x' = x+2
        refPadC = pr.tile([128, PADX * C], F16, tag="refPadC")
        nc.vector.memset(
            _ap(refPadC[:], 0, [[PADX * C, 128], [C, 2], [1, C]]), 0.0)
        nc.vector.memset(
            _ap(refPadC[:], 130 * C, [[PADX * C, 128], [C, 2], [1, C]]), 0.0)
        nc.scalar.copy(out=refPadC[:, 2 * C:(2 + W) * C], in_=refC[:])
        # 4 partition-shifted copies (ty in {0,1,3,4}; center ty=2 = refPadC)
        refSC = {}
        for ty in (0, 1, 3, 4):
            t = pr.tile([128, PADX * C], F16, tag=f"refSC{ty}")
            k = ty - 2
            if k < 0:
                nc.sync.dma_start(out=t[-k:128, :], in_=refPadC[0:128 + k, :])
                nc.sync.dma_start(out=t[0:-k, :], in_=ztile[0:-k, :])
            else:
                nc.sync.dma_start(out=t[0:128 - k, :], in_=refPadC[k:128, :])
                nc.sync.dma_start(out=t[128 - k:128, :], in_=ztile[0:k, :])
            refSC[ty] = t
        refSC[2] = refPadC

        # ---------------- patch map build (per src view) ----------------
        build_dumps = []
        for v in range(NV):
            # staged [y, (xp132, dy2, dx2, c16, pad64)] fp16; row y -> pos
            # row y+2.  Zero only the data slots of border cols xp in
            # {0,1} u {129,130,131}; pad elems are never read downstream.
            staged = pr.tile([128, PW * CH], F16, tag=f"staged{v}")
            nc.vector.memset(
                _ap(staged[:], 0, [[PW * CH, 128], [CH, 2], [1, 64]]), 0.0)
            nc.vector.memset(
                _ap(staged[:], 129 * CH, [[PW * CH, 128], [CH, 3], [1, 64]]),
                0.0)
            # TF16 [y, (c,x)] fp16  (cast during DMA, SWDGE)
            tf = pr.tile([128, C * W], F16, tag="tf")
            nc.gpsimd.dma_start(
                out=tf[:], in_=_ap(srcF.ap(), v * C * H * W,
                                   [[W, 128], [H * W, C], [1, W]]))
            # down-shifted copy: tfdn[y] = tf[y+1]; row 127 = 0
            tfdn = pr.tile([128, C * W], F16, tag="tfdn")
            nc.sync.dma_start(out=tfdn[0:127, :], in_=tf[1:128, :])
            nc.sync.dma_start(out=tfdn[127:128, :],
                              in_=_ap(ztile[:], 0, [[PADX * C, 1], [1, C * W]]))

            copy_fns = [nc.vector.tensor_copy,
                        lambda out, in_: nc.scalar.copy(out=out, in_=in_),
                        nc.gpsimd.tensor_copy,
                        lambda out, in_: nc.scalar.copy(out=out, in_=in_)]
            i = 0
            for dy, src in ((0, tf), (1, tfdn)):
                for dx in (0, 1):
                    # staged[y, xp, dy, dx, c] = src[y, c, xp-2+dx]
                    xp_lo = 2 - dx
                    n_xp = 128
                    src_ap = _ap(src[:], 0,
                                 [[C * W, 128], [1, n_xp], [W, C]])
                    dst_ap = _ap(staged[:], xp_lo * CH + dy * 32 + dx * 16,
                                 [[PW * CH, 128], [CH, n_xp], [1, C]])
                    copy_fns[i](out=dst_ap, in_=src_ap)
                    i += 1
            # dump rows 2..129 of the pos grid (partition y -> row y+2)
            d1 = nc.sync.dma_start(
                out=_ap(pmap[v].ap(), 2 * PW * CH,
                        [[PW * CH, 128], [1, PW * CH]]),
                in_=staged[:])
            # zero rows 0, 130, 131 from ztile (128x132 slice covers a row)
            dz = []
            for r in (0, 130, 131):
                dz.append(nc.sync.dma_start(
                    out=_ap(pmap[v].ap(), r * PW * CH,
                            [[PW * CH, 1], [1, PW * CH]]),
                    in_=_ap(ztile[:], 0, [[PADX * C, 8], [1, PW * CH // 8]])))
            # row r=1 (y0=-1): dy=1 slots hold F row 0 = staged[0]'s dy=0
            # window (same cells, slot offset +32); rest zero
            d4z = nc.sync.dma_start(
                out=_ap(pmap[v].ap(), PW * CH, [[PW * CH, 1], [1, PW * CH]]),
                in_=_ap(ztile[:], 0, [[PADX * C, 8], [1, PW * CH // 8]]))
            d4 = nc.sync.dma_start(
                out=_ap(pmap[v].ap(), PW * CH + 32,
                        [[PW * CH, 1], [CH, PW], [1, 32]]),
                in_=_ap(staged[0:1, :], 0, [[PW * CH, 1], [CH, PW], [1, 32]]))
            add_dep_helper(d4.ins, d4z.ins, reason="row1 zero before windows")
            build_dumps.append(tuple([d1, d4, d4z] + dz))
            if debug and v == 0:
                dp = nc.sync.dma_start(
                    out=_ap(dbg["dbgPmap"].ap(), 0,
                            [[PW * CH, PW], [1, PW * CH]]),
                    in_=_ap(pmap[0].ap(), 0, [[PW * CH, PW], [1, PW * CH]]))
                for dma_i in build_dumps[0]:
                    add_dep_helper(dp.ins, dma_i.ins,
                                   reason="pmap build before debug dump")

        # ---------------- w_feat volume [y, (t25, x)] fp16 ----
        wf25 = pp.tile([128, TM], F16, tag="wf25")
        nc.vector.memset(wf25[:, 12 * W:13 * W], 0.0)
        for ty in (2, 1, 3, 0, 4):
            for tx in range(5):
                t = ty * 5 + tx
                if t == 12:
                    continue
                # diff = ref[y,x,c] - refSC[ty][y, x+tx, c]
                dfw = pr.tile([128, W * C], F16, tag="wfdf")
                nc.vector.tensor_tensor(
                    out=dfw[:],
                    in0=_ap(refSC[ty][:], tx * C,
                            [[PADX * C, 128], [C, W], [1, C]]),
                    in1=refC[:], op=OP.subtract)
                sqw = pr.tile([128, W * C], F16, tag="wfsq")
                nc.scalar.activation(out=sqw[:], in_=dfw[:], func=AF.Square)
                # c16 tree reduction (packed fp16 halves)
                c8 = pr.tile([128, W * 8], F16, tag="c8w")
                nc.vector.tensor_tensor(
                    out=_ap(c8[:], 0, [[W * 8, 128], [8, W], [1, 8]]),
                    in0=_ap(sqw[:], 0, [[W * C, 128], [C, W], [1, 8]]),
                    in1=_ap(sqw[:], 8, [[W * C, 128], [C, W], [1, 8]]),
                    op=OP.add)
                c4 = pr.tile([128, W * 4], F16, tag="c4w")
                nc.vector.tensor_tensor(
                    out=_ap(c4[:], 0, [[W * 4, 128], [4, W], [1, 4]]),
                    in0=_ap(c8[:], 0, [[W * 8, 128], [8, W], [1, 4]]),
                    in1=_ap(c8[:], 4, [[W * 8, 128], [8, W], [1, 4]]),
                    op=OP.add)
                c2 = pr.tile([128, W * 2], F16, tag="c2w")
                nc.vector.tensor_tensor(
                    out=_ap(c2[:], 0, [[W * 2, 128], [2, W], [1, 2]]),
                    in0=_ap(c4[:], 0, [[W * 4, 128], [4, W], [1, 2]]),
                    in1=_ap(c4[:], 2, [[W * 4, 128], [4, W], [1, 2]]),
                    op=OP.add)
                cs = pr.tile([128, W], F16, tag="csw")
                nc.vector.tensor_tensor(
                    out=cs[:],
                    in0=_ap(c2[:], 0, [[W * 2, 128], [2, W]]),
                    in1=_ap(c2[:], 1, [[W * 2, 128], [2, W]]),
                    op=OP.add)
                # wf = sqrt(cs)  (direct Sqrt act; Ln is table-hostile here)
                nc.scalar.activation(out=wf25[:, t * W:(t + 1) * W],
                                     in_=cs[:], func=AF.Sqrt)


        prep.__exit__(None, None, None)

        # ---------------- batched warp coordinates (all planes) ----------
        # depB [y, (d, x)] f32; posIv / wdB computed once per view for all
        # DPC planes -> the per-(d,v) loop only gathers + interpolates.
        depB = pp.tile([128, DPC * W], F32, tag="depB")
        nc.sync.dma_start(out=depB[:], in_=_ap(
            dep.ap(), 0, [[W, 128], [H * W, DPC], [1, W]]))
        posIv = [pp.tile([128, DPC * W], I32, tag=f"posIv{v}",
                         name=f"posIv{v}") for v in range(NV)]
        wdB = [{}, {}]
        for v in range(NV):
            for tnm in ("00", "01", "10", "11"):
                wdB[v][tnm] = pp.tile([128, DPC * W * 2], F16,
                                      tag=f"wdB{v}{tnm}",
                                      name=f"wdB{v}{tnm}")

        cprep = tc.tile_pool(name="coord", bufs=1)
        cq = cprep.__enter__()
        DW = DPC * W
        for v in range(NV):
            def arowB(r):
                return _ap(amapL[:], (v * 3 + r) * W,
                           [[NV * 3 * W, 128], [0, DPC], [1, W]])
            mx = cq.tile([128, DW], F32, tag="mxB")
            my = cq.tile([128, DW], F32, tag="myB")
            dn = cq.tile([128, DW], F32, tag="dnB")
            nc.vector.tensor_tensor(out=mx[:], in0=arowB(0), in1=depB[:],
                                    op=OP.mult)
            nc.vector.tensor_tensor(out=my[:], in0=arowB(1), in1=depB[:],
                                    op=OP.mult)
            nc.vector.tensor_tensor(out=dn[:], in0=arowB(2), in1=depB[:],
                                    op=OP.mult)
            nx = cq.tile([128, DW], F32, tag="nxB")
            ny = cq.tile([128, DW], F32, tag="nyB")
            dnt = cq.tile([128, DW], F32, tag="dntB")
            nc.scalar.activation(out=nx[:], in_=mx[:], func=AF.Identity,
                                 bias=tv(v * 3 + 0))
            nc.scalar.activation(out=ny[:], in_=my[:], func=AF.Identity,
                                 bias=tv(v * 3 + 1))
            nc.scalar.activation(out=dnt[:], in_=dn[:], func=AF.Identity,
                                 bias=tv(v * 3 + 2))
            rec = cq.tile([128, DW], F32, tag="recB")
            nc.vector.reciprocal(out=rec[:], in_=dnt[:])
            gx = cq.tile([128, DW], F32, tag="gxB")
            gy = cq.tile([128, DW], F32, tag="gyB")
            nc.vector.tensor_tensor(out=gx[:], in0=nx[:], in1=rec[:],
                                    op=OP.mult)
            nc.vector.tensor_tensor(out=gy[:], in0=ny[:], in1=rec[:],
                                    op=OP.mult)

            # floor(g + 1.5) exact under round-to-nearest-even OR trunc
            def floorfracB(g, nm):
                xi = cq.tile([128, DW], I32, tag=f"iB{nm}")
                nc.scalar.activation(out=xi[:], in_=g[:],
                                     func=AF.Identity, bias=tv(6))
                xf = cq.tile([128, DW], F32, tag=f"xfB{nm}")
                nc.scalar.copy(out=xf[:], in_=xi[:])
                fr = cq.tile([128, DW], F32, tag=f"frB{nm}")
                nc.vector.scalar_tensor_tensor(
                    out=fr[:], in0=g[:], scalar=1.5, in1=xf[:],
                    op0=OP.add, op1=OP.subtract)
                neg = cq.tile([128, DW], F32, tag=f"ngB{nm}")
                nc.vector.tensor_scalar(out=neg[:], in0=fr[:],
                                        scalar1=0.0, scalar2=None,
                                        op0=OP.is_lt)
                nc.vector.tensor_tensor(out=fr[:], in0=fr[:],
                                        in1=neg[:], op=OP.add)
                xa = cq.tile([128, DW], F32, tag=f"xaB{nm}")
                nc.vector.tensor_tensor(out=xa[:], in0=xf[:],
                                        in1=neg[:], op=OP.subtract)
                xc = cq.tile([128, DW], F32, tag=f"xcB{nm}")
                nc.vector.tensor_scalar(out=xc[:], in0=xa[:],
                                        scalar1=131.0, scalar2=0.0,
                                        op0=OP.min, op1=OP.max)
                return xc, fr

            xsC, fx = floorfracB(gx, "x")
            ysC, fy = floorfracB(gy, "y")
            posF = cq.tile([128, DW], F32, tag="posFB")
            nc.vector.scalar_tensor_tensor(
                out=posF[:], in0=ysC[:], scalar=float(PW), in1=xsC[:],
                op0=OP.mult, op1=OP.add)
            nc.scalar.copy(out=posIv[v][:], in_=posF[:])

            # fp16 fractional weights + bilinear tap weights, batched
            fx16 = cq.tile([128, DW], F16, tag="fx16B")
            fy16 = cq.tile([128, DW], F16, tag="fy16B")
            fxm16 = cq.tile([128, DW], F16, tag="fxm16B")
            fym16 = cq.tile([128, DW], F16, tag="fym16B")
            nc.scalar.copy(out=fx16[:], in_=fx[:])
            nc.scalar.copy(out=fy16[:], in_=fy[:])
            nc.scalar.activation(out=fxm16[:], in_=fx[:],
                                 func=AF.Identity, scale=tv(9), bias=tv(8))
            nc.scalar.activation(out=fym16[:], in_=fy[:],
                                 func=AF.Identity, scale=tv(9), bias=tv(8))
            for (tnm, fa, fb) in (("00", fxm16, fym16), ("01", fx16, fym16),
                                  ("10", fxm16, fy16), ("11", fx16, fy16)):
                nc.vector.tensor_tensor(
                    out=_ap(wdB[v][tnm][:], 0,
                            [[DW * 2, 128], [2, DW], [1, 2]]),
                    in0=_ap(fa[:], 0, [[DW, 128], [1, DW], [0, 2]]),
                    in1=_ap(fb[:], 0, [[DW, 128], [1, DW], [0, 2]]),
                    op=OP.mult)
        cprep.__exit__(None, None, None)

        wp = pool("work", 2)        # small per-(b,d) working tiles
        bp = pool("big4", 4)        # shared 4KB scratch (tag s4k)
        cp = pool("cumdif", 1)      # cum/diff accumulators
        gp = pool("gath", 2)        # gather destinations
        ap_ = pool("aggp", 2)       # agg volumes

        # agg store
        aggT = pp.tile([128, W * DPC], F32, tag="aggT")

        # ---------------- per depth-plane pipeline ----------------
        for di in range(DPC):
            def depf_ap():
                return _ap(depB[:], di * W, [[DPC * W, 128], [1, W]])
            # depSC5 [y, (ty5, xp132)] fp16: center block + 4 shifted blocks
            depSC5 = wp.tile([128, 5 * PADX], F16, tag="depSC5")
            nc.scalar.copy(
                out=_ap(depSC5[:], 2 * PADX + 2, [[5 * PADX, 128], [1, W]]),
                in_=depf_ap())
            nc.vector.memset(
                _ap(depSC5[:], 2 * PADX, [[5 * PADX, 128], [130, 2], [1, 2]]),
                0.0)
            for ty in (0, 1, 3, 4):
                k = ty - 2
                dlo = ty * PADX
                clo = 2 * PADX
                if k < 0:
                    nc.sync.dma_start(
                        out=depSC5[-k:128, dlo:dlo + PADX],
                        in_=depSC5[0:128 + k, clo:clo + PADX])
                    nc.sync.dma_start(out=depSC5[0:-k, dlo:dlo + PADX],
                                      in_=ztile[0:-k, 0:PADX])
                else:
                    nc.sync.dma_start(
                        out=depSC5[0:128 - k, dlo:dlo + PADX],
                        in_=depSC5[k:128, clo:clo + PADX])
                    nc.sync.dma_start(out=depSC5[128 - k:128, dlo:dlo + PADX],
                                      in_=ztile[0:k, 0:PADX])

            cum = cp.tile([128, W * C], F16, tag="cum")
            diff = cp.tile([128, W * C], F16, tag="diff")
            csq = {}
            for v in range(NV):
                # ---- gather: HW DynamicAP honors ONE offset per partition
                # (scalar_dynamic_offset DGE level; vector offsets are
                # compiled out), so issue one 128-descriptor indirect DMA
                # per x column: idx [128,1] -> dest [128, CH].
                G = gp.tile([128, W * CH], F16, tag="G")
                if ablate_gather:
                    nc.vector.memset(G[:], 0.25)
                else:
                    prev = [None] * NDQ
                    for j in range(W):
                        q = j % NDQ
                        gi = nc.gpsimd.indirect_dma_start(
                            out=_ap(G[:], j * CH, [[W * CH, 128], [1, CH]]),
                            out_offset=None,
                            in_=pmap[v].ap(),
                            in_offset=bass.IndirectOffsetOnAxis(
                                ap=posIv[v][:, di * W + j:di * W + j + 1],
                                axis=0))
                        if q:
                            gi.ins.queue = f"qPoolDynamic{q}"
                        if prev[q] is None:
                            for dma_i in build_dumps[v]:
                                add_dep_helper(
                                    gi.ins, dma_i.ins,
                                    reason="pmap build before gather")
                        else:
                            add_dep_helper(gi.ins, prev[q].ins,
                                           reason="gather chain (same queue)")
                        prev[q] = gi
                if debug and di == 0 and v == 0:
                    nc.sync.dma_start(
                        out=dbg["dbgPos"].ap(),
                        in_=_ap(posIv[0][:], 0, [[DPC * W, 128], [1, W]]))
                    nc.sync.dma_start(out=dbg["dbgG"].ap(), in_=G[:])

                # ---- bilinear taps: acc = sum_t w_t * G_t  [y,(x,c)] ----
                def tap(dy, dx):
                    return _ap(G[:], (dy * 2 + dx) * 16,
                               [[W * CH, 128], [CH, W], [2, 8], [1, 2]])

                def wap(tnm):
                    return _ap(wdB[v][tnm][:], di * W * 2,
                               [[DPC * W * 2, 128], [2, W], [0, 8], [1, 2]])

                cview = [[2048, 128], [16, W], [2, 8], [1, 2]]
                acc = cum if v == 0 else bp.tile([128, W * C], F16, tag="s4k")
                p0 = bp.tile([128, W * C], F16, tag="s4k")
                p1 = bp.tile([128, W * C], F16, tag="s4k")
                nc.vector.tensor_tensor(out=_ap(acc[:], 0, cview),
                                        in0=tap(0, 0), in1=wap("00"),
                                        op=OP.mult)
                nc.vector.tensor_tensor(out=_ap(p0[:], 0, cview),
                                        in0=tap(0, 1), in1=wap("01"),
                                        op=OP.mult)
                nc.vector.tensor_tensor(out=_ap(p1[:], 0, cview),
                                        in0=tap(1, 0), in1=wap("10"),
                                        op=OP.mult)
                nc.vector.tensor_tensor(out=acc[:], in0=acc[:], in1=p0[:],
                                        op=OP.add)
                nc.vector.tensor_tensor(out=_ap(p0[:], 0, cview),
                                        in0=tap(1, 1), in1=wap("11"),
                                        op=OP.mult)
                nc.vector.tensor_tensor(out=p1[:], in0=p1[:], in1=p0[:],
                                        op=OP.add)
                nc.vector.tensor_tensor(out=acc[:], in0=acc[:], in1=p1[:],
                                        op=OP.add)

                # ---- cost_v = sum_c (ref - cum)^2 (Act square + DVE tree) --
                if v == 0:
                    nc.vector.tensor_tensor(out=diff[:], in0=refC[:],
                                            in1=cum[:], op=OP.subtract)
                else:
                    nc.vector.tensor_tensor(out=diff[:], in0=diff[:],
                                            in1=acc[:], op=OP.subtract)
                sq = bp.tile([128, W * C], F16, tag="s4k")
                nc.scalar.activation(out=sq[:], in_=diff[:], func=AF.Square)
                c8 = wp.tile([128, W * 8], F16, tag="c8")
                nc.vector.tensor_tensor(
                    out=_ap(c8[:], 0, [[W * 8, 128], [8, W], [1, 8]]),
                    in0=_ap(sq[:], 0, [[W * C, 128], [C, W], [1, 8]]),
                    in1=_ap(sq[:], 8, [[W * C, 128], [C, W], [1, 8]]),
                    op=OP.add)
                c4 = wp.tile([128, W * 4], F16, tag="c4")
                nc.vector.tensor_tensor(
                    out=_ap(c4[:], 0, [[W * 4, 128], [4, W], [1, 4]]),
                    in0=_ap(c8[:], 0, [[W * 8, 128], [8, W], [1, 4]]),
                    in1=_ap(c8[:], 4, [[W * 8, 128], [8, W], [1, 4]]),
                    op=OP.add)
                c2 = wp.tile([128, W * 2], F16, tag="c2")
                nc.vector.tensor_tensor(
                    out=_ap(c2[:], 0, [[W * 2, 128], [2, W], [1, 2]]),
                    in0=_ap(c4[:], 0, [[W * 4, 128], [4, W], [1, 2]]),
                    in1=_ap(c4[:], 2, [[W * 4, 128], [4, W], [1, 2]]),
                    op=OP.add)
                cs = wp.tile([128, W], F16, tag=f"csq{v}")
                nc.vector.tensor_tensor(
                    out=cs[:],
                    in0=_ap(c2[:], 0, [[W * 2, 128], [2, W]]),
                    in1=_ap(c2[:], 1, [[W * 2, 128], [2, W]]),
                    op=OP.add)
                csq[v] = cs
                if debug and di == 0:
                    if v == 0:
                        nc.sync.dma_start(out=dbg["dbgCum"].ap(), in_=cum[:])
                        nc.sync.dma_start(out=dbg["dbgCsq0"].ap(), in_=cs[:])
                    else:
                        nc.sync.dma_start(out=dbg["dbgDiff"].ap(), in_=diff[:])
                        nc.sync.dma_start(out=dbg["dbgCsq1"].ap(), in_=cs[:])

            # cost = sqrt(min(c1sq, c2sq)) into costSC5 center block
            cmin = wp.tile([128, W], F16, tag="cmin")
            nc.vector.tensor_tensor(out=cmin[:], in0=csq[0][:], in1=csq[1][:],
                                    op=OP.min)
            if debug and di == 0:
                nc.sync.dma_start(out=dbg["dbgCmin"].ap(), in_=cmin[:])
            costSC5 = wp.tile([128, 5 * PADX], F16, tag="costSC5")
            nc.scalar.activation(
                out=_ap(costSC5[:], 2 * PADX + 2, [[5 * PADX, 128], [1, W]]),
                in_=cmin[:], func=AF.Sqrt)
            nc.vector.memset(
                _ap(costSC5[:], 2 * PADX, [[5 * PADX, 128], [130, 2], [1, 2]]),
                0.0)
            for ty in (0, 1, 3, 4):
                k = ty - 2
                dlo = ty * PADX
                clo = 2 * PADX
                if k < 0:
                    nc.sync.dma_start(
                        out=costSC5[-k:128, dlo:dlo + PADX],
                        in_=costSC5[0:128 + k, clo:clo + PADX])
                    nc.sync.dma_start(out=costSC5[0:-k, dlo:dlo + PADX],
                                      in_=ztile[0:-k, 0:PADX])
                else:
                    nc.sync.dma_start(
                        out=costSC5[0:128 - k, dlo:dlo + PADX],
                        in_=costSC5[k:128, clo:clo + PADX])
                    nc.sync.dma_start(out=costSC5[128 - k:128, dlo:dlo + PADX],
                                      in_=ztile[0:k, 0:PADX])

            # ---- aggregation, tap-major [y, (t25, x)] ----
            dvol = ap_.tile([128, TM], F16, tag="dvol")
            nc.vector.tensor_tensor(
                out=_ap(dvol[:], 0, [[TM, 128], [5 * W, 5], [W, 5], [1, W]]),
                in0=_ap(depSC5[:], 0,
                        [[5 * PADX, 128], [PADX, 5], [1, 5], [1, W]]),
                in1=_ap(depSC5[:], 2 * PADX + 2,
                        [[5 * PADX, 128], [0, 5], [0, 5], [1, W]]),
                op=OP.subtract)
            nc.scalar.activation(out=dvol[:], in_=dvol[:], func=AF.Abs)
            evol = ap_.tile([128, TM], F16, tag="evol")
            nc.scalar.activation(out=evol[:], in_=dvol[:], func=AF.Exp,
                                 bias=tv(7))
            uvol = dvol  # reuse: dvol's last reader is the exp above
            nc.vector.tensor_tensor(out=uvol[:], in0=evol[:], in1=wf25[:],
                                    op=OP.mult)
            nc.vector.tensor_tensor(
                out=_ap(uvol[:], 0, [[TM, 128], [5 * W, 5], [W, 5], [1, W]]),
                in0=_ap(uvol[:], 0, [[TM, 128], [5 * W, 5], [W, 5], [1, W]]),
                in1=_ap(costSC5[:], 0,
                        [[5 * PADX, 128], [PADX, 5], [1, 5], [1, W]]),
                op=OP.mult)

            def tap_tree(vol, nm):
                s12 = wp.tile([128, 12 * W], F16, tag="s12")
                nc.vector.tensor_tensor(out=s12[:], in0=vol[:, 0:12 * W],
                                        in1=vol[:, 12 * W:24 * W], op=OP.add)
                s6 = wp.tile([128, 6 * W], F16, tag="s6")
                nc.vector.tensor_tensor(out=s6[:], in0=s12[:, 0:6 * W],
                                        in1=s12[:, 6 * W:12 * W], op=OP.add)
                s3 = wp.tile([128, 3 * W], F16, tag="s3")
                nc.vector.tensor_tensor(out=s3[:], in0=s6[:, 0:3 * W],
                                        in1=s6[:, 3 * W:6 * W], op=OP.add)
                r = wp.tile([128, W], F16, tag=f"r{nm}")
                nc.vector.tensor_tensor(out=r[:], in0=s3[:, 0:W],
                                        in1=s3[:, W:2 * W], op=OP.add)
                nc.vector.tensor_tensor(out=r[:], in0=r[:],
                                        in1=s3[:, 2 * W:3 * W], op=OP.add)
                nc.vector.tensor_tensor(out=r[:], in0=r[:],
                                        in1=vol[:, 24 * W:25 * W], op=OP.add)
                return r

            num = tap_tree(uvol, "n")
            den = tap_tree(evol, "d")
            if debug and di == 0:
                nc.sync.dma_start(out=dbg["dbgCost"].ap(), in_=costSC5[:])
                nc.sync.dma_start(out=dbg["dbgWf"].ap(), in_=wf25[:])
                nc.sync.dma_start(out=dbg["dbgNum"].ap(), in_=num[:])
                nc.sync.dma_start(out=dbg["dbgDen"].ap(), in_=den[:])
            rden = wp.tile([128, W], F32, tag="rden")
            nc.vector.reciprocal(out=rden[:], in_=den[:])
            agg_ap = _ap(aggT[:], di, [[W * DPC, 128], [DPC, W], [1, 1]])
            nc.vector.tensor_tensor(out=agg_ap, in0=num[:], in1=rden[:],
                                    op=OP.mult)

        # ---------------- per-core softmax partials ----------------
        def aggap(di):
            return _ap(aggT[:], di, [[W * DPC, 128], [DPC, W], [1, 1]])

        def depap(di):
            return _ap(depB[:], di * W, [[DPC * W, 128], [1, W]])

        if debug:
            nc.sync.dma_start(out=dbg["dbgAgg"].ap(), in_=aggT[:])
        m = pp.tile([128, W], F32, tag="m")
        nc.vector.tensor_tensor(out=m[:], in0=aggap(0), in1=aggap(1),
                                op=OP.max)
        for di in range(2, DPC):
            nc.vector.tensor_tensor(out=m[:], in0=m[:], in1=aggap(di),
                                    op=OP.max)
        s0 = pp.tile([128, W], F32, tag="s0")
        s1 = pp.tile([128, W], F32, tag="s1")
        for di in range(DPC):
            t = wp.tile([128, W], F32, tag="et")
            nc.vector.tensor_tensor(out=t[:], in0=aggap(di), in1=m[:],
                                    op=OP.subtract)
            e = wp.tile([128, W], F32, tag="ee")
            nc.scalar.activation(out=e[:], in_=t[:], func=AF.Exp)
            t1 = wp.tile([128, W], F32, tag="t1")
            nc.vector.tensor_tensor(out=t1[:], in0=e[:], in1=depap(di),
                                    op=OP.mult)
            if di == 0:
                nc.vector.tensor_copy(out=s0[:], in_=e[:])
                nc.vector.tensor_copy(out=s1[:], in_=t1[:])
            else:
                nc.vector.tensor_tensor(out=s0[:], in0=s0[:], in1=e[:],
                                        op=OP.add)
                nc.vector.tensor_tensor(out=s1[:], in0=s1[:], in1=t1[:],
                                        op=OP.add)
        nc.sync.dma_start(out=out3.ap()[0], in_=m[:])
        nc.sync.dma_start(out=out3.ap()[1], in_=s0[:])
        nc.sync.dma_start(out=out3.ap()[2], in_=s1[:])

        for p in reversed(ctx_pools):
            p.__exit__(None, None, None)

    nc.compile()
    return nc


def host_prep(features, intrinsics, cam_to_world, depth_hypo):
    """Build the 8 per-core input maps. All O(small) except slicing."""
    features = np.asarray(features, dtype=np.float32)
    intrinsics = np.asarray(intrinsics, dtype=np.float32)
    cam_to_world = np.asarray(cam_to_world, dtype=np.float32)
    depth_hypo = np.asarray(depth_hypo, dtype=np.float32)

    ys, xs = np.meshgrid(np.arange(H, dtype=np.float32),
                         np.arange(W, dtype=np.float32), indexing="ij")
    in_maps = []
    for k in range(NCORES):
        b = k // (NCORES // B)
        dlo = DPC * (k % (NCORES // B))
        amap = np.zeros((NV, 3, H, W), np.float32)
        tvv = np.zeros((12,), np.float32)
        for vi in range(1, V):
            src_w2c = np.linalg.inv(cam_to_world[vi, b])
            ref_w2c = np.linalg.inv(cam_to_world[0, b])
            src_KK = src_w2c.copy()
            src_KK[:3, :3] = intrinsics[vi, b]
            ref_KK = ref_w2c.copy()
            ref_KK[:3, :3] = intrinsics[0, b]
            proj = (src_KK @ src_w2c) @ np.linalg.inv(ref_KK @ ref_w2c)
            rot, trans = proj[:3, :3], proj[:3, 3]
            A = (rot[:, 0:1, None] * xs[None] + rot[:, 1:2, None] * ys[None]
                 + rot[:, 2:3, None])  # [3, H, W]
            v = vi - 1
            amap[v, 0] = A[0] * SCALE
            amap[v, 1] = A[1] * SCALE
            amap[v, 2] = A[2]
            tvv[v * 3 + 0] = trans[0] * SCALE
            tvv[v * 3 + 1] = trans[1] * SCALE
            tvv[v * 3 + 2] = trans[2]
        tvv[6] = 1.5      # floor-by-trunc bias (+2 border shift - 0.5 center)
        tvv[7] = ESHIFT
        tvv[8] = 1.0
        tvv[9] = -1.0
        tvv[10] = 0.5
        in_maps.append({
            "refF": np.ascontiguousarray(features[0, b].transpose(0, 2, 1)),
            "srcF": np.ascontiguousarray(features[1:, b]),
            "dep": np.ascontiguousarray(
                depth_hypo[b, dlo:dlo + DPC].transpose(0, 2, 1)),
            "amap": np.ascontiguousarray(amap.transpose(0, 1, 3, 2)),
            "tvec": np.tile(tvv[None, :], (128, 1)).astype(np.float32),
        })
    return in_maps


def host_combine(results):
    """Merge per-core softmax partials (m, s0, s1) into [B, H, W]."""
    out = np.zeros((B, H, W), np.float32)
    per_b = NCORES // B
    for b in range(B):
        parts = [np.asarray(results[b * per_b + j]["out3"]) for j in range(per_b)]
        parts = [p.transpose(0, 2, 1) for p in parts]  # [3, x, y] -> [3, y, x]
        ms = np.stack([p[0] for p in parts])         # [4, H, W]
        M = ms.max(axis=0)
        S0 = np.zeros((H, W), np.float64)
        S1 = np.zeros((H, W), np.float64)
        for p in parts:
            w = np.exp(p[0] - M)
            S0 += w * p[1]
            S1 += w * p[2]
        out[b] = (S1 / S0).astype(np.float32)
    return out


def _run_sim(nc, in_maps):
    from concourse.bass_interp import CoreSim
    results = []
    for core in range(NCORES):
        sim = CoreSim(nc, trace=False, publish_trace=False)
        for k, v in in_maps[core].items():
            sim.tensor(k)[:] = v
        sim.simulate()
        results.append({"out3": np.array(sim.tensor("out3"))})
    return results


def _get_exec():
    """Build the Bass program once and wrap it in a CACHED jitted
    shard_map executable (the stock run_bass_kernel_spmd path rebuilds
    the jax closure per call -> full retrace + compile every time)."""
    if "exec" in _cached:
        return _cached["exec"]
    import jax
    import concourse.mybir as _mybir
    from concourse import bass2jax
    from jax.experimental.shard_map import shard_map
    from jax.sharding import Mesh, NamedSharding, PartitionSpec

    nc = build_program()
    bass2jax.install_neuronx_cc_hook()

    partition_name = (nc.partition_id_tensor.name
                      if nc.partition_id_tensor else None)
    in_names, out_names, out_avals = [], [], []
    for alloc in nc.m.functions[0].allocations:
        if not isinstance(alloc, _mybir.MemoryLocationSet):
            continue
        name = alloc.memorylocations[0].name
        if alloc.kind == "ExternalInput":
            if name != partition_name:
                in_names.append(name)
        elif alloc.kind == "ExternalOutput":
            shape = tuple(alloc.tensor_shape)
            dtype = _mybir.dt.np(alloc.dtype)
            out_names.append(name)
            out_avals.append(jax.core.ShapedArray(shape, dtype))
    n_params = len(in_names)
    all_names = list(in_names) + list(out_names)
    if partition_name is not None:
        all_names.append(partition_name)
    donate = tuple(range(n_params, n_params + len(out_names)))

    def _body(*args):
        operands = list(args)
        if partition_name is not None:
            operands.append(bass2jax.partition_id_tensor())
        outs = bass2jax._bass_exec_p.bind(
            *operands,
            out_avals=tuple(out_avals),
            in_names=tuple(all_names),
            out_names=tuple(out_names),
            lowering_input_output_aliases=(),
            sim_require_finite=True,
            sim_require_nnan=True,
            nc=nc,
        )
        return tuple(outs)

    devices = jax.devices()[:NCORES]
    mesh = Mesh(np.asarray(devices), ("core",))
    n_in = n_params + len(out_names)
    jitted = jax.jit(
        shard_map(_body, mesh=mesh,
                  in_specs=(PartitionSpec("core"),) * n_in,
                  out_specs=(PartitionSpec("core"),) * len(out_names),
                  check_rep=False),
        donate_argnums=donate, keep_unused=True)
    sharding = NamedSharding(mesh, PartitionSpec("core"))
    st = {"nc": nc, "jitted": jitted, "in_names": in_names,
          "out_names": out_names, "out_avals": out_avals,
          "dbg_name": nc.dbg_addr.name if nc.dbg_addr is not None else None,
          "sharding": sharding}
    _cached["exec"] = st
    return st


def _concat_inputs(st, in_maps):
    dbg = st["dbg_name"]
    if dbg is not None:
        in_maps = [{**m, dbg: np.zeros((1, 2), np.uint32)} for m in in_maps]
    return [np.concatenate([np.asarray(m[name]) for m in in_maps], axis=0)
            for name in st["in_names"]]


def _zero_outs(st):
    return [np.zeros((NCORES * a.shape[0],) + tuple(a.shape[1:]), a.dtype)
            for a in st["out_avals"]]


def _run_cached(st, in_maps):
    out_arrs = st["jitted"](*_concat_inputs(st, in_maps), *_zero_outs(st))
    res = []
    for c in range(NCORES):
        res.append({name: np.asarray(out_arrs[i]).reshape(
            (NCORES,) + tuple(st["out_avals"][i].shape))[c]
            for i, name in enumerate(st["out_names"])})
    return res


def kernel(**inputs):
    in_maps = host_prep(**inputs)
    if _cached.get("hw_broken"):
        if "nc" not in _cached:
            _cached["nc"] = build_program()
        return host_combine(_run_sim(_cached["nc"], in_maps))
    try:
        st = _get_exec()
        return host_combine(_run_cached(st, in_maps))
    except Exception:
        try:
            if "nc" not in _cached:
                _cached["nc"] = build_program()
            res = run_bass_kernel_spmd(_cached["nc"], in_maps,
                                       core_ids=list(range(NCORES)))
            return host_combine(res.results)
        except Exception:
            _cached["hw_broken"] = True
            return host_combine(_run_sim(_cached["nc"], in_maps))


if __name__ == "__main__":
    import reference
    inp = reference.setup_inputs()
    inp = {k: np.asarray(v) for k, v in inp.items()}
    out = kernel(**inp)
    print("kernel out", out.shape, out.dtype)



# revision 39
# speedup vs baseline: 10108.1857x; 1.0320x over previous
"""Trainium2 Bass kernel for nn_DepthPrediction (multi-view stereo depth).

Strategy (8 NeuronCores, SPMD single program):
  - Shard: core k handles batch b = k//4 and depth planes [8*(k%4), 8*(k%4)+8).
  - Per (b,d): homography warp of 2 src views via on-device fp16 "patch maps"
    (132x132 positions x [dy2,dx2,c16] fp16 chunks = 128B) gathered with
    standard indirect DMAs (DynamicAP). The HW DGE honors ONE dynamic
    offset per partition (scalar_dynamic_offset), so each x column is one
    128-descriptor indirect DMA: 128 calls per (plane, view). Bilinear
    interp + cumulative cost (L2 over 16 ch) in pixel-major fp16 on DVE;
    5x5 adaptive aggregation (depth-similarity softmax x feature-similarity
    weight) in tap-major layout; per-core softmax partials over the 8
    local planes. Warp coordinates/bilinear weights are batched over all
    8 planes per view before the depth loop.
  - Engine budget: DVE does the packed-fp16 tensor math; Activation does
    affine/exp/ln/square/casts (single act table: natural_log_exp);
    Pool does the indirect gathers + memsets; SP does DMAs.
  - NOTE: custom-ucode GPSIMD ops (dma_gather/ap_gather) hard-crash this
    runtime (NRT_EXEC_UNIT_UNRECOVERABLE) — only stock-NEFF instructions
    are used here.
  - Host: trivial glue — 4x4 matrix algebra, shard/pack inputs, combine the
    4-way softmax partials per batch (log-sum-exp merge) into the output.
  - Execution: the jitted shard_map executable is built ONCE and cached;
    repeat kernel() calls reuse it (the stock per-call path retraces).

Self-contained: hardcodes all shapes from the problem spec.
"""

import numpy as np

import concourse.bacc as bacc
import concourse.bass as bass
import concourse.mybir as mybir
import concourse.tile as tile
from concourse.bass_utils import run_bass_kernel_spmd
from concourse.tile_rust import add_dep_helper

F32 = mybir.dt.float32
F16 = mybir.dt.float16
I32 = mybir.dt.int32
I16 = mybir.dt.int16

AF = mybir.ActivationFunctionType
OP = mybir.AluOpType
AX = mybir.AxisListType

# problem shapes
V, B, C, H, W, D = 3, 2, 16, 128, 128, 32
NCORES = 8
DPC = D // (NCORES // B)  # depth planes per core = 8
NV = V - 1  # src views = 2

PW = W + 4          # padded map width (x0 in [-2,129] -> cols 0..131)
POS = PW * PW       # patch positions
CH = 64             # chunk elems: [dy2,dx2,c16] fp16 = 128B
PADX = W + 4        # x-padded tiles for 5-tap aggregation
SCALE = W / (W - 1.0)  # grid_sample align_corners=False fold
ESHIFT = -9.0       # exp(|dnb-d|) stabilization shift (|delta| <= 9)
NT = 25             # aggregation taps
TM = NT * W         # tap-major volume free size
NDQ = 1             # SWDGE dynamic queues (2 measured no faster than 1)

_cached = {}


def _ap(base, off, dims):
    """Raw AP on the same tensor as `base` (an AP), offset in elements."""
    return bass.AP(base.tensor, base.offset + off, dims)


def build_program(debug=False, ablate_gather=False):
    # NOTE: do NOT reorder the get_activation_tables() dict — the emitted
    # act_func_set_id indexes the ORIGINAL act_info.json order; walrus
    # resolves ids against that file, so a reorder makes HW load the wrong
    # table (Ln evaluated off the exp table = garbage). CoreSim ignores
    # tables, which hid this.
    nc = bacc.Bacc("TRN2", target_bir_lowering=False, debug=False,
                   num_devices=NCORES, num_swdge_queues=NDQ)

    refF = nc.dram_tensor("refF", [C, H, W], F32, kind="ExternalInput")
    srcF = nc.dram_tensor("srcF", [NV, C, H, W], F32, kind="ExternalInput")
    dep = nc.dram_tensor("dep", [DPC, H, W], F32, kind="ExternalInput")
    amap = nc.dram_tensor("amap", [NV, 3, H, W], F32, kind="ExternalInput")
    tvec = nc.dram_tensor("tvec", [128, 12], F32, kind="ExternalInput")
    out3 = nc.dram_tensor("out3", [3, H, W], F32, kind="ExternalOutput")
    # internal patch maps, one per src view
    pmap = [nc.dram_tensor(f"pmap{v}", [POS, CH], F16, kind="Internal")
            for v in range(NV)]
    dbg = {}
    if debug:
        for nm, shape, dt in (
                ("dbgPos", [128, W], I32), ("dbgG", [128, W * CH], F16),
                ("dbgCum", [128, W * C], F16), ("dbgDiff", [128, W * C], F16),
                ("dbgCsq0", [128, W], F16), ("dbgCsq1", [128, W], F16),
                ("dbgCost", [128, 5 * PADX], F16),
                ("dbgCmin", [128, W], F16),
                ("dbgWf", [128, NT * W], F16),
                ("dbgNum", [128, W], F16), ("dbgDen", [128, W], F16),
                ("dbgAgg", [128, W * DPC], F32),
                ("dbgPmap", [POS, CH], F16)):
            dbg[nm] = nc.dram_tensor(nm, shape, dt, kind="ExternalOutput")

    with nc.allow_low_precision("fp16 pipeline by design"), \
            tile.TileContext(nc) as tc:
        ctx_pools = []

        def pool(name, bufs=1, **kw):
            p = tc.tile_pool(name=name, bufs=bufs, **kw)
            ctx_pools.append(p)
            return p.__enter__()

        pp = pool("persist", 1)     # long-lived tiles
        prep = tc.tile_pool(name="prep", bufs=1)
        pr = prep.__enter__()

        # ---------------- constant / persistent loads ----------------
        amapL = pp.tile([128, NV * 3 * W], F32, tag="amapL")  # [y,(v,row,x)]
        nc.sync.dma_start(out=amapL[:], in_=_ap(
            amap.ap(), 0, [[W, 128], [3 * H * W, NV], [H * W, 3], [1, W]]))
        tvecT = pp.tile([128, 12], F32, tag="tvecT")
        nc.sync.dma_start(out=tvecT[:], in_=tvec.ap())

        def tv(col):  # [128,1] per-partition scalar AP
            return tvecT[:, col:col + 1]

        # zero-source tile for edge DMAs (f16)
        ztile = pp.tile([128, PADX * C], F16, tag="ztile")
        nc.gpsimd.memset(ztile[:], 0.0)

        # ---------------- ref prep ----------------
        tr = pr.tile([128, C * W], F32, tag="tr")  # [y,(c,x)] f32
        nc.sync.dma_start(out=tr[:], in_=_ap(
            refF.ap(), 0, [[W, 128], [H * W, C], [1, W]]))
        # refC [y,(x,c)] fp16
        refC = pp.tile([128, W * C], F16, tag="refC")
        nc.vector.tensor_copy(
            out=_ap(refC[:], 0, [[W * C, 128], [C, W], [1, C]]),
            in_=_ap(tr[:], 0, [[C * W, 128], [1, W], [W, C]]))
        # refPadC [y,(xp132,c)] fp16, x' = x+2
        refPadC = pr.tile([128, PADX * C], F16, tag="refPadC")
        nc.vector.memset(
            _ap(refPadC[:], 0, [[PADX * C, 128], [C, 2], [1, C]]), 0.0)
        nc.vector.memset(
            _ap(refPadC[:], 130 * C, [[PADX * C, 128], [C, 2], [1, C]]), 0.0)
        nc.scalar.copy(out=refPadC[:, 2 * C:(2 + W) * C], in_=refC[:])
        # 4 partition-shifted copies (ty in {0,1,3,4}; center ty=2 = refPadC)
        refSC = {}
        for ty in (0, 1, 3, 4):
            t = pr.tile([128, PADX * C], F16, tag=f"refSC{ty}")
            k = ty - 2
            if k < 0:
                nc.sync.dma_start(out=t[-k:128, :], in_=refPadC[0:128 + k, :])
                nc.sync.dma_start(out=t[0:-k, :], in_=ztile[0:-k, :])
            else:
                nc.sync.dma_start(out=t[0:128 - k, :], in_=refPadC[k:128, :])
                nc.sync.dma_start(out=t[128 - k:128, :], in_=ztile[0:k, :])
            refSC[ty] = t
        refSC[2] = refPadC

        # ---------------- patch map build (per src view) ----------------
        build_dumps = []
        for v in range(NV):
            # staged [y, (xp132, dy2, dx2, c16, pad64)] fp16; row y -> pos
            # row y+2.  Zero only the data slots of border cols xp in
            # {0,1} u {129,130,131}; pad elems are never read downstream.
            staged = pr.tile([128, PW * CH], F16, tag=f"staged{v}")
            nc.vector.memset(
                _ap(staged[:], 0, [[PW * CH, 128], [CH, 2], [1, 64]]), 0.0)
            nc.vector.memset(
                _ap(staged[:], 129 * CH, [[PW * CH, 128], [CH, 3], [1, 64]]),
                0.0)
            # TF16 [y, (c,x)] fp16  (cast during DMA, SWDGE)
            tf = pr.tile([128, C * W], F16, tag="tf")
            nc.gpsimd.dma_start(
                out=tf[:], in_=_ap(srcF.ap(), v * C * H * W,
                                   [[W, 128], [H * W, C], [1, W]]))
            # down-shifted copy: tfdn[y] = tf[y+1]; row 127 = 0
            tfdn = pr.tile([128, C * W], F16, tag="tfdn")
            nc.sync.dma_start(out=tfdn[0:127, :], in_=tf[1:128, :])
            nc.sync.dma_start(out=tfdn[127:128, :],
                              in_=_ap(ztile[:], 0, [[PADX * C, 1], [1, C * W]]))

            copy_fns = [nc.vector.tensor_copy,
                        lambda out, in_: nc.scalar.copy(out=out, in_=in_),
                        nc.gpsimd.tensor_copy,
                        lambda out, in_: nc.scalar.copy(out=out, in_=in_)]
            i = 0
            for dy, src in ((0, tf), (1, tfdn)):
                for dx in (0, 1):
                    # staged[y, xp, dy, dx, c] = src[y, c, xp-2+dx]
                    xp_lo = 2 - dx
                    n_xp = 128
                    src_ap = _ap(src[:], 0,
                                 [[C * W, 128], [1, n_xp], [W, C]])
                    dst_ap = _ap(staged[:], xp_lo * CH + dy * 32 + dx * 16,
                                 [[PW * CH, 128], [CH, n_xp], [1, C]])
                    copy_fns[i](out=dst_ap, in_=src_ap)
                    i += 1
            # dump rows 2..129 of the pos grid (partition y -> row y+2)
            d1 = nc.sync.dma_start(
                out=_ap(pmap[v].ap(), 2 * PW * CH,
                        [[PW * CH, 128], [1, PW * CH]]),
                in_=staged[:])
            # zero rows 0, 130, 131 from ztile (128x132 slice covers a row)
            dz = []
            for r in (0, 130, 131):
                dz.append(nc.sync.dma_start(
                    out=_ap(pmap[v].ap(), r * PW * CH,
                            [[PW * CH, 1], [1, PW * CH]]),
                    in_=_ap(ztile[:], 0, [[PADX * C, 8], [1, PW * CH // 8]])))
            # row r=1 (y0=-1): dy=1 slots hold F row 0 = staged[0]'s dy=0
            # window (same cells, slot offset +32); rest zero
            d4z = nc.sync.dma_start(
                out=_ap(pmap[v].ap(), PW * CH, [[PW * CH, 1], [1, PW * CH]]),
                in_=_ap(ztile[:], 0, [[PADX * C, 8], [1, PW * CH // 8]]))
            d4 = nc.sync.dma_start(
                out=_ap(pmap[v].ap(), PW * CH + 32,
                        [[PW * CH, 1], [CH, PW], [1, 32]]),
                in_=_ap(staged[0:1, :], 0, [[PW * CH, 1], [CH, PW], [1, 32]]))
            add_dep_helper(d4.ins, d4z.ins, reason="row1 zero before windows")
            build_dumps.append(tuple([d1, d4, d4z] + dz))
            if debug and v == 0:
                dp = nc.sync.dma_start(
                    out=_ap(dbg["dbgPmap"].ap(), 0,
                            [[PW * CH, PW], [1, PW * CH]]),
                    in_=_ap(pmap[0].ap(), 0, [[PW * CH, PW], [1, PW * CH]]))
                for dma_i in build_dumps[0]:
                    add_dep_helper(dp.ins, dma_i.ins,
                                   reason="pmap build before debug dump")

        # ---------------- w_feat volume [y, (t25, x)] fp16 ----
        wf25 = pp.tile([128, TM], F16, tag="wf25")
        nc.vector.memset(wf25[:, 12 * W:13 * W], 0.0)
        for ty in (2, 1, 3, 0, 4):
            for tx in range(5):
                t = ty * 5 + tx
                if t == 12:
                    continue
                # diff = ref[y,x,c] - refSC[ty][y, x+tx, c]
                dfw = pr.tile([128, W * C], F16, tag="wfdf")
                nc.vector.tensor_tensor(
                    out=dfw[:],
                    in0=_ap(refSC[ty][:], tx * C,
                            [[PADX * C, 128], [C, W], [1, C]]),
                    in1=refC[:], op=OP.subtract)
                sqw = pr.tile([128, W * C], F16, tag="wfsq")
                nc.scalar.activation(out=sqw[:], in_=dfw[:], func=AF.Square)
                # c16 tree reduction (packed fp16 halves)
                c8 = pr.tile([128, W * 8], F16, tag="c8w")
                nc.vector.tensor_tensor(
                    out=_ap(c8[:], 0, [[W * 8, 128], [8, W], [1, 8]]),
                    in0=_ap(sqw[:], 0, [[W * C, 128], [C, W], [1, 8]]),
                    in1=_ap(sqw[:], 8, [[W * C, 128], [C, W], [1, 8]]),
                    op=OP.add)
                c4 = pr.tile([128, W * 4], F16, tag="c4w")
                nc.vector.tensor_tensor(
                    out=_ap(c4[:], 0, [[W * 4, 128], [4, W], [1, 4]]),
                    in0=_ap(c8[:], 0, [[W * 8, 128], [8, W], [1, 4]]),
                    in1=_ap(c8[:], 4, [[W * 8, 128], [8, W], [1, 4]]),
                    op=OP.add)
                c2 = pr.tile([128, W * 2], F16, tag="c2w")
                nc.vector.tensor_tensor(
                    out=_ap(c2[:], 0, [[W * 2, 128], [2, W], [1, 2]]),
                    in0=_ap(c4[:], 0, [[W * 4, 128], [4, W], [1, 2]]),
                    in1=_ap(c4[:], 2, [[W * 4, 128], [4, W], [1, 2]]),
                    op=OP.add)
                cs = pr.tile([128, W], F16, tag="csw")
                nc.vector.tensor_tensor(
                    out=cs[:],
                    in0=_ap(c2[:], 0, [[W * 2, 128], [2, W]]),
                    in1=_ap(c2[:], 1, [[W * 2, 128], [2, W]]),
                    op=OP.add)
                # wf = sqrt(cs)  (direct Sqrt act; Ln is table-hostile here)
                nc.scalar.activation(out=wf25[:, t * W:(t + 1) * W],
                                     in_=cs[:], func=AF.Sqrt)


        prep.__exit__(None, None, None)

        # ---------------- batched warp coordinates (all planes) ----------
        # depB [y, (d, x)] f32; posIv / wdB computed once per view for all
        # DPC planes -> the per-(d,v) loop only gathers + interpolates.
        depB = pp.tile([128, DPC * W], F32, tag="depB")
        nc.sync.dma_start(out=depB[:], in_=_ap(
            dep.ap(), 0, [[W, 128], [H * W, DPC], [1, W]]))
        posIv = [pp.tile([128, DPC * W], I32, tag=f"posIv{v}",
                         name=f"posIv{v}") for v in range(NV)]
        wdB = [{}, {}]
        for v in range(NV):
            for tnm in ("00", "01", "10", "11"):
                wdB[v][tnm] = pp.tile([128, DPC * W * 2], F16,
                                      tag=f"wdB{v}{tnm}",
                                      name=f"wdB{v}{tnm}")

        cprep = tc.tile_pool(name="coord", bufs=1)
        cq = cprep.__enter__()
        DW = DPC * W
        for v in range(NV):
            def arowB(r):
                return _ap(amapL[:], (v * 3 + r) * W,
                           [[NV * 3 * W, 128], [0, DPC], [1, W]])
            mx = cq.tile([128, DW], F32, tag="mxB")
            my = cq.tile([128, DW], F32, tag="myB")
            dn = cq.tile([128, DW], F32, tag="dnB")
            nc.vector.tensor_tensor(out=mx[:], in0=arowB(0), in1=depB[:],
                                    op=OP.mult)
            nc.vector.tensor_tensor(out=my[:], in0=arowB(1), in1=depB[:],
                                    op=OP.mult)
            nc.vector.tensor_tensor(out=dn[:], in0=arowB(2), in1=depB[:],
                                    op=OP.mult)
            nx = cq.tile([128, DW], F32, tag="nxB")
            ny = cq.tile([128, DW], F32, tag="nyB")
            dnt = cq.tile([128, DW], F32, tag="dntB")
            nc.scalar.activation(out=nx[:], in_=mx[:], func=AF.Identity,
                                 bias=tv(v * 3 + 0))
            nc.scalar.activation(out=ny[:], in_=my[:], func=AF.Identity,
                                 bias=tv(v * 3 + 1))
            nc.scalar.activation(out=dnt[:], in_=dn[:], func=AF.Identity,
                                 bias=tv(v * 3 + 2))
            rec = cq.tile([128, DW], F32, tag="recB")
            nc.vector.reciprocal(out=rec[:], in_=dnt[:])
            gx = cq.tile([128, DW], F32, tag="gxB")
            gy = cq.tile([128, DW], F32, tag="gyB")
            nc.vector.tensor_tensor(out=gx[:], in0=nx[:], in1=rec[:],
                                    op=OP.mult)
            nc.vector.tensor_tensor(out=gy[:], in0=ny[:], in1=rec[:],
                                    op=OP.mult)

            # floor(g + 1.5) exact under round-to-nearest-even OR trunc
            def floorfracB(g, nm):
                xi = cq.tile([128, DW], I32, tag=f"iB{nm}")
                nc.scalar.activation(out=xi[:], in_=g[:],
                                     func=AF.Identity, bias=tv(6))
                xf = cq.tile([128, DW], F32, tag=f"xfB{nm}")
                nc.scalar.copy(out=xf[:], in_=xi[:])
                fr = cq.tile([128, DW], F32, tag=f"frB{nm}")
                nc.vector.scalar_tensor_tensor(
                    out=fr[:], in0=g[:], scalar=1.5, in1=xf[:],
                    op0=OP.add, op1=OP.subtract)
                neg = cq.tile([128, DW], F32, tag=f"ngB{nm}")
                nc.vector.tensor_scalar(out=neg[:], in0=fr[:],
                                        scalar1=0.0, scalar2=None,
                                        op0=OP.is_lt)
                nc.vector.tensor_tensor(out=fr[:], in0=fr[:],
                                        in1=neg[:], op=OP.add)
                xa = cq.tile([128, DW], F32, tag=f"xaB{nm}")
                nc.vector.tensor_tensor(out=xa[:], in0=xf[:],
                                        in1=neg[:], op=OP.subtract)
                xc = cq.tile([128, DW], F32, tag=f"xcB{nm}")
                nc.vector.tensor_scalar(out=xc[:], in0=xa[:],
                                        scalar1=131.0, scalar2=0.0,
                                        op0=OP.min, op1=OP.max)
                return xc, fr

            xsC, fx = floorfracB(gx, "x")
            ysC, fy = floorfracB(gy, "y")
            posF = cq.tile([128, DW], F32, tag="posFB")
            nc.vector.scalar_tensor_tensor(
                out=posF[:], in0=ysC[:], scalar=float(PW), in1=xsC[:],
                op0=OP.mult, op1=OP.add)
            nc.scalar.copy(out=posIv[v][:], in_=posF[:])

            # fp16 fractional weights + bilinear tap weights, batched
            fx16 = cq.tile([128, DW], F16, tag="fx16B")
            fy16 = cq.tile([128, DW], F16, tag="fy16B")
            fxm16 = cq.tile([128, DW], F16, tag="fxm16B")
            fym16 = cq.tile([128, DW], F16, tag="fym16B")
            nc.scalar.copy(out=fx16[:], in_=fx[:])
            nc.scalar.copy(out=fy16[:], in_=fy[:])
            nc.scalar.activation(out=fxm16[:], in_=fx[:],
                                 func=AF.Identity, scale=tv(9), bias=tv(8))
            nc.scalar.activation(out=fym16[:], in_=fy[:],
                                 func=AF.Identity, scale=tv(9), bias=tv(8))
            for (tnm, fa, fb) in (("00", fxm16, fym16), ("01", fx16, fym16),
                                  ("10", fxm16, fy16), ("11", fx16, fy16)):
                nc.vector.tensor_tensor(
                    out=_ap(wdB[v][tnm][:], 0,
                            [[DW * 2, 128], [2, DW], [1, 2]]),
                    in0=_ap(fa[:], 0, [[DW, 128], [1, DW], [0, 2]]),
                    in1=_ap(fb[:], 0, [[DW, 128], [1, DW], [0, 2]]),
                    op=OP.mult)
        cprep.__exit__(None, None, None)

        wp = pool("work", 2)        # small per-(b,d) working tiles
        bp = pool("big4", 4)        # shared 4KB scratch (tag s4k)
        cp = pool("cumdif", 1)      # cum/diff accumulators
        gp = pool("gath", 2)        # gather destinations
        ap_ = pool("aggp", 2)       # agg volumes

        # agg store
        aggT = pp.tile([128, W * DPC], F32, tag="aggT")

        # ---------------- per depth-plane pipeline ----------------
        for di in range(DPC):
            def depf_ap():
                return _ap(depB[:], di * W, [[DPC * W, 128], [1, W]])
            # depSC5 [y, (ty5, xp132)] fp16: center block + 4 shifted blocks
            depSC5 = wp.tile([128, 5 * PADX], F16, tag="depSC5")
            nc.scalar.copy(
                out=_ap(depSC5[:], 2 * PADX + 2, [[5 * PADX, 128], [1, W]]),
                in_=depf_ap())
            nc.vector.memset(
                _ap(depSC5[:], 2 * PADX, [[5 * PADX, 128], [130, 2], [1, 2]]),
                0.0)
            for ty in (0, 1, 3, 4):
                k = ty - 2
                dlo = ty * PADX
                clo = 2 * PADX
                if k < 0:
                    nc.sync.dma_start(
                        out=depSC5[-k:128, dlo:dlo + PADX],
                        in_=depSC5[0:128 + k, clo:clo + PADX])
                    nc.sync.dma_start(out=depSC5[0:-k, dlo:dlo + PADX],
                                      in_=ztile[0:-k, 0:PADX])
                else:
                    nc.sync.dma_start(
                        out=depSC5[0:128 - k, dlo:dlo + PADX],
                        in_=depSC5[k:128, clo:clo + PADX])
                    nc.sync.dma_start(out=depSC5[128 - k:128, dlo:dlo + PADX],
                                      in_=ztile[0:k, 0:PADX])

            cum = cp.tile([128, W * C], F16, tag="cum")
            diff = cp.tile([128, W * C], F16, tag="diff")
            csq = {}
            for v in range(NV):
                # ---- gather: HW DynamicAP honors ONE offset per partition
                # (scalar_dynamic_offset DGE level; vector offsets are
                # compiled out), so issue one 128-descriptor indirect DMA
                # per x column: idx [128,1] -> dest [128, CH].
                G = gp.tile([128, W * CH], F16, tag="G")
                if ablate_gather:
                    nc.vector.memset(G[:], 0.25)
                else:
                    prev = [None] * NDQ
                    for j in range(W):
                        q = j % NDQ
                        gi = nc.gpsimd.indirect_dma_start(
                            out=_ap(G[:], j * CH, [[W * CH, 128], [1, CH]]),
                            out_offset=None,
                            in_=pmap[v].ap(),
                            in_offset=bass.IndirectOffsetOnAxis(
                                ap=posIv[v][:, di * W + j:di * W + j + 1],
                                axis=0))
                        if q:
                            gi.ins.queue = f"qPoolDynamic{q}"
                        if prev[q] is None:
                            for dma_i in build_dumps[v]:
                                add_dep_helper(
                                    gi.ins, dma_i.ins,
                                    reason="pmap build before gather")
                        else:
                            add_dep_helper(gi.ins, prev[q].ins,
                                           reason="gather chain (same queue)")
                        prev[q] = gi
                if debug and di == 0 and v == 0:
                    nc.sync.dma_start(
                        out=dbg["dbgPos"].ap(),
                        in_=_ap(posIv[0][:], 0, [[DPC * W, 128], [1, W]]))
                    nc.sync.dma_start(out=dbg["dbgG"].ap(), in_=G[:])

                # ---- bilinear taps: acc = sum_t w_t * G_t  [y,(x,c)] ----
                def tap(dy, dx):
                    return _ap(G[:], (dy * 2 + dx) * 16,
                               [[W * CH, 128], [CH, W], [2, 8], [1, 2]])

                def wap(tnm):
                    return _ap(wdB[v][tnm][:], di * W * 2,
                               [[DPC * W * 2, 128], [2, W], [0, 8], [1, 2]])

                cview = [[2048, 128], [16, W], [2, 8], [1, 2]]
                acc = cum if v == 0 else bp.tile([128, W * C], F16, tag="s4k")
                p0 = bp.tile([128, W * C], F16, tag="s4k")
                p1 = bp.tile([128, W * C], F16, tag="s4k")
                nc.vector.tensor_tensor(out=_ap(acc[:], 0, cview),
                                        in0=tap(0, 0), in1=wap("00"),
                                        op=OP.mult)
                nc.vector.tensor_tensor(out=_ap(p0[:], 0, cview),
                                        in0=tap(0, 1), in1=wap("01"),
                                        op=OP.mult)
                nc.vector.tensor_tensor(out=_ap(p1[:], 0, cview),
                                        in0=tap(1, 0), in1=wap("10"),
                                        op=OP.mult)
                nc.vector.tensor_tensor(out=acc[:], in0=acc[:], in1=p0[:],
                                        op=OP.add)
                nc.vector.tensor_tensor(out=_ap(p0[:], 0, cview),
                                        in0=tap(1, 1), in1=wap("11"),
                                        op=OP.mult)
                nc.vector.tensor_tensor(out=p1[:], in0=p1[:], in1=p0[:],
                                        op=OP.add)
                nc.vector.tensor_tensor(out=acc[:], in0=acc[:], in1=p1[:],
                                        op=OP.add)

                # ---- cost_v = sum_c (ref - cum)^2 (Act square + DVE tree) --
                if v == 0:
                    nc.vector.tensor_tensor(out=diff[:], in0=refC[:],
                                            in1=cum[:], op=OP.subtract)
                else:
                    nc.vector.tensor_tensor(out=diff[:], in0=diff[:],
                                            in1=acc[:], op=OP.subtract)
                sq = bp.tile([128, W * C], F16, tag="s4k")
                nc.scalar.activation(out=sq[:], in_=diff[:], func=AF.Square)
                c8 = wp.tile([128, W * 8], F16, tag="c8")
                nc.vector.tensor_tensor(
                    out=_ap(c8[:], 0, [[W * 8, 128], [8, W], [1, 8]]),
                    in0=_ap(sq[:], 0, [[W * C, 128], [C, W], [1, 8]]),
                    in1=_ap(sq[:], 8, [[W * C, 128], [C, W], [1, 8]]),
                    op=OP.add)
                c4 = wp.tile([128, W * 4], F16, tag="c4")
                nc.vector.tensor_tensor(
                    out=_ap(c4[:], 0, [[W * 4, 128], [4, W], [1, 4]]),
                    in0=_ap(c8[:], 0, [[W * 8, 128], [8, W], [1, 4]]),
                    in1=_ap(c8[:], 4, [[W * 8, 128], [8, W], [1, 4]]),
                    op=OP.add)
                c2 = wp.tile([128, W * 2], F16, tag="c2")
                nc.vector.tensor_tensor(
                    out=_ap(c2[:], 0, [[W * 2, 128], [2, W], [1, 2]]),
                    in0=_ap(c4[:], 0, [[W * 4, 128], [4, W], [1, 2]]),
                    in1=_ap(c4[:], 2, [[W * 4, 128], [4, W], [1, 2]]),
                    op=OP.add)
                cs = wp.tile([128, W], F16, tag=f"csq{v}")
                nc.vector.tensor_tensor(
                    out=cs[:],
                    in0=_ap(c2[:], 0, [[W * 2, 128], [2, W]]),
                    in1=_ap(c2[:], 1, [[W * 2, 128], [2, W]]),
                    op=OP.add)
                csq[v] = cs
                if debug and di == 0:
                    if v == 0:
                        nc.sync.dma_start(out=dbg["dbgCum"].ap(), in_=cum[:])
                        nc.sync.dma_start(out=dbg["dbgCsq0"].ap(), in_=cs[:])
                    else:
                        nc.sync.dma_start(out=dbg["dbgDiff"].ap(), in_=diff[:])
                        nc.sync.dma_start(out=dbg["dbgCsq1"].ap(), in_=cs[:])

            # cost = sqrt(min(c1sq, c2sq)) into costSC5 center block
            cmin = wp.tile([128, W], F16, tag="cmin")
            nc.vector.tensor_tensor(out=cmin[:], in0=csq[0][:], in1=csq[1][:],
                                    op=OP.min)
            if debug and di == 0:
                nc.sync.dma_start(out=dbg["dbgCmin"].ap(), in_=cmin[:])
            costSC5 = wp.tile([128, 5 * PADX], F16, tag="costSC5")
            nc.scalar.activation(
                out=_ap(costSC5[:], 2 * PADX + 2, [[5 * PADX, 128], [1, W]]),
                in_=cmin[:], func=AF.Sqrt)
            nc.vector.memset(
                _ap(costSC5[:], 2 * PADX, [[5 * PADX, 128], [130, 2], [1, 2]]),
                0.0)
            for ty in (0, 1, 3, 4):
                k = ty - 2
                dlo = ty * PADX
                clo = 2 * PADX
                if k < 0:
                    nc.sync.dma_start(
                        out=costSC5[-k:128, dlo:dlo + PADX],
                        in_=costSC5[0:128 + k, clo:clo + PADX])
                    nc.sync.dma_start(out=costSC5[0:-k, dlo:dlo + PADX],
                                      in_=ztile[0:-k, 0:PADX])
                else:
                    nc.sync.dma_start(
                        out=costSC5[0:128 - k, dlo:dlo + PADX],
                        in_=costSC5[k:128, clo:clo + PADX])
                    nc.sync.dma_start(out=costSC5[128 - k:128, dlo:dlo + PADX],
                                      in_=ztile[0:k, 0:PADX])

            # ---- aggregation, tap-major [y, (t25, x)] ----
            dvol = ap_.tile([128, TM], F16, tag="dvol")
            nc.vector.tensor_tensor(
                out=_ap(dvol[:], 0, [[TM, 128], [5 * W, 5], [W, 5], [1, W]]),
                in0=_ap(depSC5[:], 0,
                        [[5 * PADX, 128], [PADX, 5], [1, 5], [1, W]]),
                in1=_ap(depSC5[:], 2 * PADX + 2,
                        [[5 * PADX, 128], [0, 5], [0, 5], [1, W]]),
                op=OP.subtract)
            nc.scalar.activation(out=dvol[:], in_=dvol[:], func=AF.Abs)
            evol = ap_.tile([128, TM], F16, tag="evol")
            nc.scalar.activation(out=evol[:], in_=dvol[:], func=AF.Exp,
                                 bias=tv(7))
            uvol = dvol  # reuse: dvol's last reader is the exp above
            nc.vector.tensor_tensor(out=uvol[:], in0=evol[:], in1=wf25[:],
                                    op=OP.mult)
            nc.vector.tensor_tensor(
                out=_ap(uvol[:], 0, [[TM, 128], [5 * W, 5], [W, 5], [1, W]]),
                in0=_ap(uvol[:], 0, [[TM, 128], [5 * W, 5], [W, 5], [1, W]]),
                in1=_ap(costSC5[:], 0,
                        [[5 * PADX, 128], [PADX, 5], [1, 5], [1, W]]),
                op=OP.mult)

            def tap_tree(vol, nm):
                s12 = wp.tile([128, 12 * W], F16, tag="s12")
                nc.vector.tensor_tensor(out=s12[:], in0=vol[:, 0:12 * W],
                                        in1=vol[:, 12 * W:24 * W], op=OP.add)
                s6 = wp.tile([128, 6 * W], F16, tag="s6")
                nc.vector.tensor_tensor(out=s6[:], in0=s12[:, 0:6 * W],
                                        in1=s12[:, 6 * W:12 * W], op=OP.add)
                s3 = wp.tile([128, 3 * W], F16, tag="s3")
                nc.vector.tensor_tensor(out=s3[:], in0=s6[:, 0:3 * W],
                                        in1=s6[:, 3 * W:6 * W], op=OP.add)
                r = wp.tile([128, W], F16, tag=f"r{nm}")
                nc.vector.tensor_tensor(out=r[:], in0=s3[:, 0:W],
                                        in1=s3[:, W:2 * W], op=OP.add)
                nc.vector.tensor_tensor(out=r[:], in0=r[:],
                                        in1=s3[:, 2 * W:3 * W], op=OP.add)
                nc.vector.tensor_tensor(out=r[:], in0=r[:],
                                        in1=vol[:, 24 * W:25 * W], op=OP.add)
                return r

            num = tap_tree(uvol, "n")
            den = tap_tree(evol, "d")
            if debug and di == 0:
                nc.sync.dma_start(out=dbg["dbgCost"].ap(), in_=costSC5[:])
                nc.sync.dma_start(out=dbg["dbgWf"].ap(), in_=wf25[:])
                nc.sync.dma_start(out=dbg["dbgNum"].ap(), in_=num[:])
                nc.sync.dma_start(out=dbg["dbgDen"].ap(), in_=den[:])
            rden = wp.tile([128, W], F32, tag="rden")
            nc.vector.reciprocal(out=rden[:], in_=den[:])
            agg_ap = _ap(aggT[:], di, [[W * DPC, 128], [DPC, W], [1, 1]])
            nc.vector.tensor_tensor(out=agg_ap, in0=num[:], in1=rden[:],
                                    op=OP.mult)

        # ---------------- per-core softmax partials ----------------
        def aggap(di):
            return _ap(aggT[:], di, [[W * DPC, 128], [DPC, W], [1, 1]])

        def depap(di):
            return _ap(depB[:], di * W, [[DPC * W, 128], [1, W]])

        if debug:
            nc.sync.dma_start(out=dbg["dbgAgg"].ap(), in_=aggT[:])
        m = pp.tile([128, W], F32, tag="m")
        nc.vector.tensor_tensor(out=m[:], in0=aggap(0), in1=aggap(1),
                                op=OP.max)
        for di in range(2, DPC):
            nc.vector.tensor_tensor(out=m[:], in0=m[:], in1=aggap(di),
                                    op=OP.max)
        s0 = pp.tile([128, W], F32, tag="s0")
        s1 = pp.tile([128, W], F32, tag="s1")
        for di in range(DPC):
            t = wp.tile([128, W], F32, tag="et")
            nc.vector.tensor_tensor(out=t[:], in0=aggap(di), in1=m[:],
                                    op=OP.subtract)
            e = wp.tile([128, W], F32, tag="ee")
            nc.scalar.activation(out=e[:], in_=t[:], func=AF.Exp)
            t1 = wp.tile([128, W], F32, tag="t1")
            nc.vector.tensor_tensor(out=t1[:], in0=e[:], in1=depap(di),
                                    op=OP.mult)
            if di == 0:
                nc.vector.tensor_copy(out=s0[:], in_=e[:])
                nc.vector.tensor_copy(out=s1[:], in_=t1[:])
            else:
                nc.vector.tensor_tensor(out=s0[:], in0=s0[:], in1=e[:],
                                        op=OP.add)
                nc.vector.tensor_tensor(out=s1[:], in0=s1[:], in1=t1[:],
                                        op=OP.add)
        nc.sync.dma_start(out=out3.ap()[0], in_=m[:])
        nc.sync.dma_start(out=out3.ap()[1], in_=s0[:])
        nc.sync.dma_start(out=out3.ap()[2], in_=s1[:])

        for p in reversed(ctx_pools):
            p.__exit__(None, None, None)

    nc.compile()
    return nc


def host_prep(features, intrinsics, cam_to_world, depth_hypo):
    """Build the 8 per-core input maps. All O(small) except slicing."""
    features = np.asarray(features, dtype=np.float32)
    intrinsics = np.asarray(intrinsics, dtype=np.float32)
    cam_to_world = np.asarray(cam_to_world, dtype=np.float32)
    depth_hypo = np.asarray(depth_hypo, dtype=np.float32)

    ys, xs = np.meshgrid(np.arange(H, dtype=np.float32),
                         np.arange(W, dtype=np.float32), indexing="ij")
    in_maps = []
    for k in range(NCORES):
        b = k // (NCORES // B)
        dlo = DPC * (k % (NCORES // B))
        amap = np.zeros((NV, 3, H, W), np.float32)
        tvv = np.zeros((12,), np.float32)
        for vi in range(1, V):
            src_w2c = np.linalg.inv(cam_to_world[vi, b])
            ref_w2c = np.linalg.inv(cam_to_world[0, b])
            src_KK = src_w2c.copy()
            src_KK[:3, :3] = intrinsics[vi, b]
            ref_KK = ref_w2c.copy()
            ref_KK[:3, :3] = intrinsics[0, b]
            proj = (src_KK @ src_w2c) @ np.linalg.inv(ref_KK @ ref_w2c)
            rot, trans = proj[:3, :3], proj[:3, 3]
            A = (rot[:, 0:1, None] * xs[None] + rot[:, 1:2, None] * ys[None]
                 + rot[:, 2:3, None])  # [3, H, W]
            v = vi - 1
            amap[v, 0] = A[0] * SCALE
            amap[v, 1] = A[1] * SCALE
            amap[v, 2] = A[2]
            tvv[v * 3 + 0] = trans[0] * SCALE
            tvv[v * 3 + 1] = trans[1] * SCALE
            tvv[v * 3 + 2] = trans[2]
        tvv[6] = 1.5      # floor-by-trunc bias (+2 border shift - 0.5 center)
        tvv[7] = ESHIFT
        tvv[8] = 1.0
        tvv[9] = -1.0
        tvv[10] = 0.5
        in_maps.append({
            "refF": np.ascontiguousarray(features[0, b].transpose(0, 2, 1)),
            "srcF": np.ascontiguousarray(features[1:, b]),
            "dep": np.ascontiguousarray(
                depth_hypo[b, dlo:dlo + DPC].transpose(0, 2, 1)),
            "amap": np.ascontiguousarray(amap.transpose(0, 1, 3, 2)),
            "tvec": np.tile(tvv[None, :], (128, 1)).astype(np.float32),
        })
    return in_maps


def host_combine(results):
    """Merge per-core softmax partials (m, s0, s1) into [B, H, W]."""
    out = np.zeros((B, H, W), np.float32)
    per_b = NCORES // B
    for b in range(B):
        parts = [np.asarray(results[b * per_b + j]["out3"]) for j in range(per_b)]
        parts = [p.transpose(0, 2, 1) for p in parts]  # [3, x, y] -> [3, y, x]
        ms = np.stack([p[0] for p in parts])         # [4, H, W]
        M = ms.max(axis=0)
        S0 = np.zeros((H, W), np.float64)
        S1 = np.zeros((H, W), np.float64)
        for p in parts:
            w = np.exp(p[0] - M)
            S0 += w * p[1]
            S1 += w * p[2]
        out[b] = (S1 / S0).astype(np.float32)
    return out


def _run_sim(nc, in_maps):
    from concourse.bass_interp import CoreSim
    results = []
    for core in range(NCORES):
        sim = CoreSim(nc, trace=False, publish_trace=False)
        for k, v in in_maps[core].items():
            sim.tensor(k)[:] = v
        sim.simulate()
        results.append({"out3": np.array(sim.tensor("out3"))})
    return results


def _get_exec():
    """Build the Bass program once and wrap it in a CACHED jitted
    shard_map executable (the stock run_bass_kernel_spmd path rebuilds
    the jax closure per call -> full retrace + compile every time)."""
    if "exec" in _cached:
        return _cached["exec"]
    import jax
    import concourse.mybir as _mybir
    from concourse import bass2jax
    from jax.experimental.shard_map import shard_map
    from jax.sharding import Mesh, NamedSharding, PartitionSpec

    nc = build_program()
    bass2jax.install_neuronx_cc_hook()

    partition_name = (nc.partition_id_tensor.name
                      if nc.partition_id_tensor else None)
    in_names, out_names, out_avals = [], [], []
    for alloc in nc.m.functions[0].allocations:
        if not isinstance(alloc, _mybir.MemoryLocationSet):
            continue
        name = alloc.memorylocations[0].name
        if alloc.kind == "ExternalInput":
            if name != partition_name:
                in_names.append(name)
        elif alloc.kind == "ExternalOutput":
            shape = tuple(alloc.tensor_shape)
            dtype = _mybir.dt.np(alloc.dtype)
            out_names.append(name)
            out_avals.append(jax.core.ShapedArray(shape, dtype))
    n_params = len(in_names)
    all_names = list(in_names) + list(out_names)
    if partition_name is not None:
        all_names.append(partition_name)
    donate = tuple(range(n_params, n_params + len(out_names)))

    def _body(*args):
        operands = list(args)
        if partition_name is not None:
            operands.append(bass2jax.partition_id_tensor())
        outs = bass2jax._bass_exec_p.bind(
            *operands,
            out_avals=tuple(out_avals),
            in_names=tuple(all_names),
            out_names=tuple(out_names),
            lowering_input_output_aliases=(),
            sim_require_finite=True,
            sim_require_nnan=True,
            nc=nc,
        )
        return tuple(outs)

    devices = jax.devices()[:NCORES]
    mesh = Mesh(np.asarray(devices), ("core",))
    n_in = n_params + len(out_names)
    jitted = jax.jit(
        shard_map(_body, mesh=mesh,
                  in_specs=(PartitionSpec("core"),) * n_in,
                  out_specs=(PartitionSpec("core"),) * len(out_names),
                  check_rep=False),
        donate_argnums=donate, keep_unused=True)
    sharding = NamedSharding(mesh, PartitionSpec("core"))
    st = {"nc": nc, "jitted": jitted, "in_names": in_names,
          "out_names": out_names, "out_avals": out_avals,
          "dbg_name": nc.dbg_addr.name if nc.dbg_addr is not None else None,
          "sharding": sharding}
    _cached["exec"] = st
    return st


def _concat_inputs(st, in_maps):
    dbg = st["dbg_name"]
    if dbg is not None:
        in_maps = [{**m, dbg: np.zeros((1, 2), np.uint32)} for m in in_maps]
    return [np.concatenate([np.asarray(m[name]) for m in in_maps], axis=0)
            for name in st["in_names"]]


def _zero_outs(st):
    return [np.zeros((NCORES * a.shape[0],) + tuple(a.shape[1:]), a.dtype)
            for a in st["out_avals"]]


def _run_cached(st, in_maps):
    out_arrs = st["jitted"](*_concat_inputs(st, in_maps), *_zero_outs(st))
    res = []
    for c in range(NCORES):
        res.append({name: np.asarray(out_arrs[i]).reshape(
            (NCORES,) + tuple(st["out_avals"][i].shape))[c]
            for i, name in enumerate(st["out_names"])})
    return res


def kernel(**inputs):
    in_maps = host_prep(**inputs)
    if _cached.get("hw_broken"):
        if "nc" not in _cached:
            _cached["nc"] = build_program()
        return host_combine(_run_sim(_cached["nc"], in_maps))
    try:
        st = _get_exec()
        return host_combine(_run_cached(st, in_maps))
    except Exception:
        try:
            if "nc" not in _cached:
                _cached["nc"] = build_program()
            res = run_bass_kernel_spmd(_cached["nc"], in_maps,
                                       core_ids=list(range(NCORES)))
            return host_combine(res.results)
        except Exception:
            _cached["hw_broken"] = True
            return host_combine(_run_sim(_cached["nc"], in_maps))


if __name__ == "__main__":
    import reference
    inp = reference.setup_inputs()
    inp = {k: np.asarray(v) for k, v in inp.items()}
    out = kernel(**inp)
    print("kernel out", out.shape, out.dtype)



# revision 40
# speedup vs baseline: 11531.9952x; 1.1409x over previous
"""Trainium2 Bass kernel for nn_DepthPrediction (multi-view stereo depth).

Strategy (8 NeuronCores, SPMD single program):
  - Shard: core k handles batch b = k//4 and depth planes [8*(k%4), 8*(k%4)+8).
  - Per (b,d): homography warp of 2 src views via on-device fp16 "patch maps"
    (132x132 positions x [dy2,dx2,c16] fp16 chunks = 128B) gathered with
    standard indirect DMAs (DynamicAP). The HW DGE honors ONE dynamic
    offset per partition (scalar_dynamic_offset), so each x column is one
    128-descriptor indirect DMA: 128 calls per (plane, view). Bilinear
    interp + cumulative cost (L2 over 16 ch) in pixel-major fp16 on DVE;
    5x5 adaptive aggregation (depth-similarity softmax x feature-similarity
    weight) in tap-major layout; per-core softmax partials over the 8
    local planes. Warp coordinates/bilinear weights are batched over all
    8 planes per view before the depth loop.
  - Engine budget: DVE does the packed-fp16 tensor math; Activation does
    affine/exp/ln/square/casts (single act table: natural_log_exp);
    Pool does the indirect gathers + memsets; SP does DMAs.
  - NOTE: custom-ucode GPSIMD ops (dma_gather/ap_gather) hard-crash this
    runtime (NRT_EXEC_UNIT_UNRECOVERABLE) — only stock-NEFF instructions
    are used here.
  - Host: trivial glue — 4x4 matrix algebra, shard/pack inputs, combine the
    4-way softmax partials per batch (log-sum-exp merge) into the output.
  - Execution: the jitted shard_map executable is built ONCE and cached;
    repeat kernel() calls reuse it (the stock per-call path retraces).

Self-contained: hardcodes all shapes from the problem spec.
"""

import numpy as np

import concourse.bacc as bacc
import concourse.bass as bass
import concourse.mybir as mybir
import concourse.tile as tile
from concourse.bass_utils import run_bass_kernel_spmd
from concourse.tile_rust import add_dep_helper

F32 = mybir.dt.float32
F16 = mybir.dt.float16
I32 = mybir.dt.int32
I16 = mybir.dt.int16

AF = mybir.ActivationFunctionType
OP = mybir.AluOpType
AX = mybir.AxisListType

# problem shapes
V, B, C, H, W, D = 3, 2, 16, 128, 128, 32
NCORES = 8
DPC = D // (NCORES // B)  # depth planes per core = 8
NV = V - 1  # src views = 2

PW = W + 4          # padded map width (x0 in [-2,129] -> cols 0..131)
POS = PW * PW       # patch positions
CH = 64             # chunk elems: [dy2,dx2,c16] fp16 = 128B
PADX = W + 4        # x-padded tiles for 5-tap aggregation
SCALE = W / (W - 1.0)  # grid_sample align_corners=False fold
ESHIFT = -9.0       # exp(|dnb-d|) stabilization shift (|delta| <= 9)
NT = 25             # aggregation taps
TM = NT * W         # tap-major volume free size
NDQ = 1             # SWDGE dynamic queues (2 measured no faster than 1)

_cached = {}


def _ap(base, off, dims):
    """Raw AP on the same tensor as `base` (an AP), offset in elements."""
    return bass.AP(base.tensor, base.offset + off, dims)


def build_program(debug=False, ablate_gather=False):
    # NOTE: do NOT reorder the get_activation_tables() dict — the emitted
    # act_func_set_id indexes the ORIGINAL act_info.json order; walrus
    # resolves ids against that file, so a reorder makes HW load the wrong
    # table (Ln evaluated off the exp table = garbage). CoreSim ignores
    # tables, which hid this.
    nc = bacc.Bacc("TRN2", target_bir_lowering=False, debug=False,
                   num_devices=NCORES, num_swdge_queues=NDQ)

    refF = nc.dram_tensor("refF", [C, H, W], F32, kind="ExternalInput")
    srcF = nc.dram_tensor("srcF", [NV, C, H, W], F32, kind="ExternalInput")
    dep = nc.dram_tensor("dep", [DPC, H, W], F32, kind="ExternalInput")
    amap = nc.dram_tensor("amap", [NV, 3, H, W], F32, kind="ExternalInput")
    tvec = nc.dram_tensor("tvec", [128, 12], F32, kind="ExternalInput")
    out3 = nc.dram_tensor("out3", [3, H, W], F32, kind="ExternalOutput")
    # internal patch maps, one per src view
    pmap = [nc.dram_tensor(f"pmap{v}", [POS, CH], F16, kind="Internal")
            for v in range(NV)]
    dbg = {}
    if debug:
        for nm, shape, dt in (
                ("dbgPos", [128, W], I32), ("dbgG", [128, W * CH], F16),
                ("dbgCum", [128, W * C], F16), ("dbgDiff", [128, W * C], F16),
                ("dbgCsq0", [128, W], F16), ("dbgCsq1", [128, W], F16),
                ("dbgCost", [128, 5 * PADX], F16),
                ("dbgCmin", [128, W], F16),
                ("dbgWf", [128, NT * W], F16),
                ("dbgNum", [128, W], F16), ("dbgDen", [128, W], F16),
                ("dbgAgg", [128, W * DPC], F32),
                ("dbgPmap", [POS, CH], F16)):
            dbg[nm] = nc.dram_tensor(nm, shape, dt, kind="ExternalOutput")

    with nc.allow_low_precision("fp16 pipeline by design"), \
            tile.TileContext(nc) as tc:
        ctx_pools = []

        def pool(name, bufs=1, **kw):
            p = tc.tile_pool(name=name, bufs=bufs, **kw)
            ctx_pools.append(p)
            return p.__enter__()

        pp = pool("persist", 1)     # long-lived tiles
        prep = tc.tile_pool(name="prep", bufs=1)
        pr = prep.__enter__()

        # ---------------- constant / persistent loads ----------------
        amapL = pp.tile([128, NV * 3 * W], F32, tag="amapL")  # [y,(v,row,x)]
        nc.sync.dma_start(out=amapL[:], in_=_ap(
            amap.ap(), 0, [[W, 128], [3 * H * W, NV], [H * W, 3], [1, W]]))
        tvecT = pp.tile([128, 12], F32, tag="tvecT")
        nc.sync.dma_start(out=tvecT[:], in_=tvec.ap())

        def tv(col):  # [128,1] per-partition scalar AP
            return tvecT[:, col:col + 1]

        # zero-source tile for edge DMAs (f16)
        ztile = pp.tile([128, PADX * C], F16, tag="ztile")
        nc.gpsimd.memset(ztile[:], 0.0)

        # ---------------- ref prep ----------------
        tr = pr.tile([128, C * W], F32, tag="tr")  # [y,(c,x)] f32
        nc.sync.dma_start(out=tr[:], in_=_ap(
            refF.ap(), 0, [[W, 128], [H * W, C], [1, W]]))
        # refC [y,(x,c)] fp16
        refC = pp.tile([128, W * C], F16, tag="refC")
        nc.vector.tensor_copy(
            out=_ap(refC[:], 0, [[W * C, 128], [C, W], [1, C]]),
            in_=_ap(tr[:], 0, [[C * W, 128], [1, W], [W, C]]))
        # refPadC [y,(xp132,c)] fp16, x' = x+2
        refPadC = pr.tile([128, PADX * C], F16, tag="refPadC")
        nc.vector.memset(
            _ap(refPadC[:], 0, [[PADX * C, 128], [C, 2], [1, C]]), 0.0)
        nc.vector.memset(
            _ap(refPadC[:], 130 * C, [[PADX * C, 128], [C, 2], [1, C]]), 0.0)
        nc.scalar.copy(out=refPadC[:, 2 * C:(2 + W) * C], in_=refC[:])
        # 4 partition-shifted copies (ty in {0,1,3,4}; center ty=2 = refPadC)
        refSC = {}
        for ty in (0, 1, 3, 4):
            t = pr.tile([128, PADX * C], F16, tag=f"refSC{ty}")
            k = ty - 2
            if k < 0:
                nc.sync.dma_start(out=t[-k:128, :], in_=refPadC[0:128 + k, :])
                nc.sync.dma_start(out=t[0:-k, :], in_=ztile[0:-k, :])
            else:
                nc.sync.dma_start(out=t[0:128 - k, :], in_=refPadC[k:128, :])
                nc.sync.dma_start(out=t[128 - k:128, :], in_=ztile[0:k, :])
            refSC[ty] = t
        refSC[2] = refPadC

        # ---------------- patch map build (per src view) ----------------
        build_dumps = []
        for v in range(NV):
            # staged [y, (xp132, dy2, dx2, c16, pad64)] fp16; row y -> pos
            # row y+2.  Zero only the data slots of border cols xp in
            # {0,1} u {129,130,131}; pad elems are never read downstream.
            staged = pr.tile([128, PW * CH], F16, tag=f"staged{v}")
            nc.vector.memset(
                _ap(staged[:], 0, [[PW * CH, 128], [CH, 2], [1, 64]]), 0.0)
            nc.vector.memset(
                _ap(staged[:], 129 * CH, [[PW * CH, 128], [CH, 3], [1, 64]]),
                0.0)
            # TF16 [y, (c,x)] fp16  (cast during DMA, SWDGE)
            tf = pr.tile([128, C * W], F16, tag="tf")
            nc.gpsimd.dma_start(
                out=tf[:], in_=_ap(srcF.ap(), v * C * H * W,
                                   [[W, 128], [H * W, C], [1, W]]))
            # down-shifted copy: tfdn[y] = tf[y+1]; row 127 = 0
            tfdn = pr.tile([128, C * W], F16, tag="tfdn")
            nc.sync.dma_start(out=tfdn[0:127, :], in_=tf[1:128, :])
            nc.sync.dma_start(out=tfdn[127:128, :],
                              in_=_ap(ztile[:], 0, [[PADX * C, 1], [1, C * W]]))

            copy_fns = [nc.vector.tensor_copy,
                        lambda out, in_: nc.scalar.copy(out=out, in_=in_),
                        nc.gpsimd.tensor_copy,
                        lambda out, in_: nc.scalar.copy(out=out, in_=in_)]
            i = 0
            for dy, src in ((0, tf), (1, tfdn)):
                for dx in (0, 1):
                    # staged[y, xp, dy, dx, c] = src[y, c, xp-2+dx]
                    xp_lo = 2 - dx
                    n_xp = 128
                    src_ap = _ap(src[:], 0,
                                 [[C * W, 128], [1, n_xp], [W, C]])
                    dst_ap = _ap(staged[:], xp_lo * CH + dy * 32 + dx * 16,
                                 [[PW * CH, 128], [CH, n_xp], [1, C]])
                    copy_fns[i](out=dst_ap, in_=src_ap)
                    i += 1
            # dump rows 2..129 of the pos grid (partition y -> row y+2)
            d1 = nc.sync.dma_start(
                out=_ap(pmap[v].ap(), 2 * PW * CH,
                        [[PW * CH, 128], [1, PW * CH]]),
                in_=staged[:])
            # zero rows 0, 130, 131 from ztile (128x132 slice covers a row)
            dz = []
            for r in (0, 130, 131):
                dz.append(nc.sync.dma_start(
                    out=_ap(pmap[v].ap(), r * PW * CH,
                            [[PW * CH, 1], [1, PW * CH]]),
                    in_=_ap(ztile[:], 0, [[PADX * C, 8], [1, PW * CH // 8]])))
            # row r=1 (y0=-1): dy=1 slots hold F row 0 = staged[0]'s dy=0
            # window (same cells, slot offset +32); rest zero
            d4z = nc.sync.dma_start(
                out=_ap(pmap[v].ap(), PW * CH, [[PW * CH, 1], [1, PW * CH]]),
                in_=_ap(ztile[:], 0, [[PADX * C, 8], [1, PW * CH // 8]]))
            d4 = nc.sync.dma_start(
                out=_ap(pmap[v].ap(), PW * CH + 32,
                        [[PW * CH, 1], [CH, PW], [1, 32]]),
                in_=_ap(staged[0:1, :], 0, [[PW * CH, 1], [CH, PW], [1, 32]]))
            add_dep_helper(d4.ins, d4z.ins, reason="row1 zero before windows")
            build_dumps.append(tuple([d1, d4, d4z] + dz))
            if debug and v == 0:
                dp = nc.sync.dma_start(
                    out=_ap(dbg["dbgPmap"].ap(), 0,
                            [[PW * CH, PW], [1, PW * CH]]),
                    in_=_ap(pmap[0].ap(), 0, [[PW * CH, PW], [1, PW * CH]]))
                for dma_i in build_dumps[0]:
                    add_dep_helper(dp.ins, dma_i.ins,
                                   reason="pmap build before debug dump")

        # ---------------- w_feat volume [y, (t25, x)] fp16 ----
        wf25 = pp.tile([128, TM], F16, tag="wf25")
        nc.vector.memset(wf25[:, 12 * W:13 * W], 0.0)
        for ty in (2, 1, 3, 0, 4):
            for tx in range(5):
                t = ty * 5 + tx
                if t == 12:
                    continue
                # diff = ref[y,x,c] - refSC[ty][y, x+tx, c]
                dfw = pr.tile([128, W * C], F16, tag="wfdf")
                nc.vector.tensor_tensor(
                    out=dfw[:],
                    in0=_ap(refSC[ty][:], tx * C,
                            [[PADX * C, 128], [C, W], [1, C]]),
                    in1=refC[:], op=OP.subtract)
                sqw = pr.tile([128, W * C], F16, tag="wfsq")
                nc.scalar.activation(out=sqw[:], in_=dfw[:], func=AF.Square)
                # c16 tree reduction (packed fp16 halves)
                c8 = pr.tile([128, W * 8], F16, tag="c8w")
                nc.vector.tensor_tensor(
                    out=_ap(c8[:], 0, [[W * 8, 128], [8, W], [1, 8]]),
                    in0=_ap(sqw[:], 0, [[W * C, 128], [C, W], [1, 8]]),
                    in1=_ap(sqw[:], 8, [[W * C, 128], [C, W], [1, 8]]),
                    op=OP.add)
                c4 = pr.tile([128, W * 4], F16, tag="c4w")
                nc.vector.tensor_tensor(
                    out=_ap(c4[:], 0, [[W * 4, 128], [4, W], [1, 4]]),
                    in0=_ap(c8[:], 0, [[W * 8, 128], [8, W], [1, 4]]),
                    in1=_ap(c8[:], 4, [[W * 8, 128], [8, W], [1, 4]]),
                    op=OP.add)
                c2 = pr.tile([128, W * 2], F16, tag="c2w")
                nc.vector.tensor_tensor(
                    out=_ap(c2[:], 0, [[W * 2, 128], [2, W], [1, 2]]),
                    in0=_ap(c4[:], 0, [[W * 4, 128], [4, W], [1, 2]]),
                    in1=_ap(c4[:], 2, [[W * 4, 128], [4, W], [1, 2]]),
                    op=OP.add)
                cs = pr.tile([128, W], F16, tag="csw")
                nc.vector.tensor_tensor(
                    out=cs[:],
                    in0=_ap(c2[:], 0, [[W * 2, 128], [2, W]]),
                    in1=_ap(c2[:], 1, [[W * 2, 128], [2, W]]),
                    op=OP.add)
                # wf = sqrt(cs)  (direct Sqrt act; Ln is table-hostile here)
                nc.scalar.activation(out=wf25[:, t * W:(t + 1) * W],
                                     in_=cs[:], func=AF.Sqrt)


        prep.__exit__(None, None, None)

        # ---------------- batched warp coordinates (all planes) ----------
        # depB [y, (d, x)] f32; posIv / wdB computed once per view for all
        # DPC planes -> the per-(d,v) loop only gathers + interpolates.
        depB = pp.tile([128, DPC * W], F32, tag="depB")
        nc.sync.dma_start(out=depB[:], in_=_ap(
            dep.ap(), 0, [[W, 128], [H * W, DPC], [1, W]]))
        posIv = [pp.tile([128, DPC * W], I32, tag=f"posIv{v}",
                         name=f"posIv{v}") for v in range(NV)]
        wdB = [{}, {}]
        for v in range(NV):
            for tnm in ("00", "01", "10", "11"):
                wdB[v][tnm] = pp.tile([128, DPC * W * 2], F16,
                                      tag=f"wdB{v}{tnm}",
                                      name=f"wdB{v}{tnm}")

        cprep = tc.tile_pool(name="coord", bufs=1)
        cq = cprep.__enter__()
        DW = DPC * W
        for v in range(NV):
            def arowB(r):
                return _ap(amapL[:], (v * 3 + r) * W,
                           [[NV * 3 * W, 128], [0, DPC], [1, W]])
            mx = cq.tile([128, DW], F32, tag="mxB")
            my = cq.tile([128, DW], F32, tag="myB")
            dn = cq.tile([128, DW], F32, tag="dnB")
            nc.vector.tensor_tensor(out=mx[:], in0=arowB(0), in1=depB[:],
                                    op=OP.mult)
            nc.vector.tensor_tensor(out=my[:], in0=arowB(1), in1=depB[:],
                                    op=OP.mult)
            nc.vector.tensor_tensor(out=dn[:], in0=arowB(2), in1=depB[:],
                                    op=OP.mult)
            nx = cq.tile([128, DW], F32, tag="nxB")
            ny = cq.tile([128, DW], F32, tag="nyB")
            dnt = cq.tile([128, DW], F32, tag="dntB")
            nc.scalar.activation(out=nx[:], in_=mx[:], func=AF.Identity,
                                 bias=tv(v * 3 + 0))
            nc.scalar.activation(out=ny[:], in_=my[:], func=AF.Identity,
                                 bias=tv(v * 3 + 1))
            nc.scalar.activation(out=dnt[:], in_=dn[:], func=AF.Identity,
                                 bias=tv(v * 3 + 2))
            rec = cq.tile([128, DW], F32, tag="recB")
            nc.vector.reciprocal(out=rec[:], in_=dnt[:])
            gx = cq.tile([128, DW], F32, tag="gxB")
            gy = cq.tile([128, DW], F32, tag="gyB")
            nc.vector.tensor_tensor(out=gx[:], in0=nx[:], in1=rec[:],
                                    op=OP.mult)
            nc.vector.tensor_tensor(out=gy[:], in0=ny[:], in1=rec[:],
                                    op=OP.mult)

            # floor(g + 1.5) exact under round-to-nearest-even OR trunc
            def floorfracB(g, nm):
                xi = cq.tile([128, DW], I32, tag=f"iB{nm}")
                nc.scalar.activation(out=xi[:], in_=g[:],
                                     func=AF.Identity, bias=tv(6))
                xf = cq.tile([128, DW], F32, tag=f"xfB{nm}")
                nc.scalar.copy(out=xf[:], in_=xi[:])
                fr = cq.tile([128, DW], F32, tag=f"frB{nm}")
                nc.vector.scalar_tensor_tensor(
                    out=fr[:], in0=g[:], scalar=1.5, in1=xf[:],
                    op0=OP.add, op1=OP.subtract)
                neg = cq.tile([128, DW], F32, tag=f"ngB{nm}")
                nc.vector.tensor_scalar(out=neg[:], in0=fr[:],
                                        scalar1=0.0, scalar2=None,
                                        op0=OP.is_lt)
                nc.vector.tensor_tensor(out=fr[:], in0=fr[:],
                                        in1=neg[:], op=OP.add)
                xa = cq.tile([128, DW], F32, tag=f"xaB{nm}")
                nc.vector.tensor_tensor(out=xa[:], in0=xf[:],
                                        in1=neg[:], op=OP.subtract)
                xc = cq.tile([128, DW], F32, tag=f"xcB{nm}")
                nc.vector.tensor_scalar(out=xc[:], in0=xa[:],
                                        scalar1=131.0, scalar2=0.0,
                                        op0=OP.min, op1=OP.max)
                return xc, fr

            xsC, fx = floorfracB(gx, "x")
            ysC, fy = floorfracB(gy, "y")
            posF = cq.tile([128, DW], F32, tag="posFB")
            nc.vector.scalar_tensor_tensor(
                out=posF[:], in0=ysC[:], scalar=float(PW), in1=xsC[:],
                op0=OP.mult, op1=OP.add)
            nc.scalar.copy(out=posIv[v][:], in_=posF[:])

            # fp16 fractional weights + bilinear tap weights, batched
            fx16 = cq.tile([128, DW], F16, tag="fx16B")
            fy16 = cq.tile([128, DW], F16, tag="fy16B")
            fxm16 = cq.tile([128, DW], F16, tag="fxm16B")
            fym16 = cq.tile([128, DW], F16, tag="fym16B")
            nc.scalar.copy(out=fx16[:], in_=fx[:])
            nc.scalar.copy(out=fy16[:], in_=fy[:])
            nc.scalar.activation(out=fxm16[:], in_=fx[:],
                                 func=AF.Identity, scale=tv(9), bias=tv(8))
            nc.scalar.activation(out=fym16[:], in_=fy[:],
                                 func=AF.Identity, scale=tv(9), bias=tv(8))
            for (tnm, fa, fb) in (("00", fxm16, fym16), ("01", fx16, fym16),
                                  ("10", fxm16, fy16), ("11", fx16, fy16)):
                nc.vector.tensor_tensor(
                    out=_ap(wdB[v][tnm][:], 0,
                            [[DW * 2, 128], [2, DW], [1, 2]]),
                    in0=_ap(fa[:], 0, [[DW, 128], [1, DW], [0, 2]]),
                    in1=_ap(fb[:], 0, [[DW, 128], [1, DW], [0, 2]]),
                    op=OP.mult)
        cprep.__exit__(None, None, None)

        wp = pool("work", 2)        # small per-(b,d) working tiles
        bp = pool("big4", 4)        # shared 4KB scratch (tag s4k)
        cp = pool("cumdif", 1)      # cum/diff accumulators
        gp = pool("gath", 2)        # gather destinations
        ap_ = pool("aggp", 2)       # agg volumes

        # agg store
        aggT = pp.tile([128, W * DPC], F32, tag="aggT")

        # ---------------- per depth-plane pipeline ----------------
        for di in range(DPC):
            def depf_ap():
                return _ap(depB[:], di * W, [[DPC * W, 128], [1, W]])
            # depSC5 [y, (ty5, xp132)] fp16: center block + 4 shifted blocks
            depSC5 = wp.tile([128, 5 * PADX], F16, tag="depSC5")
            nc.scalar.copy(
                out=_ap(depSC5[:], 2 * PADX + 2, [[5 * PADX, 128], [1, W]]),
                in_=depf_ap())
            nc.vector.memset(
                _ap(depSC5[:], 2 * PADX, [[5 * PADX, 128], [130, 2], [1, 2]]),
                0.0)
            for ty in (0, 1, 3, 4):
                k = ty - 2
                dlo = ty * PADX
                clo = 2 * PADX
                if k < 0:
                    nc.sync.dma_start(
                        out=depSC5[-k:128, dlo:dlo + PADX],
                        in_=depSC5[0:128 + k, clo:clo + PADX])
                    nc.sync.dma_start(out=depSC5[0:-k, dlo:dlo + PADX],
                                      in_=ztile[0:-k, 0:PADX])
                else:
                    nc.sync.dma_start(
                        out=depSC5[0:128 - k, dlo:dlo + PADX],
                        in_=depSC5[k:128, clo:clo + PADX])
                    nc.sync.dma_start(out=depSC5[128 - k:128, dlo:dlo + PADX],
                                      in_=ztile[0:k, 0:PADX])

            cum = cp.tile([128, W * C], F16, tag="cum")
            diff = cp.tile([128, W * C], F16, tag="diff")
            csq = {}
            for v in range(NV):
                # ---- gather: HW DynamicAP honors ONE offset per partition
                # (scalar_dynamic_offset DGE level; vector offsets are
                # compiled out), so issue one 128-descriptor indirect DMA
                # per x column: idx [128,1] -> dest [128, CH].
                G = gp.tile([128, W * CH], F16, tag="G")
                if ablate_gather:
                    nc.vector.memset(G[:], 0.25)
                else:
                    prev = [None] * NDQ
                    for j in range(W):
                        q = j % NDQ
                        gi = nc.gpsimd.indirect_dma_start(
                            out=_ap(G[:], j * CH, [[W * CH, 128], [1, CH]]),
                            out_offset=None,
                            in_=pmap[v].ap(),
                            in_offset=bass.IndirectOffsetOnAxis(
                                ap=posIv[v][:, di * W + j:di * W + j + 1],
                                axis=0))
                        if q:
                            gi.ins.queue = f"qPoolDynamic{q}"
                        if prev[q] is None:
                            for dma_i in build_dumps[v]:
                                add_dep_helper(
                                    gi.ins, dma_i.ins,
                                    reason="pmap build before gather")
                        else:
                            add_dep_helper(gi.ins, prev[q].ins,
                                           reason="gather chain (same queue)")
                        prev[q] = gi
                if debug and di == 0 and v == 0:
                    nc.sync.dma_start(
                        out=dbg["dbgPos"].ap(),
                        in_=_ap(posIv[0][:], 0, [[DPC * W, 128], [1, W]]))
                    nc.sync.dma_start(out=dbg["dbgG"].ap(), in_=G[:])

                # ---- bilinear taps: acc = sum_t w_t * G_t  [y,(x,c)] ----
                def tap(dy, dx):
                    return _ap(G[:], (dy * 2 + dx) * 16,
                               [[W * CH, 128], [CH, W], [2, 8], [1, 2]])

                def wap(tnm):
                    return _ap(wdB[v][tnm][:], di * W * 2,
                               [[DPC * W * 2, 128], [2, W], [0, 8], [1, 2]])

                cview = [[2048, 128], [16, W], [2, 8], [1, 2]]
                acc = cum if v == 0 else bp.tile([128, W * C], F16, tag="s4k")
                p0 = bp.tile([128, W * C], F16, tag="s4k")
                p1 = bp.tile([128, W * C], F16, tag="s4k")
                nc.vector.tensor_tensor(out=_ap(acc[:], 0, cview),
                                        in0=tap(0, 0), in1=wap("00"),
                                        op=OP.mult)
                nc.vector.tensor_tensor(out=_ap(p0[:], 0, cview),
                                        in0=tap(0, 1), in1=wap("01"),
                                        op=OP.mult)
                nc.vector.tensor_tensor(out=_ap(p1[:], 0, cview),
                                        in0=tap(1, 0), in1=wap("10"),
                                        op=OP.mult)
                nc.vector.tensor_tensor(out=acc[:], in0=acc[:], in1=p0[:],
                                        op=OP.add)
                nc.vector.tensor_tensor(out=_ap(p0[:], 0, cview),
                                        in0=tap(1, 1), in1=wap("11"),
                                        op=OP.mult)
                nc.vector.tensor_tensor(out=p1[:], in0=p1[:], in1=p0[:],
                                        op=OP.add)
                nc.vector.tensor_tensor(out=acc[:], in0=acc[:], in1=p1[:],
                                        op=OP.add)

                # ---- cost_v = sum_c (ref - cum)^2 (Act square + DVE tree) --
                if v == 0:
                    nc.vector.tensor_tensor(out=diff[:], in0=refC[:],
                                            in1=cum[:], op=OP.subtract)
                else:
                    nc.vector.tensor_tensor(out=diff[:], in0=diff[:],
                                            in1=acc[:], op=OP.subtract)
                sq = bp.tile([128, W * C], F16, tag="s4k")
                nc.scalar.activation(out=sq[:], in_=diff[:], func=AF.Square)
                c8 = wp.tile([128, W * 8], F16, tag="c8")
                nc.vector.tensor_tensor(
                    out=_ap(c8[:], 0, [[W * 8, 128], [8, W], [1, 8]]),
                    in0=_ap(sq[:], 0, [[W * C, 128], [C, W], [1, 8]]),
                    in1=_ap(sq[:], 8, [[W * C, 128], [C, W], [1, 8]]),
                    op=OP.add)
                c4 = wp.tile([128, W * 4], F16, tag="c4")
                nc.vector.tensor_tensor(
                    out=_ap(c4[:], 0, [[W * 4, 128], [4, W], [1, 4]]),
                    in0=_ap(c8[:], 0, [[W * 8, 128], [8, W], [1, 4]]),
                    in1=_ap(c8[:], 4, [[W * 8, 128], [8, W], [1, 4]]),
                    op=OP.add)
                c2 = wp.tile([128, W * 2], F16, tag="c2")
                nc.vector.tensor_tensor(
                    out=_ap(c2[:], 0, [[W * 2, 128], [2, W], [1, 2]]),
                    in0=_ap(c4[:], 0, [[W * 4, 128], [4, W], [1, 2]]),
                    in1=_ap(c4[:], 2, [[W * 4, 128], [4, W], [1, 2]]),
                    op=OP.add)
                cs = wp.tile([128, W], F16, tag=f"csq{v}")
                nc.vector.tensor_tensor(
                    out=cs[:],
                    in0=_ap(c2[:], 0, [[W * 2, 128], [2, W]]),
                    in1=_ap(c2[:], 1, [[W * 2, 128], [2, W]]),
                    op=OP.add)
                csq[v] = cs
                if debug and di == 0:
                    if v == 0:
                        nc.sync.dma_start(out=dbg["dbgCum"].ap(), in_=cum[:])
                        nc.sync.dma_start(out=dbg["dbgCsq0"].ap(), in_=cs[:])
                    else:
                        nc.sync.dma_start(out=dbg["dbgDiff"].ap(), in_=diff[:])
                        nc.sync.dma_start(out=dbg["dbgCsq1"].ap(), in_=cs[:])

            # cost = sqrt(min(c1sq, c2sq)) into costSC5 center block
            cmin = wp.tile([128, W], F16, tag="cmin")
            nc.vector.tensor_tensor(out=cmin[:], in0=csq[0][:], in1=csq[1][:],
                                    op=OP.min)
            if debug and di == 0:
                nc.sync.dma_start(out=dbg["dbgCmin"].ap(), in_=cmin[:])
            costSC5 = wp.tile([128, 5 * PADX], F16, tag="costSC5")
            nc.scalar.activation(
                out=_ap(costSC5[:], 2 * PADX + 2, [[5 * PADX, 128], [1, W]]),
                in_=cmin[:], func=AF.Sqrt)
            nc.vector.memset(
                _ap(costSC5[:], 2 * PADX, [[5 * PADX, 128], [130, 2], [1, 2]]),
                0.0)
            for ty in (0, 1, 3, 4):
                k = ty - 2
                dlo = ty * PADX
                clo = 2 * PADX
                if k < 0:
                    nc.sync.dma_start(
                        out=costSC5[-k:128, dlo:dlo + PADX],
                        in_=costSC5[0:128 + k, clo:clo + PADX])
                    nc.sync.dma_start(out=costSC5[0:-k, dlo:dlo + PADX],
                                      in_=ztile[0:-k, 0:PADX])
                else:
                    nc.sync.dma_start(
                        out=costSC5[0:128 - k, dlo:dlo + PADX],
                        in_=costSC5[k:128, clo:clo + PADX])
                    nc.sync.dma_start(out=costSC5[128 - k:128, dlo:dlo + PADX],
                                      in_=ztile[0:k, 0:PADX])

            # ---- aggregation, tap-major [y, (t25, x)] ----
            dvol = ap_.tile([128, TM], F16, tag="dvol")
            nc.vector.tensor_tensor(
                out=_ap(dvol[:], 0, [[TM, 128], [5 * W, 5], [W, 5], [1, W]]),
                in0=_ap(depSC5[:], 0,
                        [[5 * PADX, 128], [PADX, 5], [1, 5], [1, W]]),
                in1=_ap(depSC5[:], 2 * PADX + 2,
                        [[5 * PADX, 128], [0, 5], [0, 5], [1, W]]),
                op=OP.subtract)
            nc.scalar.activation(out=dvol[:], in_=dvol[:], func=AF.Abs)
            evol = ap_.tile([128, TM], F16, tag="evol")
            nc.scalar.activation(out=evol[:], in_=dvol[:], func=AF.Exp,
                                 bias=tv(7))
            uvol = dvol  # reuse: dvol's last reader is the exp above
            nc.vector.tensor_tensor(out=uvol[:], in0=evol[:], in1=wf25[:],
                                    op=OP.mult)
            nc.vector.tensor_tensor(
                out=_ap(uvol[:], 0, [[TM, 128], [5 * W, 5], [W, 5], [1, W]]),
                in0=_ap(uvol[:], 0, [[TM, 128], [5 * W, 5], [W, 5], [1, W]]),
                in1=_ap(costSC5[:], 0,
                        [[5 * PADX, 128], [PADX, 5], [1, 5], [1, W]]),
                op=OP.mult)

            def tap_tree(vol, nm):
                s12 = wp.tile([128, 12 * W], F16, tag="s12")
                nc.vector.tensor_tensor(out=s12[:], in0=vol[:, 0:12 * W],
                                        in1=vol[:, 12 * W:24 * W], op=OP.add)
                s6 = wp.tile([128, 6 * W], F16, tag="s6")
                nc.vector.tensor_tensor(out=s6[:], in0=s12[:, 0:6 * W],
                                        in1=s12[:, 6 * W:12 * W], op=OP.add)
                s3 = wp.tile([128, 3 * W], F16, tag="s3")
                nc.vector.tensor_tensor(out=s3[:], in0=s6[:, 0:3 * W],
                                        in1=s6[:, 3 * W:6 * W], op=OP.add)
                r = wp.tile([128, W], F16, tag=f"r{nm}")
                nc.vector.tensor_tensor(out=r[:], in0=s3[:, 0:W],
                                        in1=s3[:, W:2 * W], op=OP.add)
                nc.vector.tensor_tensor(out=r[:], in0=r[:],
                                        in1=s3[:, 2 * W:3 * W], op=OP.add)
                nc.vector.tensor_tensor(out=r[:], in0=r[:],
                                        in1=vol[:, 24 * W:25 * W], op=OP.add)
                return r

            num = tap_tree(uvol, "n")
            den = tap_tree(evol, "d")
            if debug and di == 0:
                nc.sync.dma_start(out=dbg["dbgCost"].ap(), in_=costSC5[:])
                nc.sync.dma_start(out=dbg["dbgWf"].ap(), in_=wf25[:])
                nc.sync.dma_start(out=dbg["dbgNum"].ap(), in_=num[:])
                nc.sync.dma_start(out=dbg["dbgDen"].ap(), in_=den[:])
            rden = wp.tile([128, W], F32, tag="rden")
            nc.vector.reciprocal(out=rden[:], in_=den[:])
            agg_ap = _ap(aggT[:], di, [[W * DPC, 128], [DPC, W], [1, 1]])
            nc.vector.tensor_tensor(out=agg_ap, in0=num[:], in1=rden[:],
                                    op=OP.mult)

        # ---------------- per-core softmax partials ----------------
        def aggap(di):
            return _ap(aggT[:], di, [[W * DPC, 128], [DPC, W], [1, 1]])

        def depap(di):
            return _ap(depB[:], di * W, [[DPC * W, 128], [1, W]])

        if debug:
            nc.sync.dma_start(out=dbg["dbgAgg"].ap(), in_=aggT[:])
        m = pp.tile([128, W], F32, tag="m")
        nc.vector.tensor_tensor(out=m[:], in0=aggap(0), in1=aggap(1),
                                op=OP.max)
        for di in range(2, DPC):
            nc.vector.tensor_tensor(out=m[:], in0=m[:], in1=aggap(di),
                                    op=OP.max)
        s0 = pp.tile([128, W], F32, tag="s0")
        s1 = pp.tile([128, W], F32, tag="s1")
        for di in range(DPC):
            t = wp.tile([128, W], F32, tag="et")
            nc.vector.tensor_tensor(out=t[:], in0=aggap(di), in1=m[:],
                                    op=OP.subtract)
            e = wp.tile([128, W], F32, tag="ee")
            nc.scalar.activation(out=e[:], in_=t[:], func=AF.Exp)
            t1 = wp.tile([128, W], F32, tag="t1")
            nc.vector.tensor_tensor(out=t1[:], in0=e[:], in1=depap(di),
                                    op=OP.mult)
            if di == 0:
                nc.vector.tensor_copy(out=s0[:], in_=e[:])
                nc.vector.tensor_copy(out=s1[:], in_=t1[:])
            else:
                nc.vector.tensor_tensor(out=s0[:], in0=s0[:], in1=e[:],
                                        op=OP.add)
                nc.vector.tensor_tensor(out=s1[:], in0=s1[:], in1=t1[:],
                                        op=OP.add)
        nc.sync.dma_start(out=out3.ap()[0], in_=m[:])
        nc.sync.dma_start(out=out3.ap()[1], in_=s0[:])
        nc.sync.dma_start(out=out3.ap()[2], in_=s1[:])

        for p in reversed(ctx_pools):
            p.__exit__(None, None, None)

    nc.compile()
    return nc


def host_prep(features, intrinsics, cam_to_world, depth_hypo):
    """Build the 8 per-core input maps. All O(small) except slicing."""
    features = np.asarray(features, dtype=np.float32)
    intrinsics = np.asarray(intrinsics, dtype=np.float32)
    cam_to_world = np.asarray(cam_to_world, dtype=np.float32)
    depth_hypo = np.asarray(depth_hypo, dtype=np.float32)

    ys, xs = np.meshgrid(np.arange(H, dtype=np.float32),
                         np.arange(W, dtype=np.float32), indexing="ij")
    in_maps = []
    for k in range(NCORES):
        b = k // (NCORES // B)
        dlo = DPC * (k % (NCORES // B))
        amap = np.zeros((NV, 3, H, W), np.float32)
        tvv = np.zeros((12,), np.float32)
        for vi in range(1, V):
            src_w2c = np.linalg.inv(cam_to_world[vi, b])
            ref_w2c = np.linalg.inv(cam_to_world[0, b])
            src_KK = src_w2c.copy()
            src_KK[:3, :3] = intrinsics[vi, b]
            ref_KK = ref_w2c.copy()
            ref_KK[:3, :3] = intrinsics[0, b]
            proj = (src_KK @ src_w2c) @ np.linalg.inv(ref_KK @ ref_w2c)
            rot, trans = proj[:3, :3], proj[:3, 3]
            A = (rot[:, 0:1, None] * xs[None] + rot[:, 1:2, None] * ys[None]
                 + rot[:, 2:3, None])  # [3, H, W]
            v = vi - 1
            amap[v, 0] = A[0] * SCALE
            amap[v, 1] = A[1] * SCALE
            amap[v, 2] = A[2]
            tvv[v * 3 + 0] = trans[0] * SCALE
            tvv[v * 3 + 1] = trans[1] * SCALE
            tvv[v * 3 + 2] = trans[2]
        tvv[6] = 1.5      # floor-by-trunc bias (+2 border shift - 0.5 center)
        tvv[7] = ESHIFT
        tvv[8] = 1.0
        tvv[9] = -1.0
        tvv[10] = 0.5
        in_maps.append({
            "refF": np.ascontiguousarray(features[0, b].transpose(0, 2, 1)),
            "srcF": np.ascontiguousarray(features[1:, b]),
            "dep": np.ascontiguousarray(
                depth_hypo[b, dlo:dlo + DPC].transpose(0, 2, 1)),
            "amap": np.ascontiguousarray(amap.transpose(0, 1, 3, 2)),
            "tvec": np.tile(tvv[None, :], (128, 1)).astype(np.float32),
        })
    return in_maps


def host_combine(results):
    """Merge per-core softmax partials (m, s0, s1) into [B, H, W]."""
    out = np.zeros((B, H, W), np.float32)
    per_b = NCORES // B
    for b in range(B):
        parts = [np.asarray(results[b * per_b + j]["out3"]) for j in range(per_b)]
        parts = [p.transpose(0, 2, 1) for p in parts]  # [3, x, y] -> [3, y, x]
        ms = np.stack([p[0] for p in parts])         # [4, H, W]
        M = ms.max(axis=0)
        S0 = np.zeros((H, W), np.float64)
        S1 = np.zeros((H, W), np.float64)
        for p in parts:
            w = np.exp(p[0] - M)
            S0 += w * p[1]
            S1 += w * p[2]
        out[b] = (S1 / S0).astype(np.float32)
    return out


def _run_sim(nc, in_maps):
    from concourse.bass_interp import CoreSim
    results = []
    for core in range(NCORES):
        sim = CoreSim(nc, trace=False, publish_trace=False)
        for k, v in in_maps[core].items():
            sim.tensor(k)[:] = v
        sim.simulate()
        results.append({"out3": np.array(sim.tensor("out3"))})
    return results


def _get_exec():
    """Build the Bass program once and wrap it in a CACHED jitted
    shard_map executable (the stock run_bass_kernel_spmd path rebuilds
    the jax closure per call -> full retrace + compile every time)."""
    if "exec" in _cached:
        return _cached["exec"]
    import jax
    import concourse.mybir as _mybir
    from concourse import bass2jax
    from jax.experimental.shard_map import shard_map
    from jax.sharding import Mesh, NamedSharding, PartitionSpec

    nc = build_program()
    bass2jax.install_neuronx_cc_hook()

    partition_name = (nc.partition_id_tensor.name
                      if nc.partition_id_tensor else None)
    in_names, out_names, out_avals = [], [], []
    for alloc in nc.m.functions[0].allocations:
        if not isinstance(alloc, _mybir.MemoryLocationSet):
            continue
        name = alloc.memorylocations[0].name
        if alloc.kind == "ExternalInput":
            if name != partition_name:
                in_names.append(name)
        elif alloc.kind == "ExternalOutput":
            shape = tuple(alloc.tensor_shape)
            dtype = _mybir.dt.np(alloc.dtype)
            out_names.append(name)
            out_avals.append(jax.core.ShapedArray(shape, dtype))
    n_params = len(in_names)
    all_names = list(in_names) + list(out_names)
    if partition_name is not None:
        all_names.append(partition_name)
    donate = tuple(range(n_params, n_params + len(out_names)))

    def _body(*args):
        operands = list(args)
        if partition_name is not None:
            operands.append(bass2jax.partition_id_tensor())
        outs = bass2jax._bass_exec_p.bind(
            *operands,
            out_avals=tuple(out_avals),
            in_names=tuple(all_names),
            out_names=tuple(out_names),
            lowering_input_output_aliases=(),
            sim_require_finite=True,
            sim_require_nnan=True,
            nc=nc,
        )
        return tuple(outs)

    devices = jax.devices()[:NCORES]
    mesh = Mesh(np.asarray(devices), ("core",))
    n_in = n_params + len(out_names)
    jitted = jax.jit(
        shard_map(_body, mesh=mesh,
                  in_specs=(PartitionSpec("core"),) * n_in,
                  out_specs=(PartitionSpec("core"),) * len(out_names),
                  check_rep=False),
        donate_argnums=donate, keep_unused=True)
    sharding = NamedSharding(mesh, PartitionSpec("core"))
    st = {"nc": nc, "jitted": jitted, "in_names": in_names,
          "out_names": out_names, "out_avals": out_avals,
          "dbg_name": nc.dbg_addr.name if nc.dbg_addr is not None else None,
          "sharding": sharding}
    _cached["exec"] = st
    return st


def _concat_inputs(st, in_maps):
    dbg = st["dbg_name"]
    if dbg is not None:
        in_maps = [{**m, dbg: np.zeros((1, 2), np.uint32)} for m in in_maps]
    return [np.concatenate([np.asarray(m[name]) for m in in_maps], axis=0)
            for name in st["in_names"]]


def _zero_outs(st):
    return [np.zeros((NCORES * a.shape[0],) + tuple(a.shape[1:]), a.dtype)
            for a in st["out_avals"]]


def _run_cached(st, in_maps):
    out_arrs = st["jitted"](*_concat_inputs(st, in_maps), *_zero_outs(st))
    res = []
    for c in range(NCORES):
        res.append({name: np.asarray(out_arrs[i]).reshape(
            (NCORES,) + tuple(st["out_avals"][i].shape))[c]
            for i, name in enumerate(st["out_names"])})
    return res


def kernel(**inputs):
    in_maps = host_prep(**inputs)
    # HW first; transient failures (e.g. a desynced terminal) do NOT latch
    # — each call retries the fast path so a recovered device is used again.
    if not _cached.get("hw_broken"):
        try:
            st = _get_exec()
            return host_combine(_run_cached(st, in_maps))
        except Exception:
            pass
        try:
            if "nc" not in _cached:
                _cached["nc"] = build_program()
            res = run_bass_kernel_spmd(_cached["nc"], in_maps,
                                       core_ids=list(range(NCORES)))
            return host_combine(res.results)
        except Exception:
            if "exec" not in _cached:
                # never reached HW even once: stop re-paying the attempts
                _cached["hw_broken"] = True
    if "nc" not in _cached:
        _cached["nc"] = build_program()
    return host_combine(_run_sim(_cached["nc"], in_maps))


if __name__ == "__main__":
    import reference
    inp = reference.setup_inputs()
    inp = {k: np.asarray(v) for k, v in inp.items()}
    out = kernel(**inp)
    print("kernel out", out.shape, out.dtype)

